# revision 21
# baseline (speedup 1.0000x reference)
"""GQA attention block (B=1, S=2048, D=2048, H=32, G=8, HD=64) on 8 trn2 cores.

Sharding: tensor-parallel over heads/KV-groups. Core c owns q-heads
4c..4c+3 and KV group c. Wq/Wk/Wv column-parallel, Wo row-parallel;
each core computes a partial [S, D] output, host sums the 8 partials.

Per-core dataflow (all matmuls bf16, stats f32), creation-ordered for
engine overlap (the tile scheduler keeps per-engine programs roughly in
creation order, so independent work is interleaved at emission time):
  DMAs interleaved per d-tile and split over both HWDGE queues
  (SP + ACT) so x streams in at 2x; wqkv|wg merged into one tensor.
  D (gate proj, [e,s] layout): 4 sweeps of (p, q-pair); first two fill
    the x-DMA window, last two fill later PE gaps. gus = tanh(g/2).
  A: qkv proj per s-tile -> psum [s,384]; ACT: square + raw-qk copy;
    DVE: v copy + block row-sums into one batched stats tile.
  B: one batched Newton-rsqrt chain over all stats (no per-tile chain).
  C: norm-mul + rope split over gpsimd and DVE + PE transposes to
    [hd,s]; psum->sbuf copies split over ACT and DVE.
  E: per (q-slice, head-pair): software-pipelined over k-tiles —
    scores for tile jk are emitted before the ctx matmuls of jk-1, so
    the in-order PE never waits on the exp. Causal mask by accumulating
    a -30000 strict-lower-tri matmul on diagonal blocks; probs =
    exp(scale*scores) on ACT (c0-trimmed); ctxT (+den row) =
    [v|1].T @ probsT with trimmed widths. Per head: den recip (bf16,
    low-precision ok) -> PE broadcast; gud = (tanh+1)*(0.5/den) via one
    scalar_tensor_tensor; ctxg = ctx * gud.
  F: out[s,dout] = ctxg.T @ woT per q-slice, psum->bf16 copies
    alternating ACT/DVE, partials summed on host.
"""

import numpy as np
import ml_dtypes

import concourse.bass as bass
import concourse.tile as tile
from concourse import bacc, mybir
from concourse.bass_utils import run_bass_kernel_spmd
from concourse.masks import make_identity

BF16 = mybir.dt.bfloat16
F32 = mybir.dt.float32
NBF = ml_dtypes.bfloat16

S = 2048
D = 2048
H = 32
G = 8
HD = 64
NCORE = 8
NHL = H // NCORE          # 4 q heads per core
EL = NHL * HD             # 256 local q (and gate, and ctx) features
QK = EL + HD              # 320: q + k features
QKV = QK + HD             # 384: q + k + v
QG = QKV + EL             # 640: qkv + gate columns in the merged weight
P = 128
NS = S // P               # 16 s-tiles
ND = D // P               # 16 d-tiles
SQ = 512
NSQ = S // SQ             # 4 sq slices
NB = QK // HD             # 5 (hd,) blocks in the q|k strip
SCALE = HD ** -0.5
EPS = 1e-6
NEG = -30000.0


def _v(ap, dims, extra_offset=0):
    """Reshape the free dims of a 2D AP into `dims` ([step, count] pairs),
    keeping the partition dim."""
    return bass.AP(
        tensor=ap.tensor,
        offset=ap.offset + extra_offset,
        ap=[list(ap.ap[0])] + [list(d) for d in dims],
    )


def _mk(pool, shape, dtype, tag):
    return pool.tile(shape, dtype, tag=tag, name=tag)


PHASES = []  # (phase_name, first_instruction_index) — debug aid for sim.py


def _mark(nc, name):
    PHASES.append((name, int(nc.get_next_instruction_name().split("-")[1])))


def build_nc():
    nc = bacc.Bacc("TRN2", target_bir_lowering=False, debug=False,
                   num_devices=NCORE)

    xw = nc.dram_tensor("xw", [D, S + QG], BF16, kind="ExternalInput").ap()
    wo = nc.dram_tensor("wo", [EL, D], BF16, kind="ExternalInput").ap()
    cs = nc.dram_tensor("cs", [S, 2 * QK], BF16, kind="ExternalInput").ap()
    mtri = nc.dram_tensor("mtri", [P, P], BF16, kind="ExternalInput").ap()
    out = nc.dram_tensor("out", [S, D], BF16, kind="ExternalOutput").ap()

    with tile.TileContext(nc) as tc:
        with (
            tc.tile_pool(name="persist", bufs=1) as pp,
            tc.tile_pool(name="work", bufs=3) as wp,
            tc.tile_pool(name="stats", bufs=2) as sp,
            tc.tile_pool(name="probs", bufs=8) as prp,
            tc.tile_pool(name="outc", bufs=4) as ocp,
            tc.tile_pool(name="psum", bufs=4, space="PSUM") as psp,
            tc.tile_pool(name="psum2", bufs=2, space="PSUM") as psp2,
        ):
            # ---- persistent loads: one DMA per d-tile (x row-block and
            # its weight row-block merged host-side into one dram tensor) --
            xts, wqgs = [], []
            for i in range(ND):
                t = _mk(pp, [P, S + QG], BF16, f"xw{i}")
                nc.sync.dma_start(out=t, in_=xw[i * P:(i + 1) * P, :])
                xts.append(t[:, :S])
                wqgs.append(t[:, S:])
            css = []
            for j in range(NS):
                t = _mk(pp, [P, 2 * QK], BF16, f"cs{j}")
                nc.scalar.dma_start(out=t, in_=cs[j * P:(j + 1) * P, :])
                css.append(t)
            mtri_sb = _mk(pp, [P, P], BF16, "mtri")
            nc.scalar.dma_start(out=mtri_sb, in_=mtri)
            wos = []
            for e in range(2):
                t = _mk(pp, [P, D], BF16, f"wo{e}")
                nc.sync.dma_start(out=t, in_=wo[e * P:(e + 1) * P, :])
                wos.append(t)
            ident = _mk(pp, [P, P], BF16, "ident")
            make_identity(nc, ident)
            halfones = _mk(pp, [1, P], BF16, "halfones")
            nc.vector.memset(halfones, 0.5)

            # persistent intermediate tensors
            qth = [[_mk(pp, [HD, SQ], BF16, f"qt{h}_{q}") for q in range(NSQ)]
                   for h in range(NHL)]
            kts = [_mk(pp, [HD, SQ], BF16, f"kt{q}") for q in range(NSQ)]
            vs = [_mk(pp, [P, HD + 1], BF16, f"v{j}") for j in range(NS)]
            gus = [[_mk(pp, [P, SQ], BF16, f"gu{p}_{q}") for q in range(NSQ)]
                   for p in range(2)]
            ctxgs = [[_mk(pp, [P, SQ], BF16, f"cg{p}_{q}") for q in range(NSQ)]
                     for p in range(2)]
            qk_all = [_mk(pp, [P, QK], BF16, f"qk{j}") for j in range(NS)]
            ss_all = _mk(pp, [P, NB * NS], F32, "ss_all")
            y_all = _mk(pp, [P, NB * NS], F32, "y_all")

            def d_sweep(p, qs):
                """Gate projection sweep: fixed p, q-pair qs in one 2-bank
                tile from the scores pool (free outside attention)."""
                ps_g = _mk(psp2, [P, 2 * SQ], F32, "ps2")
                for i in range(ND):
                    for o, qq in enumerate(qs):
                        nc.tensor.matmul(
                            ps_g[:, o * SQ:(o + 1) * SQ],
                            wqgs[i][:, QKV + p * P:QKV + (p + 1) * P],
                            xts[i][:, qq * SQ:(qq + 1) * SQ],
                            start=(i == 0), stop=(i == ND - 1))
                for o, qq in enumerate(qs):
                    # gus = tanh(g/2); the (1 + .)*0.5/den fold happens in E
                    nc.scalar.activation(gus[p][qq],
                                         ps_g[:, o * SQ:(o + 1) * SQ],
                                         mybir.ActivationFunctionType.Tanh,
                                         scale=0.5)


            def a_group(j):
                """QKV projection + stats for s-tile j."""
                ps_qkv = _mk(psp, [P, QKV], F32, "ps")
                for i in range(ND):
                    nc.tensor.matmul(
                        ps_qkv, xts[i][:, j * P:(j + 1) * P],
                        wqgs[i][:, :QKV],
                        start=(i == 0), stop=(i == ND - 1))
                # v (+ ones column) straight to SBUF
                nc.vector.tensor_copy(out=vs[j][:, :HD], in_=ps_qkv[:, QK:QKV])
                nc.vector.memset(vs[j][:, HD:HD + 1], 1.0)
                # squares + raw qk copy on ACT, block row-sums on DVE
                qk = ps_qkv[:, :QK]
                sqr = _mk(wp, [P, QK], F32, "sqr")
                nc.scalar.activation(sqr, qk,
                                     mybir.ActivationFunctionType.Square)
                nc.vector.tensor_reduce(
                    ss_all[:, NB * j:NB * (j + 1)],
                    _v(sqr, [[HD, NB], [1, HD]]),
                    axis=mybir.AxisListType.X, op=mybir.AluOpType.add)
                nc.scalar.copy(qk_all[j], qk)

            def b_chain(lo, hi, eng=None):
                """Batched Newton rsqrt for s-tiles [lo, hi); halves run on
                DVE and gpsimd in parallel to halve the serial latency."""
                if eng is None:
                    mid = (lo + hi) // 2
                    b_chain(lo, mid, nc.vector)
                    b_chain(mid, hi, nc.gpsimd)
                    return
                c0, c1 = NB * lo, NB * hi
                n = c1 - c0
                ss = ss_all[:, c0:c1]
                y = y_all[:, c0:c1]
                m = _mk(sp, [P, n], F32, "m")
                eng.tensor_scalar(m, ss, 1.0 / HD, EPS,
                                        mybir.AluOpType.mult,
                                        mybir.AluOpType.add)
                mc = _mk(sp, [P, n], F32, "mc")
                eng.tensor_scalar(mc, m, 5.5, 0.45,
                                        mybir.AluOpType.min,
                                        mybir.AluOpType.max)
                m2 = _mk(sp, [P, n], F32, "m2")
                eng.tensor_mul(m2, mc, mc)
                lin = _mk(sp, [P, n], F32, "lin")
                eng.tensor_scalar(lin, mc, -0.48330447, 1.51774376,
                                        mybir.AluOpType.mult,
                                        mybir.AluOpType.add)
                eng.scalar_tensor_tensor(y, m2, 0.0534932, lin,
                                               mybir.AluOpType.mult,
                                               mybir.AluOpType.add)
                ytmp = _mk(sp, [P, n], F32, "ytmp")
                for _ in range(3):
                    eng.tensor_mul(ytmp, y, y)              # y^2
                    eng.tensor_mul(ytmp, ytmp, m)           # m y^2
                    eng.tensor_scalar(ytmp, ytmp, -0.5, 1.5,
                                            mybir.AluOpType.mult,
                                            mybir.AluOpType.add)
                    eng.tensor_mul(y, y, ytmp)

            def c_tile(j):
                """Norm + rope + transpose for s-tile j.

                qkn/t1 on gpsimd, t2/qkr on DVE — splits the serial chain
                across two engines and halves each one's load."""
                yb = _v(y_all[:, NB * j:NB * (j + 1)], [[1, NB], [0, HD]])
                qkn = _mk(wp, [P, QK], BF16, "qkn")
                nc.gpsimd.tensor_mul(
                    _v(qkn, [[HD, NB], [1, HD]]),
                    _v(qk_all[j][:, :], [[HD, NB], [1, HD]]), yb)
                # rope: out = qkn*cos5 + rot(qkn)*sin5  (sin pre-negated on
                # the first half on host; cos/sin already include 1+norm_w)
                t1 = _mk(wp, [P, QK], BF16, "t1")
                nc.gpsimd.tensor_mul(t1, qkn, css[j][:, :QK])
                t2 = _mk(wp, [P, QK], BF16, "t2")
                rot = _v(qkn[:, :], [[HD, NB], [-32, 2], [1, 32]],
                         extra_offset=32)
                nc.vector.tensor_mul(
                    _v(t2, [[HD, NB], [32, 2], [1, 32]]), rot,
                    _v(css[j][:, QK:], [[HD, NB], [32, 2], [1, 32]]))
                qkr = _mk(wp, [P, QK], BF16, "qkr")
                nc.vector.tensor_add(qkr, t1, t2)
                # transpose q packs and k into [hd, s] layout
                jq, jc = j // 4, (j % 4) * P
                for p in range(2):
                    pt = _mk(psp, [P, P], BF16, "ps")
                    nc.tensor.transpose(pt, qkr[:, p * P:(p + 1) * P], ident)
                    nc.vector.tensor_copy(out=qth[2 * p][jq][:, jc:jc + P],
                                          in_=pt[:HD, :])
                    nc.vector.tensor_copy(out=qth[2 * p + 1][jq][:, jc:jc + P],
                                          in_=pt[HD:, :])
                ptk = _mk(psp, [HD, P], BF16, "ps")
                nc.tensor.transpose(ptk, qkr[:, 2 * P:2 * P + HD], ident)
                nc.vector.tensor_copy(out=kts[jq][:, jc:jc + P], in_=ptk)

            def e_slice(q, fqueue=()):
                """Attention for sq slice q, head pairs, sw-pipelined.

                The two heads of a pair share one [128, 2*SQ] scores psum
                (2 banks) and a single exp instruction; the masked prefix of
                the second half holds junk that the trimmed ctx matmuls
                never read."""
                nks = 4 * q + 4   # sk tiles 0..nks-1 intersect causally
                fqueue = list(fqueue)
                for hp in range(2):
                    heads = (2 * hp, 2 * hp + 1)
                    ps_ctx = {h: _mk(psp, [HD + 1, SQ], F32, "ps")
                              for h in heads}
                    pend = None   # (jk, c0, pr2)
                    for jk in range(nks):
                        dlt = jk - 4 * q
                        c0 = max(dlt, 0) * P  # cols < c0 fully masked
                        diag = dlt >= 0
                        kslice = kts[jk // 4][:, (jk % 4) * P:(jk % 4 + 1) * P]
                        ps_s = _mk(psp2, [P, 2 * SQ], F32, "ps2")
                        for hh, h in enumerate(heads):
                            o = hh * SQ
                            nc.tensor.matmul(
                                ps_s[:, o + c0:o + SQ], kslice,
                                qth[h][q][:, c0:],
                                start=True, stop=not diag)
                            if diag:
                                # accumulate -30000 on the masked (k>q) part
                                nc.tensor.matmul(
                                    ps_s[:, o + c0:o + c0 + P], ident,
                                    mtri_sb, start=False, stop=True)
                        pr2 = _mk(prp, [P, 2 * SQ], BF16, "pr")
                        nc.scalar.activation(
                            pr2[:, c0:], ps_s[:, c0:],
                            mybir.ActivationFunctionType.Exp, scale=SCALE)
                        if pend is not None:
                            pjk, pc0, ppr = pend
                            for hh, h in enumerate(heads):
                                o = hh * SQ
                                nc.tensor.matmul(
                                    ps_ctx[h][:, pc0:], vs[pjk],
                                    ppr[:, o + pc0:o + SQ],
                                    start=(pjk == 0), stop=False)
                        pend = (jk, c0, pr2)
                        if fqueue:
                            fqueue.pop(0)()
                    pjk, pc0, ppr = pend
                    for hh, h in enumerate(heads):
                        o = hh * SQ
                        nc.tensor.matmul(
                            ps_ctx[h][:, pc0:], vs[pjk],
                            ppr[:, o + pc0:o + SQ],
                            start=(pjk == 0), stop=True)
                    # per head: den recip (bf16) -> PE broadcast -> ctxg
                    for h in heads:
                        ho = (h % 2) * HD
                        denb = _mk(sp, [1, SQ], BF16, "denb")
                        with nc.allow_low_precision("softmax denom bf16"):
                            nc.vector.reciprocal(denb,
                                                 ps_ctx[h][HD:HD + 1, :])
                        ps_db = _mk(psp, [HD, SQ], F32, "ps")
                        nc.tensor.matmul(ps_db, halfones[:, :HD], denb,
                                         start=True, stop=True)
                        # gud = (tanh(g/2) + 1) * (0.5/den)
                        gud = _mk(wp, [HD, SQ], BF16, "gud")
                        nc.vector.scalar_tensor_tensor(
                            gud, gus[hp][q][ho:ho + HD, :], 1.0, ps_db,
                            mybir.AluOpType.add, mybir.AluOpType.mult)
                        nc.vector.tensor_mul(
                            ctxgs[hp][q][ho:ho + HD, :],
                            ps_ctx[h][:HD, :], gud)
                for g in fqueue:
                    g()

            def f_groups(q):
                """Output projection for sq slice q as 16 deferred groups,
                interleaved one-per-jk-step into the next e_slice."""
                groups = []
                for jj in range(4):
                    for n in range(NSQ):
                        def emit(jj=jj, n=n):
                            jc = jj * P
                            j = 4 * q + jj
                            ps_o = _mk(psp, [P, SQ], F32, "ps")
                            for e in range(2):
                                nc.tensor.matmul(
                                    ps_o, ctxgs[e][q][:, jc:jc + P],
                                    wos[e][:, n * SQ:(n + 1) * SQ],
                                    start=(e == 0), stop=(e == 1))
                            oc = _mk(ocp, [P, SQ], BF16, "oc")
                            nc.vector.tensor_copy(out=oc, in_=ps_o)
                            nc.sync.dma_start(
                                out=out[j * P:(j + 1) * P,
                                        n * SQ:(n + 1) * SQ],
                                in_=oc)
                        groups.append(emit)
                return groups

            # ---- schedule: gate sweeps fill the DMA window; the last 8
            # qkv groups and all out-projection groups are fed one-per-jk-
            # step into the attention slices, so the in-order PE program
            # stays dense from DMA arrival to the final output DMA ----
            _mark(nc, "phaseD0")
            d_sweep(0, (0, 1))
            d_sweep(1, (0, 1))
            _mark(nc, "phaseA")
            for j in range(4):
                a_group(j)
            _mark(nc, "phaseB")
            b_chain(0, 4)
            a_group(4)
            a_group(5)
            _mark(nc, "phaseC0")
            c_tile(0)
            c_tile(1)
            a_group(6)
            c_tile(2)
            c_tile(3)
            a_group(7)
            b_chain(4, 8)
            _mark(nc, "phaseE0")
            e_slice(0, [lambda j=j: a_group(j) for j in range(8, 12)])
            b_chain(8, 12)
            _mark(nc, "phaseC1")
            for j in range(4, 8):
                c_tile(j)
            _mark(nc, "phaseD1")
            d_sweep(0, (2, 3))
            _mark(nc, "phaseE1")
            e_slice(1, [lambda j=j: a_group(j) for j in range(12, 16)]
                    + f_groups(0))
            b_chain(12, 16)
            _mark(nc, "phaseC2")
            for j in range(8, 12):
                c_tile(j)
            _mark(nc, "phaseD2")
            d_sweep(1, (2, 3))
            _mark(nc, "phaseE2")
            e_slice(2, f_groups(1))
            _mark(nc, "phaseC3")
            for j in range(12, 16):
                c_tile(j)
            _mark(nc, "phaseE3")
            e_slice(3, f_groups(2))
            for g in f_groups(3):
                g()

    nc.compile()
    return nc


def prep_inputs(x, cos, sin, Wq, Wk, Wv, Wo, q_norm_w, k_norm_w):
    """Host-side shard + layout prep. Returns per-core input maps."""
    xtn = x.reshape(S, D).T.astype(NBF)

    # rope tables with (1 + norm_w) folded in, k-block appended, and the
    # sin first-half pre-negated (so rope is out = q*cos5 + rot(q)*sin5
    # with rot(q) = [q2, q1])
    half = HD // 2
    wq1 = (1.0 + q_norm_w).astype(np.float32)
    wk1 = (1.0 + k_norm_w).astype(np.float32)

    def rotw(w):
        return np.concatenate([w[half:], w[:half]])

    sin_m = sin.copy()
    sin_m[:, :half] = -sin_m[:, :half]
    cos_q = cos * wq1
    cos_k = cos * wk1
    sin_q = sin_m * rotw(wq1)
    sin_k = sin_m * rotw(wk1)
    cos5 = np.concatenate([np.tile(cos_q, (1, NHL)), cos_k], axis=1)
    sin5 = np.concatenate([np.tile(sin_q, (1, NHL)), sin_k], axis=1)
    cs = np.ascontiguousarray(
        np.concatenate([cos5, sin5], axis=1)).astype(NBF)

    # strict lower triangle (k > q within the diagonal block) gets -30000,
    # accumulated into the scores psum before exp
    mtri = (np.tril(np.full((P, P), NEG, dtype=np.float32), k=-1)).astype(NBF)

    Wqh = Wq.reshape(H, 2 * HD, D)
    in_maps = []
    for c in range(NCORE):
        hs = slice(NHL * c, NHL * (c + 1))
        wq_c = Wqh[hs, :HD, :].reshape(EL, D)       # q rows, 4 heads
        wgt_c = Wqh[hs, HD:, :].reshape(EL, D)      # gate rows
        wk_c = Wk[HD * c:HD * (c + 1), :]
        wv_c = Wv[HD * c:HD * (c + 1), :]
        # [640, D]: q | k | v | gate
        wqg_c = np.concatenate([wq_c, wk_c, wv_c, wgt_c], axis=0)
        xw_c = np.ascontiguousarray(
            np.concatenate([xtn, wqg_c.T.astype(NBF)], axis=1))
        in_maps.append({
            "xw": xw_c,
            "wo": np.ascontiguousarray(
                Wo[:, EL * c:EL * (c + 1)].T).astype(NBF),
            "cs": cs,
            "mtri": mtri,
        })
    return in_maps


_NC_CACHE = {}


def get_nc():
    if "nc" not in _NC_CACHE:
        _NC_CACHE["nc"] = build_nc()
    return _NC_CACHE["nc"]


def run(in_maps, trace=False, **kw):
    nc = get_nc()
    return run_bass_kernel_spmd(nc, in_maps, list(range(NCORE)),
                                trace=trace, **kw)


def kernel(x, mask, cos, sin, Wq, Wk, Wv, Wo, q_norm_w, k_norm_w):
    in_maps = prep_inputs(np.asarray(x, dtype=np.float32), np.asarray(cos),
                          np.asarray(sin), np.asarray(Wq), np.asarray(Wk),
                          np.asarray(Wv), np.asarray(Wo),
                          np.asarray(q_norm_w), np.asarray(k_norm_w))
    res = run(in_maps)
    acc = np.zeros((S, D), dtype=np.float32)
    for r in res.results:
        acc += np.asarray(r["out"], dtype=np.float32)
    return acc.reshape(1, S, D)


# revision 28
# speedup vs baseline: 1.0118x; 1.0118x over previous
"""GQA attention block (B=1, S=2048, D=2048, H=32, G=8, HD=64) on 8 trn2 cores.

Sharding: tensor-parallel over heads/KV-groups. Core c owns q-heads
4c..4c+3 and KV group c. Wq/Wk/Wv column-parallel, Wo row-parallel;
each core computes a partial [S, D] output, host sums the 8 partials.

Per-core dataflow (all matmuls bf16, stats f32), creation-ordered for
engine overlap (the tile scheduler keeps per-engine programs roughly in
creation order, so independent work is interleaved at emission time):
  DMAs interleaved per d-tile and split over both HWDGE queues
  (SP + ACT) so x streams in at 2x; wqkv|wg merged into one tensor.
  D (gate proj, [e,s] layout): 4 sweeps of (p, q-pair); first two fill
    the x-DMA window, last two fill later PE gaps. gus = tanh(g/2).
  A: qkv proj per s-tile -> psum [s,384]; ACT: square + raw-qk copy;
    DVE: v copy + block row-sums into one batched stats tile.
  B: one batched Newton-rsqrt chain over all stats (no per-tile chain).
  C: norm-mul + rope split over gpsimd and DVE + PE transposes to
    [hd,s]; psum->sbuf copies split over ACT and DVE.
  E: per (q-slice, head-pair): software-pipelined over k-tiles —
    scores for tile jk are emitted before the ctx matmuls of jk-1, so
    the in-order PE never waits on the exp. Causal mask by accumulating
    a -30000 strict-lower-tri matmul on diagonal blocks; probs =
    exp(scale*scores) on ACT (c0-trimmed); ctxT (+den row) =
    [v|1].T @ probsT with trimmed widths. Per head: den recip (bf16,
    low-precision ok) -> PE broadcast; gud = (tanh+1)*(0.5/den) via one
    scalar_tensor_tensor; ctxg = ctx * gud.
  F: out[s,dout] = ctxg.T @ woT per q-slice, psum->bf16 copies
    alternating ACT/DVE, partials summed on host.
"""

import numpy as np
import ml_dtypes

import concourse.bass as bass
import concourse.tile as tile
from concourse import bacc, mybir
from concourse.bass_utils import run_bass_kernel_spmd
from concourse.masks import make_identity

BF16 = mybir.dt.bfloat16
F32 = mybir.dt.float32
NBF = ml_dtypes.bfloat16

S = 2048
D = 2048
H = 32
G = 8
HD = 64
NCORE = 8
NHL = H // NCORE          # 4 q heads per core
EL = NHL * HD             # 256 local q (and gate, and ctx) features
QK = EL + HD              # 320: q + k features
QKV = QK + HD             # 384: q + k + v
QG = QKV + EL             # 640: qkv + gate columns in the merged weight
P = 128
NS = S // P               # 16 s-tiles
ND = D // P               # 16 d-tiles
SQ = 512
NSQ = S // SQ             # 4 sq slices
NB = QK // HD             # 5 (hd,) blocks in the q|k strip
SCALE = HD ** -0.5
EPS = 1e-6
NEG = -30000.0


def _v(ap, dims, extra_offset=0):
    """Reshape the free dims of a 2D AP into `dims` ([step, count] pairs),
    keeping the partition dim."""
    return bass.AP(
        tensor=ap.tensor,
        offset=ap.offset + extra_offset,
        ap=[list(ap.ap[0])] + [list(d) for d in dims],
    )


def _mk(pool, shape, dtype, tag):
    return pool.tile(shape, dtype, tag=tag, name=tag)


PHASES = []  # (phase_name, first_instruction_index) — debug aid for sim.py


def _mark(nc, name):
    PHASES.append((name, int(nc.get_next_instruction_name().split("-")[1])))


def build_nc():
    nc = bacc.Bacc("TRN2", target_bir_lowering=False, debug=False,
                   num_devices=NCORE)

    xw = nc.dram_tensor("xw", [D, S + QG], BF16, kind="ExternalInput").ap()
    wo = nc.dram_tensor("wo", [EL, D], BF16, kind="ExternalInput").ap()
    cs = nc.dram_tensor("cs", [S, 2 * QK], BF16, kind="ExternalInput").ap()
    mtri = nc.dram_tensor("mtri", [P, P], BF16, kind="ExternalInput").ap()
    out = nc.dram_tensor("out", [S, D], BF16, kind="ExternalOutput").ap()

    with tile.TileContext(nc) as tc:
        with (
            tc.tile_pool(name="persist", bufs=1) as pp,
            tc.tile_pool(name="work", bufs=3) as wp,
            tc.tile_pool(name="stats", bufs=2) as sp,
            tc.tile_pool(name="probs", bufs=10) as prp,
            tc.tile_pool(name="outc", bufs=4) as ocp,
            tc.tile_pool(name="psum", bufs=4, space="PSUM") as psp,
            tc.tile_pool(name="psum2", bufs=2, space="PSUM") as psp2,
        ):
            # ---- persistent loads: one DMA per d-tile (x row-block and
            # its weight row-block merged host-side into one dram tensor) --
            xts, wqgs = [], []
            for i in range(ND):
                t = _mk(pp, [P, S + QG], BF16, f"xw{i}")
                nc.sync.dma_start(out=t, in_=xw[i * P:(i + 1) * P, :])
                xts.append(t[:, :S])
                wqgs.append(t[:, S:])
            css = []
            for j in range(NS):
                t = _mk(pp, [P, 2 * QK], BF16, f"cs{j}")
                nc.scalar.dma_start(out=t, in_=cs[j * P:(j + 1) * P, :])
                css.append(t)
            mtri_sb = _mk(pp, [P, P], BF16, "mtri")
            nc.scalar.dma_start(out=mtri_sb, in_=mtri)
            wos = []
            for e in range(2):
                t = _mk(pp, [P, D], BF16, f"wo{e}")
                nc.sync.dma_start(out=t, in_=wo[e * P:(e + 1) * P, :])
                wos.append(t)
            ident = _mk(pp, [P, P], BF16, "ident")
            make_identity(nc, ident)
            halfones = _mk(pp, [1, P], BF16, "halfones")
            nc.vector.memset(halfones, 0.5)

            # persistent intermediate tensors
            qth = [[_mk(pp, [HD, SQ], BF16, f"qt{h}_{q}") for q in range(NSQ)]
                   for h in range(NHL)]
            kts = [_mk(pp, [HD, SQ], BF16, f"kt{q}") for q in range(NSQ)]
            vs = [_mk(pp, [P, HD + 1], BF16, f"v{j}") for j in range(NS)]
            gus = [[_mk(pp, [P, SQ], BF16, f"gu{p}_{q}") for q in range(NSQ)]
                   for p in range(2)]
            ctxgs = [[_mk(pp, [P, SQ], BF16, f"cg{p}_{q}") for q in range(NSQ)]
                     for p in range(2)]
            qk_all = [_mk(pp, [P, QK], BF16, f"qk{j}") for j in range(NS)]
            ss_all = _mk(pp, [P, NB * NS], F32, "ss_all")
            y_all = _mk(pp, [P, NB * NS], F32, "y_all")

            def d_sweep_mm(p, qs):
                """Gate projection sweep matmuls: fixed p, q-pair qs in one
                2-bank tile from the scores pool (free outside attention)."""
                ps_g = _mk(psp2, [P, 2 * SQ], F32, "ps2")
                for i in range(ND):
                    for o, qq in enumerate(qs):
                        nc.tensor.matmul(
                            ps_g[:, o * SQ:(o + 1) * SQ],
                            wqgs[i][:, QKV + p * P:QKV + (p + 1) * P],
                            xts[i][:, qq * SQ:(qq + 1) * SQ],
                            start=(i == 0), stop=(i == ND - 1))
                return ps_g

            def d_sweep_act(p, qs, ps_g):
                """Deferred tanh part of a gate sweep (keeps the in-order
                ACT queue from blocking later work on the sweep's finish)."""
                for o, qq in enumerate(qs):
                    # gus = tanh(g/2); the (1 + .)*0.5/den fold happens in E
                    nc.scalar.activation(gus[p][qq],
                                         ps_g[:, o * SQ:(o + 1) * SQ],
                                         mybir.ActivationFunctionType.Tanh,
                                         scale=0.5)

            def d_sweep(p, qs):
                d_sweep_act(p, qs, d_sweep_mm(p, qs))


            def a_group(j):
                """QKV projection + stats for s-tile j."""
                ps_qkv = _mk(psp, [P, QKV], F32, "ps")
                for i in range(ND):
                    nc.tensor.matmul(
                        ps_qkv, xts[i][:, j * P:(j + 1) * P],
                        wqgs[i][:, :QKV],
                        start=(i == 0), stop=(i == ND - 1))
                # v (+ ones column) straight to SBUF
                nc.vector.tensor_copy(out=vs[j][:, :HD], in_=ps_qkv[:, QK:QKV])
                nc.vector.memset(vs[j][:, HD:HD + 1], 1.0)
                # squares + raw qk copy on ACT, block row-sums on DVE
                qk = ps_qkv[:, :QK]
                sqr = _mk(wp, [P, QK], F32, "sqr")
                nc.scalar.activation(sqr, qk,
                                     mybir.ActivationFunctionType.Square)
                nc.vector.tensor_reduce(
                    ss_all[:, NB * j:NB * (j + 1)],
                    _v(sqr, [[HD, NB], [1, HD]]),
                    axis=mybir.AxisListType.X, op=mybir.AluOpType.add)
                nc.scalar.copy(qk_all[j], qk)

            def b_chain(lo, hi, eng=None):
                """Batched Newton rsqrt for s-tiles [lo, hi). DVE only:
                tensor_scalar/scalar_tensor_tensor are not legal Pool-engine
                ops (walrus NCC_IXCG966), so no gpsimd half here."""
                if eng is None:
                    eng = nc.vector
                c0, c1 = NB * lo, NB * hi
                n = c1 - c0
                ss = ss_all[:, c0:c1]
                y = y_all[:, c0:c1]
                m = _mk(sp, [P, n], F32, "m")
                eng.tensor_scalar(m, ss, 1.0 / HD, EPS,
                                        mybir.AluOpType.mult,
                                        mybir.AluOpType.add)
                mc = _mk(sp, [P, n], F32, "mc")
                eng.tensor_scalar(mc, m, 5.5, 0.45,
                                        mybir.AluOpType.min,
                                        mybir.AluOpType.max)
                m2 = _mk(sp, [P, n], F32, "m2")
                eng.tensor_mul(m2, mc, mc)
                lin = _mk(sp, [P, n], F32, "lin")
                eng.tensor_scalar(lin, mc, -0.48330447, 1.51774376,
                                        mybir.AluOpType.mult,
                                        mybir.AluOpType.add)
                eng.scalar_tensor_tensor(y, m2, 0.0534932, lin,
                                               mybir.AluOpType.mult,
                                               mybir.AluOpType.add)
                ytmp = _mk(sp, [P, n], F32, "ytmp")
                for _ in range(3):
                    eng.tensor_mul(ytmp, y, y)              # y^2
                    eng.tensor_mul(ytmp, ytmp, m)           # m y^2
                    eng.tensor_scalar(ytmp, ytmp, -0.5, 1.5,
                                            mybir.AluOpType.mult,
                                            mybir.AluOpType.add)
                    eng.tensor_mul(y, y, ytmp)

            def c_tile(j):
                """Norm + rope + transpose for s-tile j.

                qkn/t1 on gpsimd, t2/qkr on DVE — splits the serial chain
                across two engines and halves each one's load."""
                yb = _v(y_all[:, NB * j:NB * (j + 1)], [[1, NB], [0, HD]])
                qkn = _mk(wp, [P, QK], BF16, "qkn")
                nc.gpsimd.tensor_mul(
                    _v(qkn, [[HD, NB], [1, HD]]),
                    _v(qk_all[j][:, :], [[HD, NB], [1, HD]]), yb)
                # rope: out = qkn*cos5 + rot(qkn)*sin5  (sin pre-negated on
                # the first half on host; cos/sin already include 1+norm_w)
                t1 = _mk(wp, [P, QK], BF16, "t1")
                nc.gpsimd.tensor_mul(t1, qkn, css[j][:, :QK])
                t2 = _mk(wp, [P, QK], BF16, "t2")
                rot = _v(qkn[:, :], [[HD, NB], [-32, 2], [1, 32]],
                         extra_offset=32)
                nc.vector.tensor_mul(
                    _v(t2, [[HD, NB], [32, 2], [1, 32]]), rot,
                    _v(css[j][:, QK:], [[HD, NB], [32, 2], [1, 32]]))
                qkr = _mk(wp, [P, QK], BF16, "qkr")
                nc.vector.tensor_add(qkr, t1, t2)
                # transpose q packs and k into [hd, s] layout
                jq, jc = j // 4, (j % 4) * P
                for p in range(2):
                    pt = _mk(psp, [P, P], BF16, "ps")
                    nc.tensor.transpose(pt, qkr[:, p * P:(p + 1) * P], ident)
                    nc.vector.tensor_copy(out=qth[2 * p][jq][:, jc:jc + P],
                                          in_=pt[:HD, :])
                    nc.vector.tensor_copy(out=qth[2 * p + 1][jq][:, jc:jc + P],
                                          in_=pt[HD:, :])
                ptk = _mk(psp, [HD, P], BF16, "ps")
                nc.tensor.transpose(ptk, qkr[:, 2 * P:2 * P + HD], ident)
                nc.vector.tensor_copy(out=kts[jq][:, jc:jc + P], in_=ptk)

            def e_slice(q, fqueue=()):
                """Attention for sq slice q, head pairs, sw-pipelined.

                The two heads of a pair share one [128, 2*SQ] scores psum
                (2 banks) and a single exp instruction; the masked prefix of
                the second half holds junk that the trimmed ctx matmuls
                never read."""
                nks = 4 * q + 4   # sk tiles 0..nks-1 intersect causally
                fqueue = list(fqueue)
                for hp in range(2):
                    heads = (2 * hp, 2 * hp + 1)
                    ps_ctx = {h: _mk(psp, [HD + 1, SQ], F32, "ps")
                              for h in heads}
                    pend = None   # (jk, c0, pr2)
                    for jk in range(nks):
                        if fqueue:
                            fqueue.pop(0)()
                        dlt = jk - 4 * q
                        c0 = max(dlt, 0) * P  # cols < c0 fully masked
                        diag = dlt >= 0
                        kslice = kts[jk // 4][:, (jk % 4) * P:(jk % 4 + 1) * P]
                        ps_s = _mk(psp2, [P, 2 * SQ], F32, "ps2")
                        for hh, h in enumerate(heads):
                            o = hh * SQ
                            nc.tensor.matmul(
                                ps_s[:, o + c0:o + SQ], kslice,
                                qth[h][q][:, c0:],
                                start=True, stop=not diag)
                            if diag:
                                # accumulate -30000 on the masked (k>q) part
                                nc.tensor.matmul(
                                    ps_s[:, o + c0:o + c0 + P], ident,
                                    mtri_sb, start=False, stop=True)
                        pr2 = _mk(prp, [P, 2 * SQ], BF16, "pr")
                        nc.scalar.activation(
                            pr2[:, c0:], ps_s[:, c0:],
                            mybir.ActivationFunctionType.Exp, scale=SCALE)
                        if pend is not None:
                            pjk, pc0, ppr = pend
                            for hh, h in enumerate(heads):
                                o = hh * SQ
                                nc.tensor.matmul(
                                    ps_ctx[h][:, pc0:], vs[pjk],
                                    ppr[:, o + pc0:o + SQ],
                                    start=(pjk == 0), stop=False)
                        pend = (jk, c0, pr2)
                    pjk, pc0, ppr = pend
                    for hh, h in enumerate(heads):
                        o = hh * SQ
                        nc.tensor.matmul(
                            ps_ctx[h][:, pc0:], vs[pjk],
                            ppr[:, o + pc0:o + SQ],
                            start=(pjk == 0), stop=True)
                    # per head: den recip (bf16) -> PE broadcast -> ctxg
                    for h in heads:
                        ho = (h % 2) * HD
                        denb = _mk(sp, [1, SQ], BF16, "denb")
                        with nc.allow_low_precision("softmax denom bf16"):
                            nc.vector.reciprocal(denb,
                                                 ps_ctx[h][HD:HD + 1, :])
                        ps_db = _mk(psp, [HD, SQ], F32, "ps")
                        nc.tensor.matmul(ps_db, halfones[:, :HD], denb,
                                         start=True, stop=True)
                        # gud = (tanh(g/2) + 1) * (0.5/den)
                        gud = _mk(wp, [HD, SQ], BF16, "gud")
                        nc.vector.scalar_tensor_tensor(
                            gud, gus[hp][q][ho:ho + HD, :], 1.0, ps_db,
                            mybir.AluOpType.add, mybir.AluOpType.mult)
                        nc.vector.tensor_mul(
                            ctxgs[hp][q][ho:ho + HD, :],
                            ps_ctx[h][:HD, :], gud)
                for g in fqueue:
                    g()

            def f_groups(q):
                """Output projection for sq slice q as 16 deferred groups,
                interleaved one-per-jk-step into the next e_slice."""
                groups = []
                for jj in range(4):
                    for n in range(NSQ):
                        def emit(jj=jj, n=n):
                            jc = jj * P
                            j = 4 * q + jj
                            ps_o = _mk(psp, [P, SQ], F32, "ps")
                            for e in range(2):
                                nc.tensor.matmul(
                                    ps_o, ctxgs[e][q][:, jc:jc + P],
                                    wos[e][:, n * SQ:(n + 1) * SQ],
                                    start=(e == 0), stop=(e == 1))
                            oc = _mk(ocp, [P, SQ], BF16, "oc")
                            nc.vector.tensor_copy(out=oc, in_=ps_o)
                            nc.sync.dma_start(
                                out=out[j * P:(j + 1) * P,
                                        n * SQ:(n + 1) * SQ],
                                in_=oc)
                        groups.append(emit)
                return groups

            # ---- schedule: gate sweeps fill the DMA window; the last 8
            # qkv groups and all out-projection groups are fed one-per-jk-
            # step into the attention slices, so the in-order PE program
            # stays dense from DMA arrival to the final output DMA ----
            _mark(nc, "phaseD0")
            g0 = d_sweep_mm(0, (0, 1))
            _mark(nc, "phaseA")
            for j in range(4):
                a_group(j)
            g1 = d_sweep_mm(1, (0, 1))
            d_sweep_act(0, (0, 1), g0)
            _mark(nc, "phaseB")
            b_chain(0, 4)
            a_group(4)
            a_group(5)
            _mark(nc, "phaseC0")
            c_tile(0)
            c_tile(1)
            a_group(6)
            d_sweep_act(1, (0, 1), g1)
            c_tile(2)
            c_tile(3)
            a_group(7)
            b_chain(4, 8)
            _mark(nc, "phaseE0")
            e_slice(0, [lambda j=j: a_group(j) for j in range(8, 12)])
            b_chain(8, 12)
            _mark(nc, "phaseC1")
            for j in range(4, 8):
                c_tile(j)
            _mark(nc, "phaseD1")
            d_sweep(0, (2, 3))
            _mark(nc, "phaseE1")
            e_slice(1, [lambda j=j: a_group(j) for j in range(12, 16)]
                    + f_groups(0))
            b_chain(12, 16)
            _mark(nc, "phaseC2")
            for j in range(8, 12):
                c_tile(j)
            _mark(nc, "phaseD2")
            d_sweep(1, (2, 3))
            _mark(nc, "phaseE2")
            e_slice(2, f_groups(1))
            _mark(nc, "phaseC3")
            for j in range(12, 16):
                c_tile(j)
            _mark(nc, "phaseE3")
            e_slice(3, f_groups(2))
            for g in f_groups(3):
                g()

    nc.compile()
    return nc


def prep_inputs(x, cos, sin, Wq, Wk, Wv, Wo, q_norm_w, k_norm_w):
    """Host-side shard + layout prep. Returns per-core input maps."""
    xtn = x.reshape(S, D).T.astype(NBF)

    # rope tables with (1 + norm_w) folded in, k-block appended, and the
    # sin first-half pre-negated (so rope is out = q*cos5 + rot(q)*sin5
    # with rot(q) = [q2, q1])
    half = HD // 2
    wq1 = (1.0 + q_norm_w).astype(np.float32)
    wk1 = (1.0 + k_norm_w).astype(np.float32)

    def rotw(w):
        return np.concatenate([w[half:], w[:half]])

    sin_m = sin.copy()
    sin_m[:, :half] = -sin_m[:, :half]
    cos_q = cos * wq1
    cos_k = cos * wk1
    sin_q = sin_m * rotw(wq1)
    sin_k = sin_m * rotw(wk1)
    cos5 = np.concatenate([np.tile(cos_q, (1, NHL)), cos_k], axis=1)
    sin5 = np.concatenate([np.tile(sin_q, (1, NHL)), sin_k], axis=1)
    cs = np.ascontiguousarray(
        np.concatenate([cos5, sin5], axis=1)).astype(NBF)

    # strict lower triangle (k > q within the diagonal block) gets -30000,
    # accumulated into the scores psum before exp
    mtri = (np.tril(np.full((P, P), NEG, dtype=np.float32), k=-1)).astype(NBF)

    Wqh = Wq.reshape(H, 2 * HD, D)
    in_maps = []
    for c in range(NCORE):
        hs = slice(NHL * c, NHL * (c + 1))
        wq_c = Wqh[hs, :HD, :].reshape(EL, D)       # q rows, 4 heads
        wgt_c = Wqh[hs, HD:, :].reshape(EL, D)      # gate rows
        wk_c = Wk[HD * c:HD * (c + 1), :]
        wv_c = Wv[HD * c:HD * (c + 1), :]
        # [640, D]: q | k | v | gate
        wqg_c = np.concatenate([wq_c, wk_c, wv_c, wgt_c], axis=0)
        xw_c = np.ascontiguousarray(
            np.concatenate([xtn, wqg_c.T.astype(NBF)], axis=1))
        in_maps.append({
            "xw": xw_c,
            "wo": np.ascontiguousarray(
                Wo[:, EL * c:EL * (c + 1)].T).astype(NBF),
            "cs": cs,
            "mtri": mtri,
        })
    return in_maps


_NC_CACHE = {}


def get_nc():
    if "nc" not in _NC_CACHE:
        _NC_CACHE["nc"] = build_nc()
    return _NC_CACHE["nc"]


def run(in_maps, trace=False, **kw):
    nc = get_nc()
    return run_bass_kernel_spmd(nc, in_maps, list(range(NCORE)),
                                trace=trace, **kw)


def kernel(x, mask, cos, sin, Wq, Wk, Wv, Wo, q_norm_w, k_norm_w):
    in_maps = prep_inputs(np.asarray(x, dtype=np.float32), np.asarray(cos),
                          np.asarray(sin), np.asarray(Wq), np.asarray(Wk),
                          np.asarray(Wv), np.asarray(Wo),
                          np.asarray(q_norm_w), np.asarray(k_norm_w))
    res = run(in_maps)
    acc = np.zeros((S, D), dtype=np.float32)
    for r in res.results:
        acc += np.asarray(r["out"], dtype=np.float32)
    return acc.reshape(1, S, D)


# revision 33
# speedup vs baseline: 1.0529x; 1.0406x over previous
"""GQA attention block (B=1, S=2048, D=2048, H=32, G=8, HD=64) on 8 trn2 cores.

Sharding: tensor-parallel over heads/KV-groups. Core c owns q-heads
4c..4c+3 and KV group c. Wq/Wk/Wv column-parallel, Wo row-parallel;
each core computes a partial [S, D] output, host sums the 8 partials.

Per-core dataflow (all matmuls bf16, stats f32), creation-ordered for
engine overlap (the tile scheduler keeps per-engine programs roughly in
creation order, so independent work is interleaved at emission time):
  DMAs interleaved per d-tile and split over both HWDGE queues
  (SP + ACT) so x streams in at 2x; wqkv|wg merged into one tensor.
  D (gate proj, [e,s] layout): 4 sweeps of (p, q-pair); first two fill
    the x-DMA window, last two fill later PE gaps. gus = tanh(g/2).
  A: qkv proj per s-tile -> psum [s,384]; ACT: square + raw-qk copy;
    DVE: v copy + block row-sums into one batched stats tile.
  B: one batched Newton-rsqrt chain over all stats (no per-tile chain).
  C: norm-mul + rope split over gpsimd and DVE + PE transposes to
    [hd,s]; psum->sbuf copies split over ACT and DVE.
  E: per (q-slice, head-pair): software-pipelined over k-tiles —
    scores for tile jk are emitted before the ctx matmuls of jk-1, so
    the in-order PE never waits on the exp. Causal mask by accumulating
    a -30000 strict-lower-tri matmul on diagonal blocks; probs =
    exp(scale*scores) on ACT (c0-trimmed); ctxT (+den row) =
    [v|1].T @ probsT with trimmed widths. Per head: den recip (bf16,
    low-precision ok) -> PE broadcast; gud = (tanh+1)*(0.5/den) via one
    scalar_tensor_tensor; ctxg = ctx * gud.
  F: out[s,dout] = ctxg.T @ woT per q-slice, psum->bf16 copies
    alternating ACT/DVE, partials summed on host.
"""

import numpy as np
import ml_dtypes

import concourse.bass as bass
import concourse.tile as tile
from concourse import bacc, mybir
from concourse.bass_utils import run_bass_kernel_spmd
from concourse.masks import make_identity

BF16 = mybir.dt.bfloat16
F32 = mybir.dt.float32
NBF = ml_dtypes.bfloat16

S = 2048
D = 2048
H = 32
G = 8
HD = 64
NCORE = 8
NHL = H // NCORE          # 4 q heads per core
EL = NHL * HD             # 256 local q (and gate, and ctx) features
QK = EL + HD              # 320: q + k features
QKV = QK + HD             # 384: q + k + v
QG = QKV + EL             # 640: qkv + gate columns in the merged weight
P = 128
NS = S // P               # 16 s-tiles
ND = D // P               # 16 d-tiles
SQ = 512
NSQ = S // SQ             # 4 sq slices
NB = QK // HD             # 5 (hd,) blocks in the q|k strip
SCALE = HD ** -0.5
EPS = 1e-6
NEG = -30000.0


def _v(ap, dims, extra_offset=0):
    """Reshape the free dims of a 2D AP into `dims` ([step, count] pairs),
    keeping the partition dim."""
    return bass.AP(
        tensor=ap.tensor,
        offset=ap.offset + extra_offset,
        ap=[list(ap.ap[0])] + [list(d) for d in dims],
    )


def _mk(pool, shape, dtype, tag):
    return pool.tile(shape, dtype, tag=tag, name=tag)


PHASES = []  # (phase_name, first_instruction_index) — debug aid for sim.py


def _mark(nc, name):
    PHASES.append((name, int(nc.get_next_instruction_name().split("-")[1])))


def build_nc():
    nc = bacc.Bacc("TRN2", target_bir_lowering=False, debug=False,
                   num_devices=NCORE)

    xw = nc.dram_tensor("xw", [D, S + QG], BF16, kind="ExternalInput").ap()
    wo = nc.dram_tensor("wo", [EL, D], BF16, kind="ExternalInput").ap()
    cs = nc.dram_tensor("cs", [S, 2 * QK], BF16, kind="ExternalInput").ap()
    mtri = nc.dram_tensor("mtri", [P, P], BF16, kind="ExternalInput").ap()
    out = nc.dram_tensor("out", [S, D], BF16, kind="ExternalOutput").ap()

    with tile.TileContext(nc) as tc:
        with (
            tc.tile_pool(name="persist", bufs=1) as pp,
            tc.tile_pool(name="work", bufs=3) as wp,
            tc.tile_pool(name="stats", bufs=2) as sp,
            tc.tile_pool(name="probs", bufs=10) as prp,
            tc.tile_pool(name="outc", bufs=4) as ocp,
            tc.tile_pool(name="psum", bufs=4, space="PSUM") as psp,
            tc.tile_pool(name="psum2", bufs=2, space="PSUM") as psp2,
        ):
            # ---- persistent loads: one DMA per d-tile (x row-block and
            # its weight row-block merged host-side into one dram tensor) --
            xts, wqgs = [], []
            for i in range(ND):
                t = _mk(pp, [P, S + QG], BF16, f"xw{i}")
                nc.sync.dma_start(out=t, in_=xw[i * P:(i + 1) * P, :])
                xts.append(t[:, :S])
                wqgs.append(t[:, S:])
            css = []
            for j in range(NS):
                t = _mk(pp, [P, 2 * QK], BF16, f"cs{j}")
                nc.scalar.dma_start(out=t, in_=cs[j * P:(j + 1) * P, :])
                css.append(t)
            mtri_sb = _mk(pp, [P, P], BF16, "mtri")
            nc.scalar.dma_start(out=mtri_sb, in_=mtri)
            wos = []
            for e in range(2):
                t = _mk(pp, [P, D], BF16, f"wo{e}")
                nc.sync.dma_start(out=t, in_=wo[e * P:(e + 1) * P, :])
                wos.append(t)
            ident = _mk(pp, [P, P], BF16, "ident")
            make_identity(nc, ident)
            halfones = _mk(pp, [1, P], BF16, "halfones")
            nc.vector.memset(halfones, 0.5)

            # persistent intermediate tensors
            qth = [[_mk(pp, [HD, SQ], BF16, f"qt{h}_{q}") for q in range(NSQ)]
                   for h in range(NHL)]
            kts = [_mk(pp, [HD, SQ], BF16, f"kt{q}") for q in range(NSQ)]
            vs = [_mk(pp, [P, HD + 1], BF16, f"v{j}") for j in range(NS)]
            gus = [[_mk(pp, [P, SQ], BF16, f"gu{p}_{q}") for q in range(NSQ)]
                   for p in range(2)]
            ctxgs = [[_mk(pp, [P, SQ], BF16, f"cg{p}_{q}") for q in range(NSQ)]
                     for p in range(2)]
            qk_all = [_mk(pp, [P, QK], BF16, f"qk{j}") for j in range(NS)]
            ss_all = _mk(pp, [P, NB * NS], F32, "ss_all")
            y_all = _mk(pp, [P, NB * NS], F32, "y_all")

            def d_sweep_mm(p, qs):
                """Gate projection sweep matmuls: fixed p, q-pair qs in one
                2-bank tile from the scores pool (free outside attention)."""
                ps_g = _mk(psp2, [P, 2 * SQ], F32, "ps2")
                for i in range(ND):
                    for o, qq in enumerate(qs):
                        nc.tensor.matmul(
                            ps_g[:, o * SQ:(o + 1) * SQ],
                            wqgs[i][:, QKV + p * P:QKV + (p + 1) * P],
                            xts[i][:, qq * SQ:(qq + 1) * SQ],
                            start=(i == 0), stop=(i == ND - 1))
                return ps_g

            def d_sweep_act(p, qs, ps_g):
                """Deferred tanh part of a gate sweep (keeps the in-order
                ACT queue from blocking later work on the sweep's finish)."""
                for o, qq in enumerate(qs):
                    # gus = tanh(g/2); the (1 + .)*0.5/den fold happens in E
                    nc.scalar.activation(gus[p][qq],
                                         ps_g[:, o * SQ:(o + 1) * SQ],
                                         mybir.ActivationFunctionType.Tanh,
                                         scale=0.5)

            def d_sweep(p, qs):
                d_sweep_act(p, qs, d_sweep_mm(p, qs))


            def a_group(j):
                """QKV projection + stats for s-tile j."""
                ps_qkv = _mk(psp, [P, QKV], F32, "ps")
                for i in range(ND):
                    nc.tensor.matmul(
                        ps_qkv, xts[i][:, j * P:(j + 1) * P],
                        wqgs[i][:, :QKV],
                        start=(i == 0), stop=(i == ND - 1))
                # v (+ ones column) straight to SBUF
                nc.vector.tensor_copy(out=vs[j][:, :HD], in_=ps_qkv[:, QK:QKV])
                nc.vector.memset(vs[j][:, HD:HD + 1], 1.0)
                # squares + raw qk copy on ACT, block row-sums on DVE
                qk = ps_qkv[:, :QK]
                sqr = _mk(wp, [P, QK], F32, "sqr")
                nc.scalar.activation(sqr, qk,
                                     mybir.ActivationFunctionType.Square)
                nc.vector.tensor_reduce(
                    ss_all[:, NB * j:NB * (j + 1)],
                    _v(sqr, [[HD, NB], [1, HD]]),
                    axis=mybir.AxisListType.X, op=mybir.AluOpType.add)
                nc.scalar.copy(qk_all[j], qk)

            def b_chain(lo, hi, eng=None):
                """Batched Newton rsqrt for s-tiles [lo, hi). DVE only:
                tensor_scalar/scalar_tensor_tensor are not legal Pool-engine
                ops (walrus NCC_IXCG966), so no gpsimd half here."""
                if eng is None:
                    eng = nc.vector
                c0, c1 = NB * lo, NB * hi
                n = c1 - c0
                ss = ss_all[:, c0:c1]
                y = y_all[:, c0:c1]
                m = _mk(sp, [P, n], F32, "m")
                eng.tensor_scalar(m, ss, 1.0 / HD, EPS,
                                        mybir.AluOpType.mult,
                                        mybir.AluOpType.add)
                mc = _mk(sp, [P, n], F32, "mc")
                eng.tensor_scalar(mc, m, 5.5, 0.45,
                                        mybir.AluOpType.min,
                                        mybir.AluOpType.max)
                m2 = _mk(sp, [P, n], F32, "m2")
                eng.tensor_mul(m2, mc, mc)
                lin = _mk(sp, [P, n], F32, "lin")
                eng.tensor_scalar(lin, mc, -0.48330447, 1.51774376,
                                        mybir.AluOpType.mult,
                                        mybir.AluOpType.add)
                eng.scalar_tensor_tensor(y, m2, 0.0534932, lin,
                                               mybir.AluOpType.mult,
                                               mybir.AluOpType.add)
                ytmp = _mk(sp, [P, n], F32, "ytmp")
                for _ in range(3):
                    eng.tensor_mul(ytmp, y, y)              # y^2
                    eng.tensor_mul(ytmp, ytmp, m)           # m y^2
                    eng.tensor_scalar(ytmp, ytmp, -0.5, 1.5,
                                            mybir.AluOpType.mult,
                                            mybir.AluOpType.add)
                    eng.tensor_mul(y, y, ytmp)

            def c_tile(j):
                """Norm + rope + transpose for s-tile j.

                qkn/t1 on gpsimd, t2/qkr on DVE — splits the serial chain
                across two engines and halves each one's load."""
                yb = _v(y_all[:, NB * j:NB * (j + 1)], [[1, NB], [0, HD]])
                qkn = _mk(wp, [P, QK], BF16, "qkn")
                nc.gpsimd.tensor_mul(
                    _v(qkn, [[HD, NB], [1, HD]]),
                    _v(qk_all[j][:, :], [[HD, NB], [1, HD]]), yb)
                # rope: out = qkn*cos5 + rot(qkn)*sin5  (sin pre-negated on
                # the first half on host; cos/sin already include 1+norm_w)
                t1 = _mk(wp, [P, QK], BF16, "t1")
                nc.gpsimd.tensor_mul(t1, qkn, css[j][:, :QK])
                t2 = _mk(wp, [P, QK], BF16, "t2")
                rot = _v(qkn[:, :], [[HD, NB], [-32, 2], [1, 32]],
                         extra_offset=32)
                nc.vector.tensor_mul(
                    _v(t2, [[HD, NB], [32, 2], [1, 32]]), rot,
                    _v(css[j][:, QK:], [[HD, NB], [32, 2], [1, 32]]))
                qkr = _mk(wp, [P, QK], BF16, "qkr")
                nc.vector.tensor_add(qkr, t1, t2)
                # transpose q packs and k into [hd, s] layout
                jq, jc = j // 4, (j % 4) * P
                for p in range(2):
                    pt = _mk(psp, [P, P], BF16, "ps")
                    nc.tensor.transpose(pt, qkr[:, p * P:(p + 1) * P], ident)
                    nc.vector.tensor_copy(out=qth[2 * p][jq][:, jc:jc + P],
                                          in_=pt[:HD, :])
                    nc.vector.tensor_copy(out=qth[2 * p + 1][jq][:, jc:jc + P],
                                          in_=pt[HD:, :])
                ptk = _mk(psp, [HD, P], BF16, "ps")
                nc.tensor.transpose(ptk, qkr[:, 2 * P:2 * P + HD], ident)
                nc.vector.tensor_copy(out=kts[jq][:, jc:jc + P], in_=ptk)

            def e_slice(q, fqueue=()):
                """Attention for sq slice q, head pairs, sw-pipelined.

                The two heads of a pair share one [128, 2*SQ] scores psum
                (2 banks) and a single exp instruction; the masked prefix of
                the second half holds junk that the trimmed ctx matmuls
                never read."""
                nks = 4 * q + 4   # sk tiles 0..nks-1 intersect causally
                fqueue = list(fqueue)
                for hp in range(2):
                    heads = (2 * hp, 2 * hp + 1)
                    ps_ctx = {h: _mk(psp, [HD + 1, SQ], F32, "ps")
                              for h in heads}
                    pend = []   # [(jk, c0, pr2)] pending ctx, depth 2
                    for jk in range(nks):
                        if fqueue:
                            fqueue.pop(0)()
                        dlt = jk - 4 * q
                        c0 = max(dlt, 0) * P  # cols < c0 fully masked
                        diag = dlt >= 0
                        kslice = kts[jk // 4][:, (jk % 4) * P:(jk % 4 + 1) * P]
                        ps_s = _mk(psp2, [P, 2 * SQ], F32, "ps2")
                        for hh, h in enumerate(heads):
                            o = hh * SQ
                            nc.tensor.matmul(
                                ps_s[:, o + c0:o + SQ], kslice,
                                qth[h][q][:, c0:],
                                start=True, stop=not diag)
                            if diag:
                                # accumulate -30000 on the masked (k>q) part
                                nc.tensor.matmul(
                                    ps_s[:, o + c0:o + c0 + P], ident,
                                    mtri_sb, start=False, stop=True)
                        pr2 = _mk(prp, [P, 2 * SQ], BF16, "pr")
                        nc.scalar.activation(
                            pr2[:, c0:], ps_s[:, c0:],
                            mybir.ActivationFunctionType.Exp, scale=SCALE)
                        pend.append((jk, c0, pr2))
                        if len(pend) > 4:
                            pjk, pc0, ppr = pend.pop(0)
                            for hh, h in enumerate(heads):
                                o = hh * SQ
                                nc.tensor.matmul(
                                    ps_ctx[h][:, pc0:], vs[pjk],
                                    ppr[:, o + pc0:o + SQ],
                                    start=(pjk == 0), stop=False)
                    while pend:
                        pjk, pc0, ppr = pend.pop(0)
                        for hh, h in enumerate(heads):
                            o = hh * SQ
                            nc.tensor.matmul(
                                ps_ctx[h][:, pc0:], vs[pjk],
                                ppr[:, o + pc0:o + SQ],
                                start=(pjk == 0), stop=(not pend))
                    # per head: den recip (bf16) -> PE broadcast -> ctxg
                    for h in heads:
                        ho = (h % 2) * HD
                        denb = _mk(sp, [1, SQ], BF16, "denb")
                        with nc.allow_low_precision("softmax denom bf16"):
                            nc.vector.reciprocal(denb,
                                                 ps_ctx[h][HD:HD + 1, :])
                        ps_db = _mk(psp, [HD, SQ], F32, "ps")
                        nc.tensor.matmul(ps_db, halfones[:, :HD], denb,
                                         start=True, stop=True)
                        # gud = (tanh(g/2) + 1) * (0.5/den)
                        gud = _mk(wp, [HD, SQ], BF16, "gud")
                        nc.vector.scalar_tensor_tensor(
                            gud, gus[hp][q][ho:ho + HD, :], 1.0, ps_db,
                            mybir.AluOpType.add, mybir.AluOpType.mult)
                        nc.vector.tensor_mul(
                            ctxgs[hp][q][ho:ho + HD, :],
                            ps_ctx[h][:HD, :], gud)
                for g in fqueue:
                    g()

            def f_groups(q):
                """Output projection for sq slice q as 16 deferred groups,
                interleaved one-per-jk-step into the next e_slice."""
                groups = []
                for jj in range(4):
                    for n in range(NSQ):
                        def emit(jj=jj, n=n):
                            jc = jj * P
                            j = 4 * q + jj
                            ps_o = _mk(psp, [P, SQ], F32, "ps")
                            for e in range(2):
                                nc.tensor.matmul(
                                    ps_o, ctxgs[e][q][:, jc:jc + P],
                                    wos[e][:, n * SQ:(n + 1) * SQ],
                                    start=(e == 0), stop=(e == 1))
                            oc = _mk(ocp, [P, SQ], BF16, "oc")
                            nc.vector.tensor_copy(out=oc, in_=ps_o)
                            nc.sync.dma_start(
                                out=out[j * P:(j + 1) * P,
                                        n * SQ:(n + 1) * SQ],
                                in_=oc)
                        groups.append(emit)
                return groups

            # ---- schedule: gate sweeps fill the DMA window; the last 8
            # qkv groups and all out-projection groups are fed one-per-jk-
            # step into the attention slices, so the in-order PE program
            # stays dense from DMA arrival to the final output DMA ----
            _mark(nc, "phaseD0")
            g0 = d_sweep_mm(0, (0, 1))
            _mark(nc, "phaseA")
            for j in range(4):
                a_group(j)
            g1 = d_sweep_mm(1, (0, 1))
            d_sweep_act(0, (0, 1), g0)
            _mark(nc, "phaseB")
            b_chain(0, 4)
            a_group(4)
            a_group(5)
            _mark(nc, "phaseC0")
            c_tile(0)
            c_tile(1)
            a_group(6)
            d_sweep_act(1, (0, 1), g1)
            c_tile(2)
            c_tile(3)
            a_group(7)
            b_chain(4, 8)
            _mark(nc, "phaseE0")
            e_slice(0, [lambda j=j: a_group(j) for j in range(8, 12)])
            b_chain(8, 12)
            _mark(nc, "phaseC1")
            for j in range(4, 8):
                c_tile(j)
            _mark(nc, "phaseD1")
            d_sweep(0, (2, 3))
            _mark(nc, "phaseE1")
            e_slice(1, [lambda j=j: a_group(j) for j in range(12, 16)]
                    + f_groups(0))
            b_chain(12, 16)
            _mark(nc, "phaseC2")
            for j in range(8, 12):
                c_tile(j)
            _mark(nc, "phaseD2")
            d_sweep(1, (2, 3))
            _mark(nc, "phaseE2")
            e_slice(2, f_groups(1))
            _mark(nc, "phaseC3")
            for j in range(12, 16):
                c_tile(j)
            _mark(nc, "phaseE3")
            e_slice(3, f_groups(2))
            for g in f_groups(3):
                g()

    nc.compile()
    return nc


def prep_inputs(x, cos, sin, Wq, Wk, Wv, Wo, q_norm_w, k_norm_w):
    """Host-side shard + layout prep. Returns per-core input maps."""
    xtn = x.reshape(S, D).T.astype(NBF)

    # rope tables with (1 + norm_w) folded in, k-block appended, and the
    # sin first-half pre-negated (so rope is out = q*cos5 + rot(q)*sin5
    # with rot(q) = [q2, q1])
    half = HD // 2
    wq1 = (1.0 + q_norm_w).astype(np.float32)
    wk1 = (1.0 + k_norm_w).astype(np.float32)

    def rotw(w):
        return np.concatenate([w[half:], w[:half]])

    sin_m = sin.copy()
    sin_m[:, :half] = -sin_m[:, :half]
    cos_q = cos * wq1
    cos_k = cos * wk1
    sin_q = sin_m * rotw(wq1)
    sin_k = sin_m * rotw(wk1)
    cos5 = np.concatenate([np.tile(cos_q, (1, NHL)), cos_k], axis=1)
    sin5 = np.concatenate([np.tile(sin_q, (1, NHL)), sin_k], axis=1)
    cs = np.ascontiguousarray(
        np.concatenate([cos5, sin5], axis=1)).astype(NBF)

    # strict lower triangle (k > q within the diagonal block) gets -30000,
    # accumulated into the scores psum before exp
    mtri = (np.tril(np.full((P, P), NEG, dtype=np.float32), k=-1)).astype(NBF)

    Wqh = Wq.reshape(H, 2 * HD, D)
    in_maps = []
    for c in range(NCORE):
        hs = slice(NHL * c, NHL * (c + 1))
        wq_c = Wqh[hs, :HD, :].reshape(EL, D)       # q rows, 4 heads
        wgt_c = Wqh[hs, HD:, :].reshape(EL, D)      # gate rows
        wk_c = Wk[HD * c:HD * (c + 1), :]
        wv_c = Wv[HD * c:HD * (c + 1), :]
        # [640, D]: q | k | v | gate
        wqg_c = np.concatenate([wq_c, wk_c, wv_c, wgt_c], axis=0)
        xw_c = np.ascontiguousarray(
            np.concatenate([xtn, wqg_c.T.astype(NBF)], axis=1))
        in_maps.append({
            "xw": xw_c,
            "wo": np.ascontiguousarray(
                Wo[:, EL * c:EL * (c + 1)].T).astype(NBF),
            "cs": cs,
            "mtri": mtri,
        })
    return in_maps


_NC_CACHE = {}


def get_nc():
    if "nc" not in _NC_CACHE:
        _NC_CACHE["nc"] = build_nc()
    return _NC_CACHE["nc"]


def run(in_maps, trace=False, **kw):
    nc = get_nc()
    return run_bass_kernel_spmd(nc, in_maps, list(range(NCORE)),
                                trace=trace, **kw)


def kernel(x, mask, cos, sin, Wq, Wk, Wv, Wo, q_norm_w, k_norm_w):
    in_maps = prep_inputs(np.asarray(x, dtype=np.float32), np.asarray(cos),
                          np.asarray(sin), np.asarray(Wq), np.asarray(Wk),
                          np.asarray(Wv), np.asarray(Wo),
                          np.asarray(q_norm_w), np.asarray(k_norm_w))
    res = run(in_maps)
    acc = np.zeros((S, D), dtype=np.float32)
    for r in res.results:
        acc += np.asarray(r["out"], dtype=np.float32)
    return acc.reshape(1, S, D)


# revision 51
# speedup vs baseline: 1.1389x; 1.0817x over previous
"""GQA attention block (B=1, S=2048, D=2048, H=32, G=8, HD=64) on 8 trn2 cores.

Sharding: tensor-parallel over heads/KV-groups. Core c owns q-heads
4c..4c+3 and KV group c. Wq/Wk/Wv column-parallel, Wo row-parallel;
each core computes a partial [S, D] output, host sums the 8 partials.

Per-core dataflow (all matmuls bf16, stats f32), creation-ordered for
engine overlap (the tile scheduler keeps per-engine programs roughly in
creation order, so independent work is interleaved at emission time):
  DMAs interleaved per d-tile and split over both HWDGE queues
  (SP + ACT) so x streams in at 2x; wqkv|wg merged into one tensor.
  D (gate proj, [e,s] layout): 4 sweeps of (p, q-pair); first two fill
    the x-DMA window, last two fill later PE gaps. gus = tanh(g/2).
  A: qkv proj per s-tile -> psum [s,384]; ACT: square + raw-qk copy;
    DVE: v copy + block row-sums into one batched stats tile.
  B: one batched Newton-rsqrt chain over all stats (no per-tile chain).
  C: norm-mul + rope split over gpsimd and DVE + PE transposes to
    [hd,s]; psum->sbuf copies split over ACT and DVE.
  E: per (q-slice, head-pair): software-pipelined over k-tiles with
    depth 4 — scores/exp for tile jk are emitted well before the ctx
    matmuls of jk-4, so the in-order PE never waits on the exp. The two
    heads of a pair share one [128,1024] scores psum and a single exp.
    Causal mask by accumulating a -30000 strict-lower-tri matmul on
    diagonal blocks; probs = exp(scale*scores) on ACT (c0-trimmed);
    ctxT (+den row) = [v|1].T @ probsT with trimmed widths. Per head:
    den recip (bf16, low-precision ok) -> PE broadcast; gud =
    (tanh+1)*(0.5/den) via one scalar_tensor_tensor; ctxg = ctx * gud.
  F: out[s,dout] = ctxg.T @ woT; its 16 groups per q-slice are fed
    one-per-jk-step into the NEXT slice's attention loop (fqueue), as
    are the last 8 qkv groups, keeping the PE program dense end-to-end;
    psum->bf16 copies on DVE, partials summed on host.
"""

import numpy as np
import ml_dtypes

import concourse.bass as bass
import concourse.tile as tile
from concourse import bacc, mybir
from concourse.bass_utils import run_bass_kernel_spmd
from concourse.masks import make_identity

BF16 = mybir.dt.bfloat16
F32 = mybir.dt.float32
NBF = ml_dtypes.bfloat16

S = 2048
D = 2048
H = 32
G = 8
HD = 64
NCORE = 8
NHL = H // NCORE          # 4 q heads per core
EL = NHL * HD             # 256 local q (and gate, and ctx) features
QK = EL + HD              # 320: q + k features
QKV = QK + HD             # 384: q + k + v
QG = QKV + EL             # 640: qkv + gate columns in the merged weight
P = 128
NS = S // P               # 16 s-tiles
ND = D // P               # 16 d-tiles
SQ = 512
NSQ = S // SQ             # 4 sq slices
NB = QK // HD             # 5 (hd,) blocks in the q|k strip
SCALE = HD ** -0.5
EPS = 1e-6
NEG = -30000.0


def _v(ap, dims, extra_offset=0):
    """Reshape the free dims of a 2D AP into `dims` ([step, count] pairs),
    keeping the partition dim."""
    return bass.AP(
        tensor=ap.tensor,
        offset=ap.offset + extra_offset,
        ap=[list(ap.ap[0])] + [list(d) for d in dims],
    )


def _mk(pool, shape, dtype, tag):
    return pool.tile(shape, dtype, tag=tag, name=tag)


PHASES = []  # (phase_name, first_instruction_index) — debug aid for sim.py


def _mark(nc, name):
    PHASES.append((name, int(nc.get_next_instruction_name().split("-")[1])))


def build_nc():
    nc = bacc.Bacc("TRN2", target_bir_lowering=False, debug=False,
                   num_devices=NCORE)

    xw = nc.dram_tensor("xw", [D, S + QG], BF16, kind="ExternalInput").ap()
    wo = nc.dram_tensor("wo", [EL, D], BF16, kind="ExternalInput").ap()
    cs = nc.dram_tensor("cs", [S, 2 * QK], BF16, kind="ExternalInput").ap()
    mtri = nc.dram_tensor("mtri", [P, P], BF16, kind="ExternalInput").ap()
    out = nc.dram_tensor("out", [S, D], BF16, kind="ExternalOutput").ap()

    with tile.TileContext(nc) as tc:
        with (
            tc.tile_pool(name="persist", bufs=1) as pp,
            tc.tile_pool(name="work", bufs=3) as wp,
            tc.tile_pool(name="stats", bufs=2) as sp,
            tc.tile_pool(name="qkr", bufs=3) as qp,
            tc.tile_pool(name="probs", bufs=8) as prp,
            tc.tile_pool(name="outc", bufs=2) as ocp,
            tc.tile_pool(name="psum", bufs=4, space="PSUM") as psp,
            tc.tile_pool(name="psum2", bufs=2, space="PSUM") as psp2,
        ):
            # ---- persistent loads: one DMA per d-tile (x row-block and
            # its weight row-block merged host-side into one dram tensor) --
            xts, wqgs = [], []
            for i in range(ND):
                t = _mk(pp, [P, S + QG], BF16, f"xw{i}")
                nc.sync.dma_start(out=t, in_=xw[i * P:(i + 1) * P, :])
                xts.append(t[:, :S])
                wqgs.append(t[:, S:])
            # cs/mtri/wo DMAs are deferred until after the x stream —
            # every dma_start holds the shared HWDGE ~625ns, and issuing
            # these early interleaves them into the critical xw cadence
            css = []
            for j in range(NS):
                css.append(_mk(pp, [P, 2 * QK], BF16, f"cs{j}"))
            mtri_sb = _mk(pp, [P, P], BF16, "mtri")
            wos = [_mk(pp, [P, D], BF16, f"wo{e}") for e in range(2)]

            def aux_dmas():
                for j in range(NS):
                    nc.sync.dma_start(out=css[j],
                                      in_=cs[j * P:(j + 1) * P, :])
                nc.sync.dma_start(out=mtri_sb, in_=mtri)
                for e in range(2):
                    nc.sync.dma_start(out=wos[e],
                                      in_=wo[e * P:(e + 1) * P, :])

            ident = _mk(pp, [P, P], BF16, "ident")
            make_identity(nc, ident)
            halfones = _mk(pp, [1, P], BF16, "halfones")
            nc.vector.memset(halfones, 0.5)

            # persistent intermediate tensors
            qth = [[_mk(pp, [HD, SQ], BF16, f"qt{h}_{q}") for q in range(NSQ)]
                   for h in range(NHL)]
            kts = [_mk(pp, [HD, SQ], BF16, f"kt{q}") for q in range(NSQ)]
            vs = [_mk(pp, [P, HD + 1], BF16, f"v{j}") for j in range(NS)]
            gus = [[_mk(pp, [P, SQ], BF16, f"gu{p}_{q}") for q in range(NSQ)]
                   for p in range(2)]
            ctxgs = [[_mk(pp, [P, SQ], BF16, f"cg{p}_{q}") for q in range(NSQ)]
                     for p in range(2)]
            qk_all = [_mk(pp, [P, QK], BF16, f"qk{j}") for j in range(NS)]
            ss_all = _mk(pp, [P, NB * NS], F32, "ss_all")
            y_all = _mk(pp, [P, NB * NS], F32, "y_all")

            def d_sweep_mm(p, qs):
                """Gate projection sweep matmuls: fixed p, q-pair qs in one
                2-bank tile from the scores pool (free outside attention)."""
                ps_g = _mk(psp2, [P, 2 * SQ], F32, "ps2")
                for i in range(ND):
                    for o, qq in enumerate(qs):
                        nc.tensor.matmul(
                            ps_g[:, o * SQ:(o + 1) * SQ],
                            wqgs[i][:, QKV + p * P:QKV + (p + 1) * P],
                            xts[i][:, qq * SQ:(qq + 1) * SQ],
                            start=(i == 0), stop=(i == ND - 1))
                return ps_g

            def d_sweep_act(p, qs, ps_g):
                """Deferred tanh part of a gate sweep (keeps the in-order
                ACT queue from blocking later work on the sweep's finish)."""
                for o, qq in enumerate(qs):
                    # gus = tanh(g/2); the (1 + .)*0.5/den fold happens in E
                    nc.scalar.activation(gus[p][qq],
                                         ps_g[:, o * SQ:(o + 1) * SQ],
                                         mybir.ActivationFunctionType.Tanh,
                                         scale=0.5)

            def d_sweep(p, qs):
                d_sweep_act(p, qs, d_sweep_mm(p, qs))


            def a_group(j):
                """QKV projection + stats for s-tile j."""
                ps_qkv = _mk(psp, [P, QKV], F32, "ps")
                for i in range(ND):
                    nc.tensor.matmul(
                        ps_qkv, xts[i][:, j * P:(j + 1) * P],
                        wqgs[i][:, :QKV],
                        start=(i == 0), stop=(i == ND - 1))
                # v (+ ones column) straight to SBUF
                nc.vector.tensor_copy(out=vs[j][:, :HD], in_=ps_qkv[:, QK:QKV])
                nc.vector.memset(vs[j][:, HD:HD + 1], 1.0)
                # squares + raw qk copy on ACT, block row-sums on DVE
                qk = ps_qkv[:, :QK]
                sqr = _mk(wp, [P, QK], F32, "sqr")
                nc.scalar.activation(sqr, qk,
                                     mybir.ActivationFunctionType.Square)
                nc.vector.tensor_reduce(
                    ss_all[:, NB * j:NB * (j + 1)],
                    _v(sqr, [[HD, NB], [1, HD]]),
                    axis=mybir.AxisListType.X, op=mybir.AluOpType.add)
                nc.scalar.copy(qk_all[j], qk)

            def b_chain(lo, hi, eng=None):
                """Batched Newton rsqrt for s-tiles [lo, hi). DVE only:
                tensor_scalar/scalar_tensor_tensor are not legal Pool-engine
                ops (walrus NCC_IXCG966), so no gpsimd half here."""
                if eng is None:
                    eng = nc.vector
                c0, c1 = NB * lo, NB * hi
                n = c1 - c0
                ss = ss_all[:, c0:c1]
                y = y_all[:, c0:c1]
                m = _mk(sp, [P, n], F32, "m")
                eng.tensor_scalar(m, ss, 1.0 / HD, EPS,
                                        mybir.AluOpType.mult,
                                        mybir.AluOpType.add)
                mc = _mk(sp, [P, n], F32, "mc")
                eng.tensor_scalar(mc, m, 5.5, 0.45,
                                        mybir.AluOpType.min,
                                        mybir.AluOpType.max)
                m2 = _mk(sp, [P, n], F32, "m2")
                eng.tensor_mul(m2, mc, mc)
                lin = _mk(sp, [P, n], F32, "lin")
                eng.tensor_scalar(lin, mc, -0.48330447, 1.51774376,
                                        mybir.AluOpType.mult,
                                        mybir.AluOpType.add)
                eng.scalar_tensor_tensor(y, m2, 0.0534932, lin,
                                               mybir.AluOpType.mult,
                                               mybir.AluOpType.add)
                ytmp = _mk(sp, [P, n], F32, "ytmp")
                for _ in range(3):
                    eng.tensor_mul(ytmp, y, y)              # y^2
                    eng.tensor_mul(ytmp, ytmp, m)           # m y^2
                    eng.tensor_scalar(ytmp, ytmp, -0.5, 1.5,
                                            mybir.AluOpType.mult,
                                            mybir.AluOpType.add)
                    eng.tensor_mul(y, y, ytmp)

            def c_rope(j, dve=False):
                """Norm + rope for s-tile j (gpsimd + DVE only, no PE).

                qkn/t1 on gpsimd, t2/qkr on DVE — splits the serial chain
                across two engines and halves each one's load. Returns the
                roped tile for the deferred transpose."""
                eng = nc.vector if dve else nc.gpsimd
                yb = _v(y_all[:, NB * j:NB * (j + 1)], [[1, NB], [0, HD]])
                qkn = _mk(wp, [P, QK], BF16, "qkn")
                eng.tensor_mul(
                    _v(qkn, [[HD, NB], [1, HD]]),
                    _v(qk_all[j][:, :], [[HD, NB], [1, HD]]), yb)
                # rope: out = qkn*cos5 + rot(qkn)*sin5  (sin pre-negated on
                # the first half on host; cos/sin already include 1+norm_w)
                t1 = _mk(wp, [P, QK], BF16, "t1")
                eng.tensor_mul(t1, qkn, css[j][:, :QK])
                t2 = _mk(wp, [P, QK], BF16, "t2")
                rot = _v(qkn[:, :], [[HD, NB], [-32, 2], [1, 32]],
                         extra_offset=32)
                nc.vector.tensor_mul(
                    _v(t2, [[HD, NB], [32, 2], [1, 32]]), rot,
                    _v(css[j][:, QK:], [[HD, NB], [32, 2], [1, 32]]))
                qkr = _mk(qp, [P, QK], BF16, "qkr")
                nc.vector.tensor_add(qkr, t1, t2)
                return qkr

            def c_trans(j, qkr, act=False):
                """Transpose q packs and k of s-tile j into [hd, s]."""
                cp = (nc.scalar.copy if act else
                      (lambda o, i: nc.vector.tensor_copy(out=o, in_=i)))
                jq, jc = j // 4, (j % 4) * P
                for p in range(2):
                    pt = _mk(psp, [P, P], BF16, "ps")
                    nc.tensor.transpose(pt, qkr[:, p * P:(p + 1) * P], ident)
                    cp(qth[2 * p][jq][:, jc:jc + P], pt[:HD, :])
                    cp(qth[2 * p + 1][jq][:, jc:jc + P], pt[HD:, :])
                ptk = _mk(psp, [HD, P], BF16, "ps")
                nc.tensor.transpose(ptk, qkr[:, 2 * P:2 * P + HD], ident)
                cp(kts[jq][:, jc:jc + P], ptk)

            def c_tile(j, dve=False, act=False):
                c_trans(j, c_rope(j, dve), act=act)

            def c_fillers(js, act=False):
                """Rope now (no PE), transposes as filler closures."""
                roped = [(j, c_rope(j)) for j in js]
                return [lambda j=j, r=r: c_trans(j, r, act=act)
                        for j, r in roped]

            def e_slice(q, fqueue=()):
                """Attention for sq slice q, head pairs, sw-pipelined.

                The two heads of a pair share one [128, 2*SQ] scores psum
                (2 banks) and a single exp instruction; the masked prefix of
                the second half holds junk that the trimmed ctx matmuls
                never read."""
                nks = 4 * q + 4   # sk tiles 0..nks-1 intersect causally
                fqueue = list(fqueue)
                for hp in range(2):
                    heads = (2 * hp, 2 * hp + 1)
                    ps_ctx = {h: _mk(psp, [HD + 1, SQ], F32, "ps")
                              for h in heads}
                    pend = []   # [(jk, c0, pr2)] pending ctx, depth 2
                    for jk in range(nks):
                        if fqueue:
                            fqueue.pop(0)()
                        dlt = jk - 4 * q
                        c0 = max(dlt, 0) * P  # cols < c0 fully masked
                        diag = dlt >= 0
                        kslice = kts[jk // 4][:, (jk % 4) * P:(jk % 4 + 1) * P]
                        ps_s = _mk(psp2, [P, 2 * SQ], F32, "ps2")
                        for hh, h in enumerate(heads):
                            o = hh * SQ
                            nc.tensor.matmul(
                                ps_s[:, o + c0:o + SQ], kslice,
                                qth[h][q][:, c0:],
                                start=True, stop=not diag)
                            if diag:
                                # accumulate -30000 on the masked (k>q) part
                                nc.tensor.matmul(
                                    ps_s[:, o + c0:o + c0 + P], ident,
                                    mtri_sb, start=False, stop=True)
                        pr2 = _mk(prp, [P, 2 * SQ], BF16, "pr")
                        nc.scalar.activation(
                            pr2[:, c0:], ps_s[:, c0:],
                            mybir.ActivationFunctionType.Exp, scale=SCALE)
                        pend.append((jk, c0, pr2))
                        if len(pend) > 4:
                            pjk, pc0, ppr = pend.pop(0)
                            for hh, h in enumerate(heads):
                                o = hh * SQ
                                nc.tensor.matmul(
                                    ps_ctx[h][:, pc0:], vs[pjk],
                                    ppr[:, o + pc0:o + SQ],
                                    start=(pjk == 0), stop=False)
                    while pend:
                        pjk, pc0, ppr = pend.pop(0)
                        for hh, h in enumerate(heads):
                            o = hh * SQ
                            nc.tensor.matmul(
                                ps_ctx[h][:, pc0:], vs[pjk],
                                ppr[:, o + pc0:o + SQ],
                                start=(pjk == 0), stop=(not pend))
                    # per head: den recip (bf16) -> PE broadcast -> ctxg
                    for h in heads:
                        ho = (h % 2) * HD
                        denb = _mk(sp, [1, SQ], BF16, "denb")
                        with nc.allow_low_precision("softmax denom bf16"):
                            nc.vector.reciprocal(denb,
                                                 ps_ctx[h][HD:HD + 1, :])
                        ps_db = _mk(psp, [HD, SQ], F32, "ps")
                        nc.tensor.matmul(ps_db, halfones[:, :HD], denb,
                                         start=True, stop=True)
                        # gud = (tanh(g/2) + 1) * (0.5/den)
                        gud = _mk(wp, [HD, SQ], BF16, "gud")
                        nc.vector.scalar_tensor_tensor(
                            gud, gus[hp][q][ho:ho + HD, :], 1.0, ps_db,
                            mybir.AluOpType.add, mybir.AluOpType.mult)
                        nc.vector.tensor_mul(
                            ctxgs[hp][q][ho:ho + HD, :],
                            ps_ctx[h][:HD, :], gud)
                for g in fqueue:
                    g()

            def f_groups(q, alt_copies=False):
                """Output projection for sq slice q as 16 deferred groups,
                interleaved one-per-jk-step into the next e_slice. The four
                n-slices of one s-row-block share an [128, 2048] staging
                tile and a single out-DMA (HWDGE holds ~625ns per dma_start
                regardless of size, so fewer+bigger DMAs win)."""
                groups = []
                for jj in range(4):
                    oc4_box = {}
                    for n in range(NSQ):
                        def emit(jj=jj, n=n, oc4_box=oc4_box):
                            jc = jj * P
                            j = 4 * q + jj
                            ps_o = _mk(psp, [P, SQ], F32, "ps")
                            for e in range(2):
                                nc.tensor.matmul(
                                    ps_o, ctxgs[e][q][:, jc:jc + P],
                                    wos[e][:, n * SQ:(n + 1) * SQ],
                                    start=(e == 0), stop=(e == 1))
                            if n == 0:
                                oc4_box["t"] = _mk(ocp, [P, 4 * SQ], BF16,
                                                   "oc4")
                            oc4 = oc4_box["t"]
                            dst = oc4[:, n * SQ:(n + 1) * SQ]
                            if alt_copies and n % 2 == 0:
                                nc.scalar.copy(dst, ps_o)
                            else:
                                nc.vector.tensor_copy(out=dst, in_=ps_o)
                            if n == NSQ - 1:
                                nc.sync.dma_start(
                                    out=out[j * P:(j + 1) * P, :], in_=oc4)
                        groups.append(emit)
                return groups

            # ---- schedule: gate sweeps fill the DMA window; the last 8
            # qkv groups and all out-projection groups are fed one-per-jk-
            # step into the attention slices, so the in-order PE program
            # stays dense from DMA arrival to the final output DMA ----
            _mark(nc, "phaseD0")
            g0 = d_sweep_mm(0, (0, 1))
            _mark(nc, "phaseA")
            for j in range(4):
                a_group(j)
            g1 = d_sweep_mm(1, (0, 1))
            aux_dmas()
            d_sweep_act(0, (0, 1), g0)
            _mark(nc, "phaseB")
            b_chain(0, 4)
            a_group(4)
            a_group(5)
            a_group(6)
            d_sweep_act(1, (0, 1), g1)
            a_group(7)
            b_chain(4, 8)
            _mark(nc, "phaseC0")
            for j in range(4):
                c_tile(j, dve=(j % 2 == 1))
            ct47 = c_fillers(range(4, 8))
            _mark(nc, "phaseE0")
            e_slice(0, [lambda j=j: a_group(j) for j in range(8, 12)]
                    + ct47)
            b_chain(8, 12)
            ct811 = c_fillers(range(8, 12))
            _mark(nc, "phaseD1")
            d_sweep(0, (2, 3))
            _mark(nc, "phaseE1")
            e_slice(1, [lambda j=j: a_group(j) for j in range(12, 16)]
                    + ct811 + f_groups(0, alt_copies=True))
            b_chain(12, 16)
            ct1215 = c_fillers(range(12, 16))
            _mark(nc, "phaseD2")
            d_sweep(1, (2, 3))
            _mark(nc, "phaseE2")
            e_slice(2, ct1215 + f_groups(1))
            _mark(nc, "phaseE3")
            e_slice(3, f_groups(2))
            for g in f_groups(3, alt_copies=True):
                g()

    nc.compile()
    return nc


def prep_inputs(x, cos, sin, Wq, Wk, Wv, Wo, q_norm_w, k_norm_w):
    """Host-side shard + layout prep. Returns per-core input maps."""
    xtn = x.reshape(S, D).T.astype(NBF)

    # rope tables with (1 + norm_w) folded in, k-block appended, and the
    # sin first-half pre-negated (so rope is out = q*cos5 + rot(q)*sin5
    # with rot(q) = [q2, q1])
    half = HD // 2
    wq1 = (1.0 + q_norm_w).astype(np.float32)
    wk1 = (1.0 + k_norm_w).astype(np.float32)

    def rotw(w):
        return np.concatenate([w[half:], w[:half]])

    sin_m = sin.copy()
    sin_m[:, :half] = -sin_m[:, :half]
    cos_q = cos * wq1
    cos_k = cos * wk1
    sin_q = sin_m * rotw(wq1)
    sin_k = sin_m * rotw(wk1)
    cos5 = np.concatenate([np.tile(cos_q, (1, NHL)), cos_k], axis=1)
    sin5 = np.concatenate([np.tile(sin_q, (1, NHL)), sin_k], axis=1)
    cs = np.ascontiguousarray(
        np.concatenate([cos5, sin5], axis=1)).astype(NBF)

    # strict lower triangle (k > q within the diagonal block) gets -30000,
    # accumulated into the scores psum before exp
    mtri = (np.tril(np.full((P, P), NEG, dtype=np.float32), k=-1)).astype(NBF)

    Wqh = Wq.reshape(H, 2 * HD, D)
    in_maps = []
    for c in range(NCORE):
        hs = slice(NHL * c, NHL * (c + 1))
        wq_c = Wqh[hs, :HD, :].reshape(EL, D)       # q rows, 4 heads
        wgt_c = Wqh[hs, HD:, :].reshape(EL, D)      # gate rows
        wk_c = Wk[HD * c:HD * (c + 1), :]
        wv_c = Wv[HD * c:HD * (c + 1), :]
        # [640, D]: q | k | v | gate
        wqg_c = np.concatenate([wq_c, wk_c, wv_c, wgt_c], axis=0)
        xw_c = np.ascontiguousarray(
            np.concatenate([xtn, wqg_c.T.astype(NBF)], axis=1))
        in_maps.append({
            "xw": xw_c,
            "wo": np.ascontiguousarray(
                Wo[:, EL * c:EL * (c + 1)].T).astype(NBF),
            "cs": cs,
            "mtri": mtri,
        })
    return in_maps


_NC_CACHE = {}


def get_nc():
    if "nc" not in _NC_CACHE:
        _NC_CACHE["nc"] = build_nc()
    return _NC_CACHE["nc"]


def run(in_maps, trace=False, **kw):
    nc = get_nc()
    return run_bass_kernel_spmd(nc, in_maps, list(range(NCORE)),
                                trace=trace, **kw)


def kernel(x, mask, cos, sin, Wq, Wk, Wv, Wo, q_norm_w, k_norm_w):
    in_maps = prep_inputs(np.asarray(x, dtype=np.float32), np.asarray(cos),
                          np.asarray(sin), np.asarray(Wq), np.asarray(Wk),
                          np.asarray(Wv), np.asarray(Wo),
                          np.asarray(q_norm_w), np.asarray(k_norm_w))
    res = run(in_maps)
    acc = np.zeros((S, D), dtype=np.float32)
    for r in res.results:
        acc += np.asarray(r["out"], dtype=np.float32)
    return acc.reshape(1, S, D)


# revision 52
# speedup vs baseline: 1.1410x; 1.0019x over previous
"""GQA attention block (B=1, S=2048, D=2048, H=32, G=8, HD=64) on 8 trn2 cores.

Sharding: tensor-parallel over heads/KV-groups. Core c owns q-heads
4c..4c+3 and KV group c. Wq/Wk/Wv column-parallel, Wo row-parallel;
each core computes a partial [S, D] output, host sums the 8 partials.

Per-core dataflow (all matmuls bf16, stats f32), creation-ordered for
engine overlap (the tile scheduler keeps per-engine programs roughly in
creation order, so independent work is interleaved at emission time):
  DMAs interleaved per d-tile and split over both HWDGE queues
  (SP + ACT) so x streams in at 2x; wqkv|wg merged into one tensor.
  D (gate proj, [e,s] layout): 4 sweeps of (p, q-pair); first two fill
    the x-DMA window, last two fill later PE gaps. gus = tanh(g/2).
  A: qkv proj per s-tile -> psum [s,384]; ACT: square + raw-qk copy;
    DVE: v copy + block row-sums into one batched stats tile.
  B: one batched Newton-rsqrt chain over all stats (no per-tile chain).
  C: norm-mul + rope split over gpsimd and DVE + PE transposes to
    [hd,s]; psum->sbuf copies split over ACT and DVE.
  E: per (q-slice, head-pair): software-pipelined over k-tiles with
    depth 4 — scores/exp for tile jk are emitted well before the ctx
    matmuls of jk-4, so the in-order PE never waits on the exp. The two
    heads of a pair share one [128,1024] scores psum and a single exp.
    Causal mask by accumulating a -30000 strict-lower-tri matmul on
    diagonal blocks; probs = exp(scale*scores) on ACT (c0-trimmed);
    ctxT (+den row) = [v|1].T @ probsT with trimmed widths. Per head:
    den recip (bf16, low-precision ok) -> PE broadcast; gud =
    (tanh+1)*(0.5/den) via one scalar_tensor_tensor; ctxg = ctx * gud.
  F: out[s,dout] = ctxg.T @ woT; its 16 groups per q-slice are fed
    one-per-jk-step into the NEXT slice's attention loop (fqueue), as
    are the last 8 qkv groups, keeping the PE program dense end-to-end;
    psum->bf16 copies on DVE, partials summed on host.
"""

import numpy as np
import ml_dtypes

import concourse.bass as bass
import concourse.tile as tile
from concourse import bacc, mybir
from concourse.bass_utils import run_bass_kernel_spmd
from concourse.masks import make_identity

BF16 = mybir.dt.bfloat16
F32 = mybir.dt.float32
NBF = ml_dtypes.bfloat16

S = 2048
D = 2048
H = 32
G = 8
HD = 64
NCORE = 8
NHL = H // NCORE          # 4 q heads per core
EL = NHL * HD             # 256 local q (and gate, and ctx) features
QK = EL + HD              # 320: q + k features
QKV = QK + HD             # 384: q + k + v
QG = QKV + EL             # 640: qkv + gate columns in the merged weight
P = 128
NS = S // P               # 16 s-tiles
ND = D // P               # 16 d-tiles
SQ = 512
NSQ = S // SQ             # 4 sq slices
NB = QK // HD             # 5 (hd,) blocks in the q|k strip
SCALE = HD ** -0.5
EPS = 1e-6
NEG = -30000.0


def _v(ap, dims, extra_offset=0):
    """Reshape the free dims of a 2D AP into `dims` ([step, count] pairs),
    keeping the partition dim."""
    return bass.AP(
        tensor=ap.tensor,
        offset=ap.offset + extra_offset,
        ap=[list(ap.ap[0])] + [list(d) for d in dims],
    )


def _mk(pool, shape, dtype, tag):
    return pool.tile(shape, dtype, tag=tag, name=tag)


PHASES = []  # (phase_name, first_instruction_index) — debug aid for sim.py


def _mark(nc, name):
    PHASES.append((name, int(nc.get_next_instruction_name().split("-")[1])))


def build_nc():
    nc = bacc.Bacc("TRN2", target_bir_lowering=False, debug=False,
                   num_devices=NCORE)

    xw = nc.dram_tensor("xw", [D, S + QG], BF16, kind="ExternalInput").ap()
    wo = nc.dram_tensor("wo", [EL, D], BF16, kind="ExternalInput").ap()
    cs = nc.dram_tensor("cs", [S, 2 * QK], BF16, kind="ExternalInput").ap()
    mtri = nc.dram_tensor("mtri", [P, P], BF16, kind="ExternalInput").ap()
    out = nc.dram_tensor("out", [S, D], BF16, kind="ExternalOutput").ap()

    with tile.TileContext(nc) as tc:
        with (
            tc.tile_pool(name="persist", bufs=1) as pp,
            tc.tile_pool(name="work", bufs=3) as wp,
            tc.tile_pool(name="stats", bufs=2) as sp,
            tc.tile_pool(name="qkr", bufs=3) as qp,
            tc.tile_pool(name="probs", bufs=8) as prp,
            tc.tile_pool(name="outc", bufs=2) as ocp,
            tc.tile_pool(name="psum", bufs=4, space="PSUM") as psp,
            tc.tile_pool(name="psum2", bufs=2, space="PSUM") as psp2,
        ):
            # ---- persistent loads: one DMA per d-tile (x row-block and
            # its weight row-block merged host-side into one dram tensor) --
            xts, wqgs = [], []
            for i in range(ND):
                t = _mk(pp, [P, S + QG], BF16, f"xw{i}")
                nc.sync.dma_start(out=t, in_=xw[i * P:(i + 1) * P, :])
                xts.append(t[:, :S])
                wqgs.append(t[:, S:])
            # cs/mtri/wo DMAs are deferred until after the x stream —
            # every dma_start holds the shared HWDGE ~625ns, and issuing
            # these early interleaves them into the critical xw cadence
            css = []
            for j in range(NS):
                css.append(_mk(pp, [P, 2 * QK], BF16, f"cs{j}"))
            mtri_sb = _mk(pp, [P, P], BF16, "mtri")
            wos = [_mk(pp, [P, D], BF16, f"wo{e}") for e in range(2)]

            def aux_dmas():
                for j in range(NS):
                    nc.sync.dma_start(out=css[j],
                                      in_=cs[j * P:(j + 1) * P, :])
                nc.sync.dma_start(out=mtri_sb, in_=mtri)
                for e in range(2):
                    nc.sync.dma_start(out=wos[e],
                                      in_=wo[e * P:(e + 1) * P, :])

            ident = _mk(pp, [P, P], BF16, "ident")
            make_identity(nc, ident)
            halfones = _mk(pp, [1, P], BF16, "halfones")
            nc.vector.memset(halfones, 0.5)

            # persistent intermediate tensors
            qth = [[_mk(pp, [HD, SQ], BF16, f"qt{h}_{q}") for q in range(NSQ)]
                   for h in range(NHL)]
            kts = [_mk(pp, [HD, SQ], BF16, f"kt{q}") for q in range(NSQ)]
            vs = [_mk(pp, [P, HD + 1], BF16, f"v{j}") for j in range(NS)]
            gus = [[_mk(pp, [P, SQ], BF16, f"gu{p}_{q}") for q in range(NSQ)]
                   for p in range(2)]
            ctxgs = [[_mk(pp, [P, SQ], BF16, f"cg{p}_{q}") for q in range(NSQ)]
                     for p in range(2)]
            qk_all = [_mk(pp, [P, QK], BF16, f"qk{j}") for j in range(NS)]
            ss_all = _mk(pp, [P, NB * NS], F32, "ss_all")
            y_all = _mk(pp, [P, NB * NS], F32, "y_all")

            def d_sweep_mm(p, qs):
                """Gate projection sweep matmuls: fixed p, q-pair qs in one
                2-bank tile from the scores pool (free outside attention)."""
                ps_g = _mk(psp2, [P, 2 * SQ], F32, "ps2")
                for i in range(ND):
                    for o, qq in enumerate(qs):
                        nc.tensor.matmul(
                            ps_g[:, o * SQ:(o + 1) * SQ],
                            wqgs[i][:, QKV + p * P:QKV + (p + 1) * P],
                            xts[i][:, qq * SQ:(qq + 1) * SQ],
                            start=(i == 0), stop=(i == ND - 1))
                return ps_g

            def d_sweep_act(p, qs, ps_g):
                """Deferred tanh part of a gate sweep (keeps the in-order
                ACT queue from blocking later work on the sweep's finish)."""
                for o, qq in enumerate(qs):
                    # gus = tanh(g/2); the (1 + .)*0.5/den fold happens in E
                    nc.scalar.activation(gus[p][qq],
                                         ps_g[:, o * SQ:(o + 1) * SQ],
                                         mybir.ActivationFunctionType.Tanh,
                                         scale=0.5)

            def d_sweep(p, qs):
                d_sweep_act(p, qs, d_sweep_mm(p, qs))


            def a_group(j):
                """QKV projection + stats for s-tile j."""
                ps_qkv = _mk(psp, [P, QKV], F32, "ps")
                for i in range(ND):
                    nc.tensor.matmul(
                        ps_qkv, xts[i][:, j * P:(j + 1) * P],
                        wqgs[i][:, :QKV],
                        start=(i == 0), stop=(i == ND - 1))
                # v (+ ones column) straight to SBUF
                nc.vector.tensor_copy(out=vs[j][:, :HD], in_=ps_qkv[:, QK:QKV])
                nc.vector.memset(vs[j][:, HD:HD + 1], 1.0)
                # squares + raw qk copy on ACT, block row-sums on DVE
                qk = ps_qkv[:, :QK]
                sqr = _mk(wp, [P, QK], F32, "sqr")
                nc.scalar.activation(sqr, qk,
                                     mybir.ActivationFunctionType.Square)
                nc.vector.tensor_reduce(
                    ss_all[:, NB * j:NB * (j + 1)],
                    _v(sqr, [[HD, NB], [1, HD]]),
                    axis=mybir.AxisListType.X, op=mybir.AluOpType.add)
                nc.scalar.copy(qk_all[j], qk)

            def b_chain(lo, hi, eng=None):
                """Batched Newton rsqrt for s-tiles [lo, hi). DVE only:
                tensor_scalar/scalar_tensor_tensor are not legal Pool-engine
                ops (walrus NCC_IXCG966), so no gpsimd half here."""
                if eng is None:
                    eng = nc.vector
                c0, c1 = NB * lo, NB * hi
                n = c1 - c0
                ss = ss_all[:, c0:c1]
                y = y_all[:, c0:c1]
                m = _mk(sp, [P, n], F32, "m")
                eng.tensor_scalar(m, ss, 1.0 / HD, EPS,
                                        mybir.AluOpType.mult,
                                        mybir.AluOpType.add)
                mc = _mk(sp, [P, n], F32, "mc")
                eng.tensor_scalar(mc, m, 5.5, 0.45,
                                        mybir.AluOpType.min,
                                        mybir.AluOpType.max)
                m2 = _mk(sp, [P, n], F32, "m2")
                eng.tensor_mul(m2, mc, mc)
                lin = _mk(sp, [P, n], F32, "lin")
                eng.tensor_scalar(lin, mc, -0.48330447, 1.51774376,
                                        mybir.AluOpType.mult,
                                        mybir.AluOpType.add)
                eng.scalar_tensor_tensor(y, m2, 0.0534932, lin,
                                               mybir.AluOpType.mult,
                                               mybir.AluOpType.add)
                ytmp = _mk(sp, [P, n], F32, "ytmp")
                for _ in range(3):
                    eng.tensor_mul(ytmp, y, y)              # y^2
                    eng.tensor_mul(ytmp, ytmp, m)           # m y^2
                    eng.tensor_scalar(ytmp, ytmp, -0.5, 1.5,
                                            mybir.AluOpType.mult,
                                            mybir.AluOpType.add)
                    eng.tensor_mul(y, y, ytmp)

            def c_rope(j, dve=False):
                """Norm + rope for s-tile j (gpsimd + DVE only, no PE).

                qkn/t1 on gpsimd, t2/qkr on DVE — splits the serial chain
                across two engines and halves each one's load. Returns the
                roped tile for the deferred transpose."""
                eng = nc.vector if dve else nc.gpsimd
                yb = _v(y_all[:, NB * j:NB * (j + 1)], [[1, NB], [0, HD]])
                qkn = _mk(wp, [P, QK], BF16, "qkn")
                eng.tensor_mul(
                    _v(qkn, [[HD, NB], [1, HD]]),
                    _v(qk_all[j][:, :], [[HD, NB], [1, HD]]), yb)
                # rope: out = qkn*cos5 + rot(qkn)*sin5  (sin pre-negated on
                # the first half on host; cos/sin already include 1+norm_w)
                t1 = _mk(wp, [P, QK], BF16, "t1")
                eng.tensor_mul(t1, qkn, css[j][:, :QK])
                t2 = _mk(wp, [P, QK], BF16, "t2")
                rot = _v(qkn[:, :], [[HD, NB], [-32, 2], [1, 32]],
                         extra_offset=32)
                nc.vector.tensor_mul(
                    _v(t2, [[HD, NB], [32, 2], [1, 32]]), rot,
                    _v(css[j][:, QK:], [[HD, NB], [32, 2], [1, 32]]))
                qkr = _mk(qp, [P, QK], BF16, "qkr")
                nc.vector.tensor_add(qkr, t1, t2)
                return qkr

            def c_trans(j, qkr, act=False):
                """Transpose q packs and k of s-tile j into [hd, s]."""
                cp = (nc.scalar.copy if act else
                      (lambda o, i: nc.vector.tensor_copy(out=o, in_=i)))
                jq, jc = j // 4, (j % 4) * P
                for p in range(2):
                    pt = _mk(psp, [P, P], BF16, "ps")
                    nc.tensor.transpose(pt, qkr[:, p * P:(p + 1) * P], ident)
                    cp(qth[2 * p][jq][:, jc:jc + P], pt[:HD, :])
                    cp(qth[2 * p + 1][jq][:, jc:jc + P], pt[HD:, :])
                ptk = _mk(psp, [HD, P], BF16, "ps")
                nc.tensor.transpose(ptk, qkr[:, 2 * P:2 * P + HD], ident)
                cp(kts[jq][:, jc:jc + P], ptk)

            def c_tile(j, dve=False, act=False):
                c_trans(j, c_rope(j, dve), act=act)

            def c_fillers(js, act=False):
                """Rope now (no PE), transposes as filler closures."""
                roped = [(j, c_rope(j)) for j in js]
                return [lambda j=j, r=r: c_trans(j, r, act=act)
                        for j, r in roped]

            def e_slice(q, fqueue=()):
                """Attention for sq slice q, head pairs, sw-pipelined.

                The two heads of a pair share one [128, 2*SQ] scores psum
                (2 banks) and a single exp instruction; the masked prefix of
                the second half holds junk that the trimmed ctx matmuls
                never read."""
                nks = 4 * q + 4   # sk tiles 0..nks-1 intersect causally
                fqueue = list(fqueue)
                deferred = []     # hp0's den/ctxg, emitted inside hp1
                for hp in range(2):
                    heads = (2 * hp, 2 * hp + 1)
                    ps_ctx = {h: _mk(psp, [HD + 1, SQ], F32, "ps")
                              for h in heads}
                    pend = []   # [(jk, c0, pr2)] pending ctx, depth 2
                    for jk in range(nks):
                        if jk == 2 and deferred:
                            deferred.pop(0)()
                        if fqueue:
                            fqueue.pop(0)()
                        dlt = jk - 4 * q
                        c0 = max(dlt, 0) * P  # cols < c0 fully masked
                        diag = dlt >= 0
                        kslice = kts[jk // 4][:, (jk % 4) * P:(jk % 4 + 1) * P]
                        ps_s = _mk(psp2, [P, 2 * SQ], F32, "ps2")
                        for hh, h in enumerate(heads):
                            o = hh * SQ
                            nc.tensor.matmul(
                                ps_s[:, o + c0:o + SQ], kslice,
                                qth[h][q][:, c0:],
                                start=True, stop=not diag)
                            if diag:
                                # accumulate -30000 on the masked (k>q) part
                                nc.tensor.matmul(
                                    ps_s[:, o + c0:o + c0 + P], ident,
                                    mtri_sb, start=False, stop=True)
                        pr2 = _mk(prp, [P, 2 * SQ], BF16, "pr")
                        nc.scalar.activation(
                            pr2[:, c0:], ps_s[:, c0:],
                            mybir.ActivationFunctionType.Exp, scale=SCALE)
                        pend.append((jk, c0, pr2))
                        if len(pend) > 4:
                            pjk, pc0, ppr = pend.pop(0)
                            for hh, h in enumerate(heads):
                                o = hh * SQ
                                nc.tensor.matmul(
                                    ps_ctx[h][:, pc0:], vs[pjk],
                                    ppr[:, o + pc0:o + SQ],
                                    start=(pjk == 0), stop=False)
                    while pend:
                        pjk, pc0, ppr = pend.pop(0)
                        for hh, h in enumerate(heads):
                            o = hh * SQ
                            nc.tensor.matmul(
                                ps_ctx[h][:, pc0:], vs[pjk],
                                ppr[:, o + pc0:o + SQ],
                                start=(pjk == 0), stop=(not pend))
                    # per head: den recip (bf16) -> PE broadcast -> ctxg;
                    # hp0's blocks are deferred into hp1's loop so the den
                    # broadcast's recip-wait doesn't head-of-line block the
                    # next pass's scores on the in-order PE
                    def den_block(hp=hp, ps_ctx=ps_ctx, heads=heads):
                        for h in heads:
                            ho = (h % 2) * HD
                            denb = _mk(sp, [1, SQ], BF16, "denb")
                            with nc.allow_low_precision("softmax den bf16"):
                                nc.vector.reciprocal(denb,
                                                     ps_ctx[h][HD:HD + 1, :])
                            ps_db = _mk(psp, [HD, SQ], F32, "ps")
                            nc.tensor.matmul(ps_db, halfones[:, :HD], denb,
                                             start=True, stop=True)
                            # gud = (tanh(g/2) + 1) * (0.5/den)
                            gud = _mk(wp, [HD, SQ], BF16, "gud")
                            nc.vector.scalar_tensor_tensor(
                                gud, gus[hp][q][ho:ho + HD, :], 1.0, ps_db,
                                mybir.AluOpType.add, mybir.AluOpType.mult)
                            nc.vector.tensor_mul(
                                ctxgs[hp][q][ho:ho + HD, :],
                                ps_ctx[h][:HD, :], gud)
                    if hp == 0:
                        deferred.append(den_block)
                    else:
                        for g in deferred:
                            g()
                        den_block()
                for g in fqueue:
                    g()

            def f_groups(q, alt_copies=False):
                """Output projection for sq slice q as 16 deferred groups,
                interleaved one-per-jk-step into the next e_slice. The four
                n-slices of one s-row-block share an [128, 2048] staging
                tile and a single out-DMA (HWDGE holds ~625ns per dma_start
                regardless of size, so fewer+bigger DMAs win)."""
                groups = []
                for jj in range(4):
                    oc4_box = {}
                    for n in range(NSQ):
                        def emit(jj=jj, n=n, oc4_box=oc4_box):
                            jc = jj * P
                            j = 4 * q + jj
                            ps_o = _mk(psp, [P, SQ], F32, "ps")
                            for e in range(2):
                                nc.tensor.matmul(
                                    ps_o, ctxgs[e][q][:, jc:jc + P],
                                    wos[e][:, n * SQ:(n + 1) * SQ],
                                    start=(e == 0), stop=(e == 1))
                            if n == 0:
                                oc4_box["t"] = _mk(ocp, [P, 4 * SQ], BF16,
                                                   "oc4")
                            oc4 = oc4_box["t"]
                            dst = oc4[:, n * SQ:(n + 1) * SQ]
                            if alt_copies and n % 2 == 0:
                                nc.scalar.copy(dst, ps_o)
                            else:
                                nc.vector.tensor_copy(out=dst, in_=ps_o)
                            if n == NSQ - 1:
                                nc.sync.dma_start(
                                    out=out[j * P:(j + 1) * P, :], in_=oc4)
                        groups.append(emit)
                return groups

            # ---- schedule: gate sweeps fill the DMA window; the last 8
            # qkv groups and all out-projection groups are fed one-per-jk-
            # step into the attention slices, so the in-order PE program
            # stays dense from DMA arrival to the final output DMA ----
            _mark(nc, "phaseD0")
            g0 = d_sweep_mm(0, (0, 1))
            _mark(nc, "phaseA")
            for j in range(4):
                a_group(j)
            g1 = d_sweep_mm(1, (0, 1))
            aux_dmas()
            d_sweep_act(0, (0, 1), g0)
            _mark(nc, "phaseB")
            b_chain(0, 4)
            a_group(4)
            a_group(5)
            a_group(6)
            d_sweep_act(1, (0, 1), g1)
            a_group(7)
            b_chain(4, 8)
            _mark(nc, "phaseC0")
            for j in range(4):
                c_tile(j, dve=(j % 2 == 1))
            ct47 = c_fillers(range(4, 8))
            _mark(nc, "phaseE0")
            e_slice(0, [lambda j=j: a_group(j) for j in range(8, 12)]
                    + ct47)
            b_chain(8, 12)
            ct811 = c_fillers(range(8, 12))
            _mark(nc, "phaseD1")
            d_sweep(0, (2, 3))
            _mark(nc, "phaseE1")
            e_slice(1, [lambda j=j: a_group(j) for j in range(12, 16)]
                    + ct811 + f_groups(0, alt_copies=True))
            b_chain(12, 16)
            ct1215 = c_fillers(range(12, 16))
            _mark(nc, "phaseD2")
            d_sweep(1, (2, 3))
            _mark(nc, "phaseE2")
            e_slice(2, ct1215 + f_groups(1))
            _mark(nc, "phaseE3")
            e_slice(3, f_groups(2))
            for g in f_groups(3, alt_copies=True):
                g()

    nc.compile()
    return nc


def prep_inputs(x, cos, sin, Wq, Wk, Wv, Wo, q_norm_w, k_norm_w):
    """Host-side shard + layout prep. Returns per-core input maps."""
    xtn = x.reshape(S, D).T.astype(NBF)

    # rope tables with (1 + norm_w) folded in, k-block appended, and the
    # sin first-half pre-negated (so rope is out = q*cos5 + rot(q)*sin5
    # with rot(q) = [q2, q1])
    half = HD // 2
    wq1 = (1.0 + q_norm_w).astype(np.float32)
    wk1 = (1.0 + k_norm_w).astype(np.float32)

    def rotw(w):
        return np.concatenate([w[half:], w[:half]])

    sin_m = sin.copy()
    sin_m[:, :half] = -sin_m[:, :half]
    cos_q = cos * wq1
    cos_k = cos * wk1
    sin_q = sin_m * rotw(wq1)
    sin_k = sin_m * rotw(wk1)
    cos5 = np.concatenate([np.tile(cos_q, (1, NHL)), cos_k], axis=1)
    sin5 = np.concatenate([np.tile(sin_q, (1, NHL)), sin_k], axis=1)
    cs = np.ascontiguousarray(
        np.concatenate([cos5, sin5], axis=1)).astype(NBF)

    # strict lower triangle (k > q within the diagonal block) gets -30000,
    # accumulated into the scores psum before exp
    mtri = (np.tril(np.full((P, P), NEG, dtype=np.float32), k=-1)).astype(NBF)

    Wqh = Wq.reshape(H, 2 * HD, D)
    in_maps = []
    for c in range(NCORE):
        hs = slice(NHL * c, NHL * (c + 1))
        wq_c = Wqh[hs, :HD, :].reshape(EL, D)       # q rows, 4 heads
        wgt_c = Wqh[hs, HD:, :].reshape(EL, D)      # gate rows
        wk_c = Wk[HD * c:HD * (c + 1), :]
        wv_c = Wv[HD * c:HD * (c + 1), :]
        # [640, D]: q | k | v | gate
        wqg_c = np.concatenate([wq_c, wk_c, wv_c, wgt_c], axis=0)
        xw_c = np.ascontiguousarray(
            np.concatenate([xtn, wqg_c.T.astype(NBF)], axis=1))
        in_maps.append({
            "xw": xw_c,
            "wo": np.ascontiguousarray(
                Wo[:, EL * c:EL * (c + 1)].T).astype(NBF),
            "cs": cs,
            "mtri": mtri,
        })
    return in_maps


_NC_CACHE = {}


def get_nc():
    if "nc" not in _NC_CACHE:
        _NC_CACHE["nc"] = build_nc()
    return _NC_CACHE["nc"]


def run(in_maps, trace=False, **kw):
    nc = get_nc()
    return run_bass_kernel_spmd(nc, in_maps, list(range(NCORE)),
                                trace=trace, **kw)


def kernel(x, mask, cos, sin, Wq, Wk, Wv, Wo, q_norm_w, k_norm_w):
    in_maps = prep_inputs(np.asarray(x, dtype=np.float32), np.asarray(cos),
                          np.asarray(sin), np.asarray(Wq), np.asarray(Wk),
                          np.asarray(Wv), np.asarray(Wo),
                          np.asarray(q_norm_w), np.asarray(k_norm_w))
    res = run(in_maps)
    acc = np.zeros((S, D), dtype=np.float32)
    for r in res.results:
        acc += np.asarray(r["out"], dtype=np.float32)
    return acc.reshape(1, S, D)


# revision 55
# speedup vs baseline: 1.1447x; 1.0032x over previous
"""GQA attention block (B=1, S=2048, D=2048, H=32, G=8, HD=64) on 8 trn2 cores.

Sharding: tensor-parallel over heads/KV-groups. Core c owns q-heads
4c..4c+3 and KV group c. Wq/Wk/Wv column-parallel, Wo row-parallel;
each core computes a partial [S, D] output, host sums the 8 partials.

Per-core dataflow (all matmuls bf16, stats f32), creation-ordered for
engine overlap (the tile scheduler keeps per-engine programs roughly in
creation order, so independent work is interleaved at emission time):
  DMAs interleaved per d-tile and split over both HWDGE queues
  (SP + ACT) so x streams in at 2x; wqkv|wg merged into one tensor.
  D (gate proj, [e,s] layout): 4 sweeps of (p, q-pair); first two fill
    the x-DMA window, last two fill later PE gaps. gus = tanh(g/2).
  A: qkv proj per s-tile -> psum [s,384]; ACT: square + raw-qk copy;
    DVE: v copy + block row-sums into one batched stats tile.
  B: one batched Newton-rsqrt chain over all stats (no per-tile chain).
  C: norm-mul + rope split over gpsimd and DVE + PE transposes to
    [hd,s]; psum->sbuf copies split over ACT and DVE.
  E: per (q-slice, head-pair): software-pipelined over k-tiles with
    depth 4 — scores/exp for tile jk are emitted well before the ctx
    matmuls of jk-4, so the in-order PE never waits on the exp. The two
    heads of a pair share one [128,1024] scores psum and a single exp.
    Causal mask by accumulating a -30000 strict-lower-tri matmul on
    diagonal blocks; probs = exp(scale*scores) on ACT (c0-trimmed);
    ctxT (+den row) = [v|1].T @ probsT with trimmed widths. Per head:
    den recip (bf16, low-precision ok) -> PE broadcast; gud =
    (tanh+1)*(0.5/den) via one scalar_tensor_tensor; ctxg = ctx * gud.
  F: out[s,dout] = ctxg.T @ woT; its 16 groups per q-slice are fed
    one-per-jk-step into the NEXT slice's attention loop (fqueue), as
    are the last 8 qkv groups, keeping the PE program dense end-to-end;
    psum->bf16 copies on DVE, partials summed on host.
"""

import numpy as np
import ml_dtypes

import concourse.bass as bass
import concourse.tile as tile
from concourse import bacc, mybir
from concourse.bass_utils import run_bass_kernel_spmd
from concourse.masks import make_identity

BF16 = mybir.dt.bfloat16
F32 = mybir.dt.float32
NBF = ml_dtypes.bfloat16

S = 2048
D = 2048
H = 32
G = 8
HD = 64
NCORE = 8
NHL = H // NCORE          # 4 q heads per core
EL = NHL * HD             # 256 local q (and gate, and ctx) features
QK = EL + HD              # 320: q + k features
QKV = QK + HD             # 384: q + k + v
QG = QKV + EL             # 640: qkv + gate columns in the merged weight
P = 128
NS = S // P               # 16 s-tiles
ND = D // P               # 16 d-tiles
SQ = 512
NSQ = S // SQ             # 4 sq slices
NB = QK // HD             # 5 (hd,) blocks in the q|k strip
SCALE = HD ** -0.5
EPS = 1e-6
NEG = -30000.0


def _v(ap, dims, extra_offset=0):
    """Reshape the free dims of a 2D AP into `dims` ([step, count] pairs),
    keeping the partition dim."""
    return bass.AP(
        tensor=ap.tensor,
        offset=ap.offset + extra_offset,
        ap=[list(ap.ap[0])] + [list(d) for d in dims],
    )


def _mk(pool, shape, dtype, tag):
    return pool.tile(shape, dtype, tag=tag, name=tag)


PHASES = []  # (phase_name, first_instruction_index) — debug aid for sim.py


def _mark(nc, name):
    PHASES.append((name, int(nc.get_next_instruction_name().split("-")[1])))


def build_nc():
    nc = bacc.Bacc("TRN2", target_bir_lowering=False, debug=False,
                   num_devices=NCORE)

    xw = nc.dram_tensor("xw", [D, S + QG], BF16, kind="ExternalInput").ap()
    wo = nc.dram_tensor("wo", [EL, D], BF16, kind="ExternalInput").ap()
    cs = nc.dram_tensor("cs", [S, 2 * QK], BF16, kind="ExternalInput").ap()
    mtri = nc.dram_tensor("mtri", [P, P], BF16, kind="ExternalInput").ap()
    out = nc.dram_tensor("out", [S, D], BF16, kind="ExternalOutput").ap()

    with tile.TileContext(nc) as tc:
        with (
            tc.tile_pool(name="persist", bufs=1) as pp,
            tc.tile_pool(name="work", bufs=3) as wp,
            tc.tile_pool(name="stats", bufs=2) as sp,
            tc.tile_pool(name="qkr", bufs=3) as qp,
            tc.tile_pool(name="probs", bufs=8) as prp,
            tc.tile_pool(name="outc", bufs=2) as ocp,
            tc.tile_pool(name="psum", bufs=4, space="PSUM") as psp,
            tc.tile_pool(name="psum2", bufs=2, space="PSUM") as psp2,
        ):
            # ---- persistent loads: one DMA per d-tile (x row-block and
            # its weight row-block merged host-side into one dram tensor) --
            xts, wqgs = [], []
            for i in range(ND):
                t = _mk(pp, [P, S + QG], BF16, f"xw{i}")
                nc.sync.dma_start(out=t, in_=xw[i * P:(i + 1) * P, :])
                xts.append(t[:, :S])
                wqgs.append(t[:, S:])
            # cs/mtri/wo DMAs are deferred until after the x stream —
            # every dma_start holds the shared HWDGE ~625ns, and issuing
            # these early interleaves them into the critical xw cadence
            css = []
            for j in range(NS):
                css.append(_mk(pp, [P, 2 * QK], BF16, f"cs{j}"))
            mtri_sb = _mk(pp, [P, P], BF16, "mtri")
            wos = [_mk(pp, [P, D], BF16, f"wo{e}") for e in range(2)]

            def aux_dmas():
                for j in range(NS):
                    nc.sync.dma_start(out=css[j],
                                      in_=cs[j * P:(j + 1) * P, :])
                nc.sync.dma_start(out=mtri_sb, in_=mtri)
                for e in range(2):
                    nc.sync.dma_start(out=wos[e],
                                      in_=wo[e * P:(e + 1) * P, :])

            ident = _mk(pp, [P, P], BF16, "ident")
            make_identity(nc, ident)
            halfones = _mk(pp, [1, P], BF16, "halfones")
            nc.vector.memset(halfones, 0.5)

            # persistent intermediate tensors
            qth = [[_mk(pp, [HD, SQ], BF16, f"qt{h}_{q}") for q in range(NSQ)]
                   for h in range(NHL)]
            kts = [_mk(pp, [HD, SQ], BF16, f"kt{q}") for q in range(NSQ)]
            vs = [_mk(pp, [P, HD + 1], BF16, f"v{j}") for j in range(NS)]
            gus = [[_mk(pp, [P, SQ], BF16, f"gu{p}_{q}") for q in range(NSQ)]
                   for p in range(2)]
            ctxgs = [[_mk(pp, [P, SQ], BF16, f"cg{p}_{q}") for q in range(NSQ)]
                     for p in range(2)]
            qk_all = [_mk(pp, [P, QK], BF16, f"qk{j}") for j in range(NS)]
            ss_all = _mk(pp, [P, NB * NS], F32, "ss_all")
            y_all = _mk(pp, [P, NB * NS], F32, "y_all")

            def d_sweep_mm(p, qs):
                """Gate projection sweep matmuls: fixed p, q-pair qs in one
                2-bank tile from the scores pool (free outside attention)."""
                ps_g = _mk(psp2, [P, 2 * SQ], F32, "ps2")
                for i in range(ND):
                    for o, qq in enumerate(qs):
                        nc.tensor.matmul(
                            ps_g[:, o * SQ:(o + 1) * SQ],
                            wqgs[i][:, QKV + p * P:QKV + (p + 1) * P],
                            xts[i][:, qq * SQ:(qq + 1) * SQ],
                            start=(i == 0), stop=(i == ND - 1))
                return ps_g

            def d_sweep_act(p, qs, ps_g):
                """Deferred tanh part of a gate sweep (keeps the in-order
                ACT queue from blocking later work on the sweep's finish)."""
                for o, qq in enumerate(qs):
                    # gus = tanh(g/2); the (1 + .)*0.5/den fold happens in E
                    nc.scalar.activation(gus[p][qq],
                                         ps_g[:, o * SQ:(o + 1) * SQ],
                                         mybir.ActivationFunctionType.Tanh,
                                         scale=0.5)

            def d_sweep(p, qs):
                d_sweep_act(p, qs, d_sweep_mm(p, qs))


            def a_group(j, dve_stats=False):
                """QKV projection + stats for s-tile j.

                dve_stats=True keeps ACT out of it entirely (for groups
                fed as fillers into the exp-paced attention regions):
                qk copy on DVE, square from the bf16 copy on DVE 2x."""
                ps_qkv = _mk(psp, [P, QKV], F32, "ps")
                for i in range(ND):
                    nc.tensor.matmul(
                        ps_qkv, xts[i][:, j * P:(j + 1) * P],
                        wqgs[i][:, :QKV],
                        start=(i == 0), stop=(i == ND - 1))
                # v (+ ones column) straight to SBUF
                nc.vector.tensor_copy(out=vs[j][:, :HD], in_=ps_qkv[:, QK:QKV])
                nc.vector.memset(vs[j][:, HD:HD + 1], 1.0)
                qk = ps_qkv[:, :QK]
                sqr = _mk(wp, [P, QK], F32, "sqr")
                if dve_stats:
                    nc.vector.tensor_copy(out=qk_all[j], in_=qk)
                    nc.vector.tensor_mul(sqr, qk_all[j], qk_all[j])
                else:
                    # squares + raw qk copy on ACT, row-sums on DVE
                    nc.scalar.activation(sqr, qk,
                                         mybir.ActivationFunctionType.Square)
                    nc.scalar.copy(qk_all[j], qk)
                nc.vector.tensor_reduce(
                    ss_all[:, NB * j:NB * (j + 1)],
                    _v(sqr, [[HD, NB], [1, HD]]),
                    axis=mybir.AxisListType.X, op=mybir.AluOpType.add)

            def b_chain(lo, hi, eng=None):
                """Batched Newton rsqrt for s-tiles [lo, hi). DVE only:
                tensor_scalar/scalar_tensor_tensor are not legal Pool-engine
                ops (walrus NCC_IXCG966), so no gpsimd half here."""
                if eng is None:
                    eng = nc.vector
                c0, c1 = NB * lo, NB * hi
                n = c1 - c0
                ss = ss_all[:, c0:c1]
                y = y_all[:, c0:c1]
                m = _mk(sp, [P, n], F32, "m")
                eng.tensor_scalar(m, ss, 1.0 / HD, EPS,
                                        mybir.AluOpType.mult,
                                        mybir.AluOpType.add)
                mc = _mk(sp, [P, n], F32, "mc")
                eng.tensor_scalar(mc, m, 5.5, 0.45,
                                        mybir.AluOpType.min,
                                        mybir.AluOpType.max)
                m2 = _mk(sp, [P, n], F32, "m2")
                eng.tensor_mul(m2, mc, mc)
                lin = _mk(sp, [P, n], F32, "lin")
                eng.tensor_scalar(lin, mc, -0.48330447, 1.51774376,
                                        mybir.AluOpType.mult,
                                        mybir.AluOpType.add)
                eng.scalar_tensor_tensor(y, m2, 0.0534932, lin,
                                               mybir.AluOpType.mult,
                                               mybir.AluOpType.add)
                ytmp = _mk(sp, [P, n], F32, "ytmp")
                # 2 Newton steps: seed err ~5% -> ~4e-3 -> ~2e-5, far below
                # bf16 resolution
                for _ in range(2):
                    eng.tensor_mul(ytmp, y, y)              # y^2
                    eng.tensor_mul(ytmp, ytmp, m)           # m y^2
                    eng.tensor_scalar(ytmp, ytmp, -0.5, 1.5,
                                            mybir.AluOpType.mult,
                                            mybir.AluOpType.add)
                    eng.tensor_mul(y, y, ytmp)

            def c_rope(j, dve=False):
                """Norm + rope for s-tile j (gpsimd + DVE only, no PE).

                qkn/t1 on gpsimd, t2/qkr on DVE — splits the serial chain
                across two engines and halves each one's load. Returns the
                roped tile for the deferred transpose."""
                eng = nc.vector if dve else nc.gpsimd
                yb = _v(y_all[:, NB * j:NB * (j + 1)], [[1, NB], [0, HD]])
                qkn = _mk(wp, [P, QK], BF16, "qkn")
                eng.tensor_mul(
                    _v(qkn, [[HD, NB], [1, HD]]),
                    _v(qk_all[j][:, :], [[HD, NB], [1, HD]]), yb)
                # rope: out = qkn*cos5 + rot(qkn)*sin5  (sin pre-negated on
                # the first half on host; cos/sin already include 1+norm_w)
                t1 = _mk(wp, [P, QK], BF16, "t1")
                eng.tensor_mul(t1, qkn, css[j][:, :QK])
                t2 = _mk(wp, [P, QK], BF16, "t2")
                rot = _v(qkn[:, :], [[HD, NB], [-32, 2], [1, 32]],
                         extra_offset=32)
                nc.vector.tensor_mul(
                    _v(t2, [[HD, NB], [32, 2], [1, 32]]), rot,
                    _v(css[j][:, QK:], [[HD, NB], [32, 2], [1, 32]]))
                qkr = _mk(qp, [P, QK], BF16, "qkr")
                nc.vector.tensor_add(qkr, t1, t2)
                return qkr

            def c_trans(j, qkr, act=False):
                """Transpose q packs and k of s-tile j into [hd, s]."""
                cp = (nc.scalar.copy if act else
                      (lambda o, i: nc.vector.tensor_copy(out=o, in_=i)))
                jq, jc = j // 4, (j % 4) * P
                for p in range(2):
                    pt = _mk(psp, [P, P], BF16, "ps")
                    nc.tensor.transpose(pt, qkr[:, p * P:(p + 1) * P], ident)
                    cp(qth[2 * p][jq][:, jc:jc + P], pt[:HD, :])
                    cp(qth[2 * p + 1][jq][:, jc:jc + P], pt[HD:, :])
                ptk = _mk(psp, [HD, P], BF16, "ps")
                nc.tensor.transpose(ptk, qkr[:, 2 * P:2 * P + HD], ident)
                cp(kts[jq][:, jc:jc + P], ptk)

            def c_tile(j, dve=False, act=False):
                c_trans(j, c_rope(j, dve), act=act)

            def c_fillers(js, act=False):
                """Rope now (no PE), transposes as filler closures."""
                roped = [(j, c_rope(j)) for j in js]
                return [lambda j=j, r=r: c_trans(j, r, act=act)
                        for j, r in roped]

            def e_slice(q, fqueue=()):
                """Attention for sq slice q, head pairs, sw-pipelined.

                The two heads of a pair share one [128, 2*SQ] scores psum
                (2 banks) and a single exp instruction; the masked prefix of
                the second half holds junk that the trimmed ctx matmuls
                never read."""
                nks = 4 * q + 4   # sk tiles 0..nks-1 intersect causally
                fqueue = list(fqueue)
                deferred = []     # hp0's den/ctxg, emitted inside hp1
                for hp in range(2):
                    heads = (2 * hp, 2 * hp + 1)
                    ps_ctx = {h: _mk(psp, [HD + 1, SQ], F32, "ps")
                              for h in heads}
                    pend = []   # [(jk, c0, pr2)] pending ctx, depth 2
                    for jk in range(nks):
                        if jk == 2 and deferred:
                            deferred.pop(0)()
                        if fqueue:
                            fqueue.pop(0)()
                        dlt = jk - 4 * q
                        c0 = max(dlt, 0) * P  # cols < c0 fully masked
                        diag = dlt >= 0
                        kslice = kts[jk // 4][:, (jk % 4) * P:(jk % 4 + 1) * P]
                        ps_s = _mk(psp2, [P, 2 * SQ], F32, "ps2")
                        for hh, h in enumerate(heads):
                            o = hh * SQ
                            nc.tensor.matmul(
                                ps_s[:, o + c0:o + SQ], kslice,
                                qth[h][q][:, c0:],
                                start=True, stop=not diag)
                            if diag:
                                # accumulate -30000 on the masked (k>q) part
                                nc.tensor.matmul(
                                    ps_s[:, o + c0:o + c0 + P], ident,
                                    mtri_sb, start=False, stop=True)
                        pr2 = _mk(prp, [P, 2 * SQ], BF16, "pr")
                        nc.scalar.activation(
                            pr2[:, c0:], ps_s[:, c0:],
                            mybir.ActivationFunctionType.Exp, scale=SCALE)
                        pend.append((jk, c0, pr2))
                        if len(pend) > 4:
                            pjk, pc0, ppr = pend.pop(0)
                            for hh, h in enumerate(heads):
                                o = hh * SQ
                                nc.tensor.matmul(
                                    ps_ctx[h][:, pc0:], vs[pjk],
                                    ppr[:, o + pc0:o + SQ],
                                    start=(pjk == 0), stop=False)
                    while pend:
                        pjk, pc0, ppr = pend.pop(0)
                        for hh, h in enumerate(heads):
                            o = hh * SQ
                            nc.tensor.matmul(
                                ps_ctx[h][:, pc0:], vs[pjk],
                                ppr[:, o + pc0:o + SQ],
                                start=(pjk == 0), stop=(not pend))
                    # per head: den recip (bf16) -> PE broadcast -> ctxg;
                    # hp0's blocks are deferred into hp1's loop so the den
                    # broadcast's recip-wait doesn't head-of-line block the
                    # next pass's scores on the in-order PE
                    def den_block(hp=hp, ps_ctx=ps_ctx, heads=heads):
                        for h in heads:
                            ho = (h % 2) * HD
                            denb = _mk(sp, [1, SQ], BF16, "denb")
                            with nc.allow_low_precision("softmax den bf16"):
                                nc.vector.reciprocal(denb,
                                                     ps_ctx[h][HD:HD + 1, :])
                            ps_db = _mk(psp, [HD, SQ], F32, "ps")
                            nc.tensor.matmul(ps_db, halfones[:, :HD], denb,
                                             start=True, stop=True)
                            # gud = (tanh(g/2) + 1) * (0.5/den)
                            gud = _mk(wp, [HD, SQ], BF16, "gud")
                            nc.vector.scalar_tensor_tensor(
                                gud, gus[hp][q][ho:ho + HD, :], 1.0, ps_db,
                                mybir.AluOpType.add, mybir.AluOpType.mult)
                            nc.vector.tensor_mul(
                                ctxgs[hp][q][ho:ho + HD, :],
                                ps_ctx[h][:HD, :], gud)
                    if hp == 0:
                        deferred.append(den_block)
                    else:
                        for g in deferred:
                            g()
                        den_block()
                for g in fqueue:
                    g()

            def f_groups(q, alt_copies=False):
                """Output projection for sq slice q as 16 deferred groups,
                interleaved one-per-jk-step into the next e_slice. The four
                n-slices of one s-row-block share an [128, 2048] staging
                tile and a single out-DMA (HWDGE holds ~625ns per dma_start
                regardless of size, so fewer+bigger DMAs win)."""
                groups = []
                for jj in range(4):
                    oc4_box = {}
                    for n in range(NSQ):
                        def emit(jj=jj, n=n, oc4_box=oc4_box):
                            jc = jj * P
                            j = 4 * q + jj
                            ps_o = _mk(psp, [P, SQ], F32, "ps")
                            for e in range(2):
                                nc.tensor.matmul(
                                    ps_o, ctxgs[e][q][:, jc:jc + P],
                                    wos[e][:, n * SQ:(n + 1) * SQ],
                                    start=(e == 0), stop=(e == 1))
                            if n == 0:
                                oc4_box["t"] = _mk(ocp, [P, 4 * SQ], BF16,
                                                   "oc4")
                            oc4 = oc4_box["t"]
                            dst = oc4[:, n * SQ:(n + 1) * SQ]
                            if alt_copies and n % 2 == 0:
                                nc.scalar.copy(dst, ps_o)
                            else:
                                nc.vector.tensor_copy(out=dst, in_=ps_o)
                            if n == NSQ - 1:
                                nc.sync.dma_start(
                                    out=out[j * P:(j + 1) * P, :], in_=oc4)
                        groups.append(emit)
                return groups

            # ---- schedule: gate sweeps fill the DMA window; the last 8
            # qkv groups and all out-projection groups are fed one-per-jk-
            # step into the attention slices, so the in-order PE program
            # stays dense from DMA arrival to the final output DMA ----
            _mark(nc, "phaseD0")
            g0 = d_sweep_mm(0, (0, 1))
            _mark(nc, "phaseA")
            for j in range(4):
                a_group(j)
            g1 = d_sweep_mm(1, (0, 1))
            aux_dmas()
            d_sweep_act(0, (0, 1), g0)
            _mark(nc, "phaseB")
            b_chain(0, 4)
            a_group(4)
            a_group(5)
            a_group(6)
            d_sweep_act(1, (0, 1), g1)
            a_group(7)
            b_chain(4, 8)
            _mark(nc, "phaseC0")
            for j in range(4):
                c_tile(j, dve=(j % 2 == 1))
            ct47 = c_fillers(range(4, 8))
            _mark(nc, "phaseE0")
            e_slice(0, [lambda j=j: a_group(j, dve_stats=True)
                        for j in range(8, 12)] + ct47)
            b_chain(8, 12)
            ct811 = c_fillers(range(8, 12))
            _mark(nc, "phaseD1")
            d_sweep(0, (2, 3))
            _mark(nc, "phaseE1")
            e_slice(1, [lambda j=j: a_group(j, dve_stats=True)
                        for j in range(12, 16)]
                    + ct811 + f_groups(0, alt_copies=True))
            b_chain(12, 16)
            ct1215 = c_fillers(range(12, 16))
            _mark(nc, "phaseD2")
            d_sweep(1, (2, 3))
            _mark(nc, "phaseE2")
            e_slice(2, ct1215 + f_groups(1))
            _mark(nc, "phaseE3")
            e_slice(3, f_groups(2))
            for g in f_groups(3, alt_copies=True):
                g()

    nc.compile()
    return nc


def prep_inputs(x, cos, sin, Wq, Wk, Wv, Wo, q_norm_w, k_norm_w):
    """Host-side shard + layout prep. Returns per-core input maps."""
    xtn = x.reshape(S, D).T.astype(NBF)

    # rope tables with (1 + norm_w) folded in, k-block appended, and the
    # sin first-half pre-negated (so rope is out = q*cos5 + rot(q)*sin5
    # with rot(q) = [q2, q1])
    half = HD // 2
    wq1 = (1.0 + q_norm_w).astype(np.float32)
    wk1 = (1.0 + k_norm_w).astype(np.float32)

    def rotw(w):
        return np.concatenate([w[half:], w[:half]])

    sin_m = sin.copy()
    sin_m[:, :half] = -sin_m[:, :half]
    cos_q = cos * wq1
    cos_k = cos * wk1
    sin_q = sin_m * rotw(wq1)
    sin_k = sin_m * rotw(wk1)
    cos5 = np.concatenate([np.tile(cos_q, (1, NHL)), cos_k], axis=1)
    sin5 = np.concatenate([np.tile(sin_q, (1, NHL)), sin_k], axis=1)
    cs = np.ascontiguousarray(
        np.concatenate([cos5, sin5], axis=1)).astype(NBF)

    # strict lower triangle (k > q within the diagonal block) gets -30000,
    # accumulated into the scores psum before exp
    mtri = (np.tril(np.full((P, P), NEG, dtype=np.float32), k=-1)).astype(NBF)

    Wqh = Wq.reshape(H, 2 * HD, D)
    in_maps = []
    for c in range(NCORE):
        hs = slice(NHL * c, NHL * (c + 1))
        wq_c = Wqh[hs, :HD, :].reshape(EL, D)       # q rows, 4 heads
        wgt_c = Wqh[hs, HD:, :].reshape(EL, D)      # gate rows
        wk_c = Wk[HD * c:HD * (c + 1), :]
        wv_c = Wv[HD * c:HD * (c + 1), :]
        # [640, D]: q | k | v | gate
        wqg_c = np.concatenate([wq_c, wk_c, wv_c, wgt_c], axis=0)
        xw_c = np.ascontiguousarray(
            np.concatenate([xtn, wqg_c.T.astype(NBF)], axis=1))
        in_maps.append({
            "xw": xw_c,
            "wo": np.ascontiguousarray(
                Wo[:, EL * c:EL * (c + 1)].T).astype(NBF),
            "cs": cs,
            "mtri": mtri,
        })
    return in_maps


_NC_CACHE = {}


def get_nc():
    if "nc" not in _NC_CACHE:
        _NC_CACHE["nc"] = build_nc()
    return _NC_CACHE["nc"]


def run(in_maps, trace=False, **kw):
    nc = get_nc()
    return run_bass_kernel_spmd(nc, in_maps, list(range(NCORE)),
                                trace=trace, **kw)


def kernel(x, mask, cos, sin, Wq, Wk, Wv, Wo, q_norm_w, k_norm_w):
    in_maps = prep_inputs(np.asarray(x, dtype=np.float32), np.asarray(cos),
                          np.asarray(sin), np.asarray(Wq), np.asarray(Wk),
                          np.asarray(Wv), np.asarray(Wo),
                          np.asarray(q_norm_w), np.asarray(k_norm_w))
    res = run(in_maps)
    acc = np.zeros((S, D), dtype=np.float32)
    for r in res.results:
        acc += np.asarray(r["out"], dtype=np.float32)
    return acc.reshape(1, S, D)


# revision 65
# speedup vs baseline: 1.1566x; 1.0104x over previous
"""GQA attention block (B=1, S=2048, D=2048, H=32, G=8, HD=64) on 8 trn2 cores.

Sharding: tensor-parallel over heads/KV-groups. Core c owns q-heads
4c..4c+3 and KV group c. Wq/Wk/Wv column-parallel, Wo row-parallel;
each core computes a partial [S, D] output, host sums the 8 partials.

Per-core dataflow (all matmuls bf16, stats f32), creation-ordered for
engine overlap (the tile scheduler keeps per-engine programs roughly in
creation order, so independent work is interleaved at emission time):
  One DMA per d-tile (x row-block + its qkv|gate weight row-block
  merged host-side), ALL input DMAs on one queue in priority order --
  HWDGE holds a fixed ~625ns per dma_start, so DMA count is the lever
  and a second queue's sequencer would front-run the critical x stream.
  Out-DMAs merged 4:1 into full [128,2048] row-blocks for the same
  reason.
  D (gate proj, [e,s] layout): 4 sweeps of (p, q-pair); first two fill
    the x-DMA window, last two fill later PE gaps. gus = tanh(g/2).
  A: qkv proj per s-tile -> psum [s,384]; ACT: square + raw-qk copy;
    DVE: v copy + block row-sums into one batched stats tile.
  B: one batched Newton-rsqrt chain over all stats (no per-tile chain).
  C: norm-mul + rope split over gpsimd and DVE + PE transposes to
    [hd,s]; psum->sbuf copies split over ACT and DVE.
  E: per (q-slice, head-pair): software-pipelined over k-tiles with
    depth 4 — scores/exp for tile jk are emitted well before the ctx
    matmuls of jk-4, so the in-order PE never waits on the exp. The two
    heads of a pair share one [128,1024] scores psum and a single exp.
    Causal mask by accumulating a -30000 strict-lower-tri matmul on
    diagonal blocks; probs = exp(scale*scores) on ACT (c0-trimmed);
    ctxT (+den row) = [v|1].T @ probsT with trimmed widths. Per head:
    den recip (bf16, low-precision ok) -> PE broadcast; gud =
    (tanh+1)*(0.5/den) via one scalar_tensor_tensor; ctxg = ctx * gud.
  F: out[s,dout] = ctxg.T @ woT; its 16 groups per q-slice are fed
    one-per-jk-step into the NEXT slice's attention loop (fqueue), as
    are the last 8 qkv groups, keeping the PE program dense end-to-end;
    psum->bf16 copies on DVE, partials summed on host.
"""

import numpy as np
import ml_dtypes

import concourse.bass as bass
import concourse.tile as tile
from concourse import bacc, mybir
from concourse.bass_utils import run_bass_kernel_spmd
from concourse.masks import make_identity

BF16 = mybir.dt.bfloat16
F32 = mybir.dt.float32
NBF = ml_dtypes.bfloat16

S = 2048
D = 2048
H = 32
G = 8
HD = 64
NCORE = 8
NHL = H // NCORE          # 4 q heads per core
EL = NHL * HD             # 256 local q (and gate, and ctx) features
QK = EL + HD              # 320: q + k features
QKV = QK + HD             # 384: q + k + v
QG = QKV + EL             # 640: qkv + gate columns in the merged weight
P = 128
NS = S // P               # 16 s-tiles
ND = D // P               # 16 d-tiles
SQ = 512
NSQ = S // SQ             # 4 sq slices
NB = QK // HD             # 5 (hd,) blocks in the q|k strip
SCALE = HD ** -0.5
EPS = 1e-6
NEG = -30000.0


def _v(ap, dims, extra_offset=0):
    """Reshape the free dims of a 2D AP into `dims` ([step, count] pairs),
    keeping the partition dim."""
    return bass.AP(
        tensor=ap.tensor,
        offset=ap.offset + extra_offset,
        ap=[list(ap.ap[0])] + [list(d) for d in dims],
    )


def _mk(pool, shape, dtype, tag):
    return pool.tile(shape, dtype, tag=tag, name=tag)


PHASES = []  # (phase_name, first_instruction_index) — debug aid for sim.py


def _mark(nc, name):
    PHASES.append((name, int(nc.get_next_instruction_name().split("-")[1])))


def build_nc():
    nc = bacc.Bacc("TRN2", target_bir_lowering=False, debug=False,
                   num_devices=NCORE)

    xw = nc.dram_tensor("xw", [D, S + QG], BF16, kind="ExternalInput").ap()
    wo = nc.dram_tensor("wo", [EL, D], BF16, kind="ExternalInput").ap()
    cs = nc.dram_tensor("cs", [S, 2 * QK], BF16, kind="ExternalInput").ap()
    mtri = nc.dram_tensor("mtri", [P, P], BF16, kind="ExternalInput").ap()
    out = nc.dram_tensor("out", [S, D], BF16, kind="ExternalOutput").ap()

    with tile.TileContext(nc) as tc:
        with (
            tc.tile_pool(name="persist", bufs=1) as pp,
            tc.tile_pool(name="work", bufs=3) as wp,
            tc.tile_pool(name="stats", bufs=2) as sp,
            tc.tile_pool(name="qkr", bufs=3) as qp,
            tc.tile_pool(name="probs", bufs=8) as prp,
            tc.tile_pool(name="outc", bufs=2) as ocp,
            tc.tile_pool(name="psum", bufs=4, space="PSUM") as psp,
            tc.tile_pool(name="psum2", bufs=2, space="PSUM") as psp2,
        ):
            # ---- persistent loads. Host layout per d-tile row-block is
            # [wqg | x]; the critical front (gate sweeps + qkv groups 0-7)
            # needs only wqg + the first half of x's columns, so each tile
            # is fetched as two DMAs and the x-half-b fetches are deferred
            # behind everything the front needs. ----
            HA = QG + S // 2          # cols in the front half
            tws, tbs, wqgs = [], [], []
            for i in range(ND):
                t = _mk(pp, [P, HA], BF16, f"xwa{i}")
                nc.sync.dma_start(out=t, in_=xw[i * P:(i + 1) * P, :HA])
                tws.append(t)
                wqgs.append(t[:, :QG])
                tbs.append(_mk(pp, [P, S // 2], BF16, f"xwb{i}"))

            def xsl(i, c0, c1):
                """x[:, c0:c1] of d-tile i (slices never span the halves)."""
                if c1 <= S // 2:
                    return tws[i][:, QG + c0:QG + c1]
                return tbs[i][:, c0 - S // 2:c1 - S // 2]

            def xb_dmas():
                for i in range(ND):
                    nc.sync.dma_start(out=tbs[i],
                                      in_=xw[i * P:(i + 1) * P, HA:])
            # cs/mtri/wo DMAs are deferred until after the x stream —
            # every dma_start holds the shared HWDGE ~625ns, and issuing
            # these early interleaves them into the critical xw cadence
            css = []
            for j in range(NS):
                css.append(_mk(pp, [P, 2 * QK], BF16, f"cs{j}"))
            mtri_sb = _mk(pp, [P, P], BF16, "mtri")
            wos = [_mk(pp, [P, D], BF16, f"wo{e}") for e in range(2)]

            def aux_dmas():
                for j in range(NS):
                    nc.sync.dma_start(out=css[j],
                                      in_=cs[j * P:(j + 1) * P, :])
                nc.sync.dma_start(out=mtri_sb, in_=mtri)
                for e in range(2):
                    nc.sync.dma_start(out=wos[e],
                                      in_=wo[e * P:(e + 1) * P, :])

            ident = _mk(pp, [P, P], BF16, "ident")
            make_identity(nc, ident)
            halfones = _mk(pp, [1, P], BF16, "halfones")
            nc.vector.memset(halfones, 0.5)

            # persistent intermediate tensors
            qth = [[_mk(pp, [HD, SQ], BF16, f"qt{h}_{q}") for q in range(NSQ)]
                   for h in range(NHL)]
            kts = [_mk(pp, [HD, SQ], BF16, f"kt{q}") for q in range(NSQ)]
            vs = [_mk(pp, [P, HD + 1], BF16, f"v{j}") for j in range(NS)]
            gus = [[_mk(pp, [P, SQ], BF16, f"gu{p}_{q}") for q in range(NSQ)]
                   for p in range(2)]
            ctxgs = [[_mk(pp, [P, SQ], BF16, f"cg{p}_{q}") for q in range(NSQ)]
                     for p in range(2)]
            qk_all = [_mk(pp, [P, QK], BF16, f"qk{j}") for j in range(NS)]
            ss_all = _mk(pp, [P, NB * NS], F32, "ss_all")
            y_all = _mk(pp, [P, NB * NS], F32, "y_all")

            def d_sweep_mm(p, qs):
                """Gate projection sweep matmuls: fixed p, q-pair qs in one
                2-bank tile from the scores pool (free outside attention)."""
                ps_g = _mk(psp2, [P, 2 * SQ], F32, "ps2")
                for i in range(ND):
                    for o, qq in enumerate(qs):
                        nc.tensor.matmul(
                            ps_g[:, o * SQ:(o + 1) * SQ],
                            wqgs[i][:, QKV + p * P:QKV + (p + 1) * P],
                            xsl(i, qq * SQ, (qq + 1) * SQ),
                            start=(i == 0), stop=(i == ND - 1))
                return ps_g

            def d_sweep_act(p, qs, ps_g):
                """Deferred tanh part of a gate sweep (keeps the in-order
                ACT queue from blocking later work on the sweep's finish)."""
                for o, qq in enumerate(qs):
                    # gus = tanh(g/2); the (1 + .)*0.5/den fold happens in E
                    nc.scalar.activation(gus[p][qq],
                                         ps_g[:, o * SQ:(o + 1) * SQ],
                                         mybir.ActivationFunctionType.Tanh,
                                         scale=0.5)

            def d_sweep(p, qs):
                d_sweep_act(p, qs, d_sweep_mm(p, qs))


            def a_group(j, dve_stats=False):
                """QKV projection + stats for s-tile j.

                dve_stats=True keeps ACT out of it entirely (for groups
                fed as fillers into the exp-paced attention regions):
                qk copy on DVE, square from the bf16 copy on DVE 2x."""
                ps_qkv = _mk(psp, [P, QKV], F32, "ps")
                for i in range(ND):
                    nc.tensor.matmul(
                        ps_qkv, xsl(i, j * P, (j + 1) * P),
                        wqgs[i][:, :QKV],
                        start=(i == 0), stop=(i == ND - 1))
                # v (+ ones column) straight to SBUF
                nc.vector.tensor_copy(out=vs[j][:, :HD], in_=ps_qkv[:, QK:QKV])
                nc.vector.memset(vs[j][:, HD:HD + 1], 1.0)
                qk = ps_qkv[:, :QK]
                sqr = _mk(wp, [P, QK], F32, "sqr")
                if dve_stats:
                    nc.vector.tensor_copy(out=qk_all[j], in_=qk)
                    nc.vector.tensor_mul(sqr, qk_all[j], qk_all[j])
                else:
                    # squares + raw qk copy on ACT, row-sums on DVE
                    nc.scalar.activation(sqr, qk,
                                         mybir.ActivationFunctionType.Square)
                    nc.scalar.copy(qk_all[j], qk)
                nc.vector.tensor_reduce(
                    ss_all[:, NB * j:NB * (j + 1)],
                    _v(sqr, [[HD, NB], [1, HD]]),
                    axis=mybir.AxisListType.X, op=mybir.AluOpType.add)

            def b_chain(lo, hi, eng=None):
                """Batched Newton rsqrt for s-tiles [lo, hi). DVE only:
                tensor_scalar/scalar_tensor_tensor are not legal Pool-engine
                ops (walrus NCC_IXCG966), so no gpsimd half here."""
                if eng is None:
                    eng = nc.vector
                c0, c1 = NB * lo, NB * hi
                n = c1 - c0
                ss = ss_all[:, c0:c1]
                y = y_all[:, c0:c1]
                m = _mk(sp, [P, n], F32, "m")
                eng.tensor_scalar(m, ss, 1.0 / HD, EPS,
                                        mybir.AluOpType.mult,
                                        mybir.AluOpType.add)
                mc = _mk(sp, [P, n], F32, "mc")
                eng.tensor_scalar(mc, m, 5.5, 0.45,
                                        mybir.AluOpType.min,
                                        mybir.AluOpType.max)
                m2 = _mk(sp, [P, n], F32, "m2")
                eng.tensor_mul(m2, mc, mc)
                lin = _mk(sp, [P, n], F32, "lin")
                eng.tensor_scalar(lin, mc, -0.48330447, 1.51774376,
                                        mybir.AluOpType.mult,
                                        mybir.AluOpType.add)
                eng.scalar_tensor_tensor(y, m2, 0.0534932, lin,
                                               mybir.AluOpType.mult,
                                               mybir.AluOpType.add)
                ytmp = _mk(sp, [P, n], F32, "ytmp")
                # 2 Newton steps: seed err ~5% -> ~4e-3 -> ~2e-5, far below
                # bf16 resolution
                for _ in range(2):
                    eng.tensor_mul(ytmp, y, y)              # y^2
                    eng.tensor_mul(ytmp, ytmp, m)           # m y^2
                    eng.tensor_scalar(ytmp, ytmp, -0.5, 1.5,
                                            mybir.AluOpType.mult,
                                            mybir.AluOpType.add)
                    eng.tensor_mul(y, y, ytmp)

            def c_rope(j, dve=False):
                """Norm + rope for s-tile j (gpsimd + DVE only, no PE).

                qkn/t1 on gpsimd, t2/qkr on DVE — splits the serial chain
                across two engines and halves each one's load. Returns the
                roped tile for the deferred transpose."""
                eng = nc.vector if dve else nc.gpsimd
                yb = _v(y_all[:, NB * j:NB * (j + 1)], [[1, NB], [0, HD]])
                qkn = _mk(wp, [P, QK], BF16, "qkn")
                eng.tensor_mul(
                    _v(qkn, [[HD, NB], [1, HD]]),
                    _v(qk_all[j][:, :], [[HD, NB], [1, HD]]), yb)
                # rope: out = qkn*cos5 + rot(qkn)*sin5  (sin pre-negated on
                # the first half on host; cos/sin already include 1+norm_w)
                t1 = _mk(wp, [P, QK], BF16, "t1")
                eng.tensor_mul(t1, qkn, css[j][:, :QK])
                t2 = _mk(wp, [P, QK], BF16, "t2")
                rot = _v(qkn[:, :], [[HD, NB], [-32, 2], [1, 32]],
                         extra_offset=32)
                nc.vector.tensor_mul(
                    _v(t2, [[HD, NB], [32, 2], [1, 32]]), rot,
                    _v(css[j][:, QK:], [[HD, NB], [32, 2], [1, 32]]))
                qkr = _mk(qp, [P, QK], BF16, "qkr")
                nc.vector.tensor_add(qkr, t1, t2)
                return qkr

            def c_trans(j, qkr, act=False):
                """Transpose q packs and k of s-tile j into [hd, s]."""
                cp = (nc.scalar.copy if act else
                      (lambda o, i: nc.vector.tensor_copy(out=o, in_=i)))
                jq, jc = j // 4, (j % 4) * P
                for p in range(2):
                    pt = _mk(psp, [P, P], BF16, "ps")
                    nc.tensor.transpose(pt, qkr[:, p * P:(p + 1) * P], ident)
                    cp(qth[2 * p][jq][:, jc:jc + P], pt[:HD, :])
                    cp(qth[2 * p + 1][jq][:, jc:jc + P], pt[HD:, :])
                ptk = _mk(psp, [HD, P], BF16, "ps")
                nc.tensor.transpose(ptk, qkr[:, 2 * P:2 * P + HD], ident)
                cp(kts[jq][:, jc:jc + P], ptk)

            def c_tile(j, dve=False, act=False):
                c_trans(j, c_rope(j, dve), act=act)

            def c_fillers(js, act=False):
                """Rope now (no PE), transposes as filler closures."""
                roped = [(j, c_rope(j)) for j in js]
                return [lambda j=j, r=r: c_trans(j, r, act=act)
                        for j, r in roped]

            def e_slice(q, fqueue=()):
                """Attention for sq slice q, head pairs, sw-pipelined.

                The two heads of a pair share one [128, 2*SQ] scores psum
                (2 banks) and a single exp instruction; the masked prefix of
                the second half holds junk that the trimmed ctx matmuls
                never read."""
                nks = 4 * q + 4   # sk tiles 0..nks-1 intersect causally
                fqueue = list(fqueue)
                deferred = []     # hp0's den/ctxg, emitted inside hp1
                for hp in range(2):
                    heads = (2 * hp, 2 * hp + 1)
                    ps_ctx = {h: _mk(psp, [HD + 1, SQ], F32, "ps")
                              for h in heads}
                    pend = []   # [(jk, c0, pr2)] pending ctx, depth 2
                    for jk in range(nks):
                        if jk == 2 and deferred:
                            deferred.pop(0)()
                        if fqueue:
                            fqueue.pop(0)()
                        dlt = jk - 4 * q
                        c0 = max(dlt, 0) * P  # cols < c0 fully masked
                        diag = dlt >= 0
                        kslice = kts[jk // 4][:, (jk % 4) * P:(jk % 4 + 1) * P]
                        ps_s = _mk(psp2, [P, 2 * SQ], F32, "ps2")
                        for hh, h in enumerate(heads):
                            o = hh * SQ
                            nc.tensor.matmul(
                                ps_s[:, o + c0:o + SQ], kslice,
                                qth[h][q][:, c0:],
                                start=True, stop=not diag)
                            if diag:
                                # accumulate -30000 on the masked (k>q) part
                                nc.tensor.matmul(
                                    ps_s[:, o + c0:o + c0 + P], ident,
                                    mtri_sb, start=False, stop=True)
                        pr2 = _mk(prp, [P, 2 * SQ], BF16, "pr")
                        nc.scalar.activation(
                            pr2[:, c0:], ps_s[:, c0:],
                            mybir.ActivationFunctionType.Exp, scale=SCALE)
                        pend.append((jk, c0, pr2))
                        if len(pend) > 4:
                            pjk, pc0, ppr = pend.pop(0)
                            for hh, h in enumerate(heads):
                                o = hh * SQ
                                nc.tensor.matmul(
                                    ps_ctx[h][:, pc0:], vs[pjk],
                                    ppr[:, o + pc0:o + SQ],
                                    start=(pjk == 0), stop=False)
                    while pend:
                        pjk, pc0, ppr = pend.pop(0)
                        for hh, h in enumerate(heads):
                            o = hh * SQ
                            nc.tensor.matmul(
                                ps_ctx[h][:, pc0:], vs[pjk],
                                ppr[:, o + pc0:o + SQ],
                                start=(pjk == 0), stop=(not pend))
                    # per head: den recip (bf16) -> PE broadcast -> ctxg;
                    # hp0's blocks are deferred into hp1's loop so the den
                    # broadcast's recip-wait doesn't head-of-line block the
                    # next pass's scores on the in-order PE
                    def den_block(hp=hp, ps_ctx=ps_ctx, heads=heads):
                        for h in heads:
                            ho = (h % 2) * HD
                            denb = _mk(sp, [1, SQ], BF16, "denb")
                            with nc.allow_low_precision("softmax den bf16"):
                                nc.vector.reciprocal(denb,
                                                     ps_ctx[h][HD:HD + 1, :])
                            ps_db = _mk(psp, [HD, SQ], F32, "ps")
                            nc.tensor.matmul(ps_db, halfones[:, :HD], denb,
                                             start=True, stop=True)
                            # gud = (tanh(g/2) + 1) * (0.5/den)
                            gud = _mk(wp, [HD, SQ], BF16, "gud")
                            nc.vector.scalar_tensor_tensor(
                                gud, gus[hp][q][ho:ho + HD, :], 1.0, ps_db,
                                mybir.AluOpType.add, mybir.AluOpType.mult)
                            nc.vector.tensor_mul(
                                ctxgs[hp][q][ho:ho + HD, :],
                                ps_ctx[h][:HD, :], gud)
                    if hp == 0:
                        deferred.append(den_block)
                    else:
                        for g in deferred:
                            g()
                        den_block()
                for g in fqueue:
                    g()

            def f_groups(q, alt_copies=False):
                """Output projection for sq slice q as 16 deferred groups,
                interleaved one-per-jk-step into the next e_slice. The four
                n-slices of one s-row-block share an [128, 2048] staging
                tile and a single out-DMA (HWDGE holds ~625ns per dma_start
                regardless of size, so fewer+bigger DMAs win)."""
                groups = []
                for jj in range(4):
                    oc4_box = {}
                    for n in range(NSQ):
                        def emit(jj=jj, n=n, oc4_box=oc4_box):
                            jc = jj * P
                            j = 4 * q + jj
                            ps_o = _mk(psp, [P, SQ], F32, "ps")
                            for e in range(2):
                                nc.tensor.matmul(
                                    ps_o, ctxgs[e][q][:, jc:jc + P],
                                    wos[e][:, n * SQ:(n + 1) * SQ],
                                    start=(e == 0), stop=(e == 1))
                            if n == 0:
                                oc4_box["t"] = _mk(ocp, [P, 4 * SQ], BF16,
                                                   "oc4")
                            oc4 = oc4_box["t"]
                            dst = oc4[:, n * SQ:(n + 1) * SQ]
                            if alt_copies and n % 2 == 0:
                                nc.scalar.copy(dst, ps_o)
                            else:
                                nc.vector.tensor_copy(out=dst, in_=ps_o)
                            if n == NSQ - 1:
                                nc.sync.dma_start(
                                    out=out[j * P:(j + 1) * P, :], in_=oc4)
                        groups.append(emit)
                return groups

            # ---- schedule: gate sweeps fill the DMA window; the last 8
            # qkv groups and all out-projection groups are fed one-per-jk-
            # step into the attention slices, so the in-order PE program
            # stays dense from DMA arrival to the final output DMA ----
            _mark(nc, "phaseD0")
            g0 = d_sweep_mm(0, (0, 1))
            _mark(nc, "phaseA")
            for j in range(4):
                a_group(j)
            g1 = d_sweep_mm(1, (0, 1))
            aux_dmas()
            xb_dmas()
            d_sweep_act(0, (0, 1), g0)
            _mark(nc, "phaseB")
            b_chain(0, 4)
            a_group(4)
            a_group(5)
            a_group(6)
            d_sweep_act(1, (0, 1), g1)
            a_group(7)
            b_chain(4, 8)
            _mark(nc, "phaseC0")
            for j in range(4):
                c_tile(j, dve=(j % 2 == 1))
            ct47 = c_fillers(range(4, 8))
            _mark(nc, "phaseE0")
            e_slice(0, [lambda j=j: a_group(j, dve_stats=True)
                        for j in range(8, 12)] + ct47)
            b_chain(8, 12)
            ct811 = c_fillers(range(8, 12))
            _mark(nc, "phaseD1")
            d_sweep(0, (2, 3))
            _mark(nc, "phaseE1")
            e_slice(1, [lambda j=j: a_group(j, dve_stats=True)
                        for j in range(12, 16)]
                    + ct811 + f_groups(0, alt_copies=True))
            b_chain(12, 16)
            ct1215 = c_fillers(range(12, 16))
            _mark(nc, "phaseD2")
            d_sweep(1, (2, 3))
            _mark(nc, "phaseE2")
            e_slice(2, ct1215 + f_groups(1))
            _mark(nc, "phaseE3")
            e_slice(3, f_groups(2))
            for g in f_groups(3, alt_copies=True):
                g()

    nc.compile()
    return nc


def prep_inputs(x, cos, sin, Wq, Wk, Wv, Wo, q_norm_w, k_norm_w):
    """Host-side shard + layout prep. Returns per-core input maps."""
    xtn = x.reshape(S, D).T.astype(NBF)

    # rope tables with (1 + norm_w) folded in, k-block appended, and the
    # sin first-half pre-negated (so rope is out = q*cos5 + rot(q)*sin5
    # with rot(q) = [q2, q1])
    half = HD // 2
    wq1 = (1.0 + q_norm_w).astype(np.float32)
    wk1 = (1.0 + k_norm_w).astype(np.float32)

    def rotw(w):
        return np.concatenate([w[half:], w[:half]])

    sin_m = sin.copy()
    sin_m[:, :half] = -sin_m[:, :half]
    cos_q = cos * wq1
    cos_k = cos * wk1
    sin_q = sin_m * rotw(wq1)
    sin_k = sin_m * rotw(wk1)
    cos5 = np.concatenate([np.tile(cos_q, (1, NHL)), cos_k], axis=1)
    sin5 = np.concatenate([np.tile(sin_q, (1, NHL)), sin_k], axis=1)
    cs = np.ascontiguousarray(
        np.concatenate([cos5, sin5], axis=1)).astype(NBF)

    # strict lower triangle (k > q within the diagonal block) gets -30000,
    # accumulated into the scores psum before exp
    mtri = (np.tril(np.full((P, P), NEG, dtype=np.float32), k=-1)).astype(NBF)

    Wqh = Wq.reshape(H, 2 * HD, D)
    in_maps = []
    for c in range(NCORE):
        hs = slice(NHL * c, NHL * (c + 1))
        wq_c = Wqh[hs, :HD, :].reshape(EL, D)       # q rows, 4 heads
        wgt_c = Wqh[hs, HD:, :].reshape(EL, D)      # gate rows
        wk_c = Wk[HD * c:HD * (c + 1), :]
        wv_c = Wv[HD * c:HD * (c + 1), :]
        # [640, D]: q | k | v | gate
        wqg_c = np.concatenate([wq_c, wk_c, wv_c, wgt_c], axis=0)
        xw_c = np.ascontiguousarray(
            np.concatenate([wqg_c.T.astype(NBF), xtn], axis=1))
        in_maps.append({
            "xw": xw_c,
            "wo": np.ascontiguousarray(
                Wo[:, EL * c:EL * (c + 1)].T).astype(NBF),
            "cs": cs,
            "mtri": mtri,
        })
    return in_maps


_NC_CACHE = {}


def get_nc():
    if "nc" not in _NC_CACHE:
        _NC_CACHE["nc"] = build_nc()
    return _NC_CACHE["nc"]


def run(in_maps, trace=False, **kw):
    nc = get_nc()
    return run_bass_kernel_spmd(nc, in_maps, list(range(NCORE)),
                                trace=trace, **kw)


def kernel(x, mask, cos, sin, Wq, Wk, Wv, Wo, q_norm_w, k_norm_w):
    in_maps = prep_inputs(np.asarray(x, dtype=np.float32), np.asarray(cos),
                          np.asarray(sin), np.asarray(Wq), np.asarray(Wk),
                          np.asarray(Wv), np.asarray(Wo),
                          np.asarray(q_norm_w), np.asarray(k_norm_w))
    res = run(in_maps)
    acc = np.zeros((S, D), dtype=np.float32)
    for r in res.results:
        acc += np.asarray(r["out"], dtype=np.float32)
    return acc.reshape(1, S, D)


# revision 71
# speedup vs baseline: 1.1573x; 1.0006x over previous
"""GQA attention block (B=1, S=2048, D=2048, H=32, G=8, HD=64) on 8 trn2 cores.

Sharding: tensor-parallel over heads/KV-groups. Core c owns q-heads
4c..4c+3 and KV group c. Wq/Wk/Wv column-parallel, Wo row-parallel;
each core computes a partial [S, D] output, host sums the 8 partials.

Per-core dataflow (all matmuls bf16, stats f32), creation-ordered for
engine overlap (the tile scheduler keeps per-engine programs roughly in
creation order, so independent work is interleaved at emission time):
  Two DMAs per d-tile ([wqg | x-half-a] first, x-half-b deferred:
  the critical front only reads w + x cols < 1024), ALL input DMAs on
  one queue in priority order --
  HWDGE holds a fixed ~625ns per dma_start, so DMA count is the lever
  and a second queue's sequencer would front-run the critical x stream.
  Out-DMAs merged 4:1 into full [128,2048] row-blocks for the same
  reason.
  D (gate proj, [e,s] layout): 4 sweeps of (p, q-pair); first two fill
    the x-DMA window, last two fill later PE gaps. gus = tanh(g/2).
  A: qkv proj per s-tile -> psum [s,384]; ACT: square + raw-qk copy;
    DVE: v copy + block row-sums into one batched stats tile.
  B: one batched Newton-rsqrt chain over all stats (no per-tile chain).
  C: norm-mul + rope split over gpsimd and DVE + PE transposes to
    [hd,s]; psum->sbuf copies split over ACT and DVE.
  E: per (q-slice, head-pair): software-pipelined over k-tiles with
    depth 4 — scores/exp for tile jk are emitted well before the ctx
    matmuls of jk-4, so the in-order PE never waits on the exp. The two
    heads of a pair share one [128,1024] scores psum and a single exp.
    Causal mask by accumulating a -30000 strict-lower-tri matmul on
    diagonal blocks; probs = exp(scale*scores) on ACT (c0-trimmed);
    ctxT (+den row) = [v|1].T @ probsT with trimmed widths. Per head:
    den recip (bf16, low-precision ok) -> PE broadcast; gud =
    (tanh+1)*(0.5/den) via one scalar_tensor_tensor; ctxg = ctx * gud.
  F: out[s,dout] = ctxg.T @ woT; its 16 groups per q-slice are fed
    one-per-jk-step into the NEXT slice's attention loop (fqueue), as
    are the last 8 qkv groups, keeping the PE program dense end-to-end;
    psum->bf16 copies on DVE, partials summed on host.
"""

import numpy as np
import ml_dtypes

import concourse.bass as bass
import concourse.tile as tile
from concourse import bacc, mybir
from concourse.bass_utils import run_bass_kernel_spmd
from concourse.masks import make_identity

BF16 = mybir.dt.bfloat16
F32 = mybir.dt.float32
NBF = ml_dtypes.bfloat16

S = 2048
D = 2048
H = 32
G = 8
HD = 64
NCORE = 8
NHL = H // NCORE          # 4 q heads per core
EL = NHL * HD             # 256 local q (and gate, and ctx) features
QK = EL + HD              # 320: q + k features
QKV = QK + HD             # 384: q + k + v
QG = QKV + EL             # 640: qkv + gate columns in the merged weight
P = 128
NS = S // P               # 16 s-tiles
ND = D // P               # 16 d-tiles
SQ = 512
NSQ = S // SQ             # 4 sq slices
NB = QK // HD             # 5 (hd,) blocks in the q|k strip
SCALE = HD ** -0.5
EPS = 1e-6
NEG = -30000.0


def _v(ap, dims, extra_offset=0):
    """Reshape the free dims of a 2D AP into `dims` ([step, count] pairs),
    keeping the partition dim."""
    return bass.AP(
        tensor=ap.tensor,
        offset=ap.offset + extra_offset,
        ap=[list(ap.ap[0])] + [list(d) for d in dims],
    )


def _mk(pool, shape, dtype, tag):
    return pool.tile(shape, dtype, tag=tag, name=tag)


PHASES = []  # (phase_name, first_instruction_index) — debug aid for sim.py


def _mark(nc, name):
    PHASES.append((name, int(nc.get_next_instruction_name().split("-")[1])))


def build_nc():
    nc = bacc.Bacc("TRN2", target_bir_lowering=False, debug=False,
                   num_devices=NCORE)

    xw = nc.dram_tensor("xw", [D, S + QG], BF16, kind="ExternalInput").ap()
    wo = nc.dram_tensor("wo", [EL, D], BF16, kind="ExternalInput").ap()
    cs = nc.dram_tensor("cs", [S, 2 * QK], BF16, kind="ExternalInput").ap()
    mtri = nc.dram_tensor("mtri", [P, P], BF16, kind="ExternalInput").ap()
    out = nc.dram_tensor("out", [S, D], BF16, kind="ExternalOutput").ap()

    with tile.TileContext(nc) as tc:
        with (
            tc.tile_pool(name="persist", bufs=1) as pp,
            tc.tile_pool(name="work", bufs=3) as wp,
            tc.tile_pool(name="stats", bufs=2) as sp,
            tc.tile_pool(name="qkr", bufs=3) as qp,
            tc.tile_pool(name="probs", bufs=8) as prp,
            tc.tile_pool(name="outc", bufs=2) as ocp,
            tc.tile_pool(name="psum", bufs=4, space="PSUM") as psp,
            tc.tile_pool(name="psum2", bufs=2, space="PSUM") as psp2,
        ):
            # ---- persistent loads. Host layout per d-tile row-block is
            # [wqg | x]; the critical front (gate sweeps + qkv groups 0-7)
            # needs only wqg + the first half of x's columns, so each tile
            # is fetched as two DMAs and the x-half-b fetches are deferred
            # behind everything the front needs. ----
            HA = QG + S // 2          # cols in the front half
            tws, tbs, wqgs = [], [], []
            for i in range(ND):
                t = _mk(pp, [P, HA], BF16, f"xwa{i}")
                nc.sync.dma_start(out=t, in_=xw[i * P:(i + 1) * P, :HA])
                tws.append(t)
                wqgs.append(t[:, :QG])
                tbs.append(_mk(pp, [P, S // 2], BF16, f"xwb{i}"))

            def xsl(i, c0, c1):
                """x[:, c0:c1] of d-tile i (slices never span the halves)."""
                if c1 <= S // 2:
                    return tws[i][:, QG + c0:QG + c1]
                return tbs[i][:, c0 - S // 2:c1 - S // 2]

            def xb_dmas():
                for i in range(ND):
                    nc.sync.dma_start(out=tbs[i],
                                      in_=xw[i * P:(i + 1) * P, HA:])
            # cs/mtri/wo DMAs are deferred until after the x stream —
            # every dma_start holds the shared HWDGE ~625ns, and issuing
            # these early interleaves them into the critical xw cadence
            css = []
            for j in range(NS):
                css.append(_mk(pp, [P, 2 * QK], BF16, f"cs{j}"))
            mtri_sb = _mk(pp, [P, P], BF16, "mtri")
            wos = [_mk(pp, [P, D], BF16, f"wo{e}") for e in range(2)]

            def aux_dmas():
                for j in range(NS):
                    nc.sync.dma_start(out=css[j],
                                      in_=cs[j * P:(j + 1) * P, :])
                nc.sync.dma_start(out=mtri_sb, in_=mtri)
                for e in range(2):
                    nc.sync.dma_start(out=wos[e],
                                      in_=wo[e * P:(e + 1) * P, :])

            ident = _mk(pp, [P, P], BF16, "ident")
            make_identity(nc, ident)
            halfones = _mk(pp, [1, P], BF16, "halfones")
            nc.vector.memset(halfones, 0.5)

            # persistent intermediate tensors
            qth = [[_mk(pp, [HD, SQ], BF16, f"qt{h}_{q}") for q in range(NSQ)]
                   for h in range(NHL)]
            kts = [_mk(pp, [HD, SQ], BF16, f"kt{q}") for q in range(NSQ)]
            vs = [_mk(pp, [P, HD + 1], BF16, f"v{j}") for j in range(NS)]
            gus = [[_mk(pp, [P, SQ], BF16, f"gu{p}_{q}") for q in range(NSQ)]
                   for p in range(2)]
            ctxgs = [[_mk(pp, [P, SQ], BF16, f"cg{p}_{q}") for q in range(NSQ)]
                     for p in range(2)]
            qk_all = [_mk(pp, [P, QK], BF16, f"qk{j}") for j in range(NS)]
            ss_all = _mk(pp, [P, NB * NS], F32, "ss_all")
            y_all = _mk(pp, [P, NB * NS], F32, "y_all")

            def d_sweep_mm(p, qs):
                """Gate projection sweep matmuls: fixed p, q-pair qs in one
                2-bank tile from the scores pool (free outside attention)."""
                ps_g = _mk(psp2, [P, 2 * SQ], F32, "ps2")
                for i in range(ND):
                    for o, qq in enumerate(qs):
                        nc.tensor.matmul(
                            ps_g[:, o * SQ:(o + 1) * SQ],
                            wqgs[i][:, QKV + p * P:QKV + (p + 1) * P],
                            xsl(i, qq * SQ, (qq + 1) * SQ),
                            start=(i == 0), stop=(i == ND - 1))
                return ps_g

            def d_sweep_act(p, qs, ps_g):
                """Deferred tanh part of a gate sweep (keeps the in-order
                ACT queue from blocking later work on the sweep's finish)."""
                for o, qq in enumerate(qs):
                    # gus = tanh(g/2); the (1 + .)*0.5/den fold happens in E
                    nc.scalar.activation(gus[p][qq],
                                         ps_g[:, o * SQ:(o + 1) * SQ],
                                         mybir.ActivationFunctionType.Tanh,
                                         scale=0.5)

            def d_sweep(p, qs):
                d_sweep_act(p, qs, d_sweep_mm(p, qs))


            def a_group(j, dve_stats=False):
                """QKV projection + stats for s-tile j.

                dve_stats=True keeps ACT out of it entirely (for groups
                fed as fillers into the exp-paced attention regions):
                qk copy on DVE, square from the bf16 copy on DVE 2x."""
                ps_qkv = _mk(psp, [P, QKV], F32, "ps")
                for i in range(ND):
                    nc.tensor.matmul(
                        ps_qkv, xsl(i, j * P, (j + 1) * P),
                        wqgs[i][:, :QKV],
                        start=(i == 0), stop=(i == ND - 1))
                # v (+ ones column) straight to SBUF
                nc.vector.tensor_copy(out=vs[j][:, :HD], in_=ps_qkv[:, QK:QKV])
                nc.vector.memset(vs[j][:, HD:HD + 1], 1.0)
                qk = ps_qkv[:, :QK]
                sqr = _mk(wp, [P, QK], F32, "sqr")
                if dve_stats:
                    nc.vector.tensor_copy(out=qk_all[j], in_=qk)
                    nc.vector.tensor_mul(sqr, qk_all[j], qk_all[j])
                else:
                    # squares + raw qk copy on ACT, row-sums on DVE
                    nc.scalar.activation(sqr, qk,
                                         mybir.ActivationFunctionType.Square)
                    nc.scalar.copy(qk_all[j], qk)
                nc.vector.tensor_reduce(
                    ss_all[:, NB * j:NB * (j + 1)],
                    _v(sqr, [[HD, NB], [1, HD]]),
                    axis=mybir.AxisListType.X, op=mybir.AluOpType.add)

            def b_chain(lo, hi, eng=None):
                """Batched Newton rsqrt for s-tiles [lo, hi). DVE only:
                tensor_scalar/scalar_tensor_tensor are not legal Pool-engine
                ops (walrus NCC_IXCG966), so no gpsimd half here."""
                if eng is None:
                    eng = nc.vector
                c0, c1 = NB * lo, NB * hi
                n = c1 - c0
                ss = ss_all[:, c0:c1]
                y = y_all[:, c0:c1]
                m = _mk(sp, [P, n], F32, "m")
                eng.tensor_scalar(m, ss, 1.0 / HD, EPS,
                                        mybir.AluOpType.mult,
                                        mybir.AluOpType.add)
                mc = _mk(sp, [P, n], F32, "mc")
                eng.tensor_scalar(mc, m, 5.5, 0.45,
                                        mybir.AluOpType.min,
                                        mybir.AluOpType.max)
                m2 = _mk(sp, [P, n], F32, "m2")
                eng.tensor_mul(m2, mc, mc)
                lin = _mk(sp, [P, n], F32, "lin")
                eng.tensor_scalar(lin, mc, -0.48330447, 1.51774376,
                                        mybir.AluOpType.mult,
                                        mybir.AluOpType.add)
                eng.scalar_tensor_tensor(y, m2, 0.0534932, lin,
                                               mybir.AluOpType.mult,
                                               mybir.AluOpType.add)
                ytmp = _mk(sp, [P, n], F32, "ytmp")
                # 2 Newton steps: seed err ~5% -> ~4e-3 -> ~2e-5, far below
                # bf16 resolution
                for _ in range(2):
                    eng.tensor_mul(ytmp, y, y)              # y^2
                    eng.tensor_mul(ytmp, ytmp, m)           # m y^2
                    eng.tensor_scalar(ytmp, ytmp, -0.5, 1.5,
                                            mybir.AluOpType.mult,
                                            mybir.AluOpType.add)
                    eng.tensor_mul(y, y, ytmp)

            def c_rope(j, dve=False):
                """Norm + rope for s-tile j (gpsimd + DVE only, no PE).

                qkn/t1 on gpsimd, t2/qkr on DVE — splits the serial chain
                across two engines and halves each one's load. Returns the
                roped tile for the deferred transpose."""
                eng = nc.vector if dve else nc.gpsimd
                yb = _v(y_all[:, NB * j:NB * (j + 1)], [[1, NB], [0, HD]])
                qkn = _mk(wp, [P, QK], BF16, "qkn")
                eng.tensor_mul(
                    _v(qkn, [[HD, NB], [1, HD]]),
                    _v(qk_all[j][:, :], [[HD, NB], [1, HD]]), yb)
                # rope: out = qkn*cos5 + rot(qkn)*sin5  (sin pre-negated on
                # the first half on host; cos/sin already include 1+norm_w)
                t1 = _mk(wp, [P, QK], BF16, "t1")
                eng.tensor_mul(t1, qkn, css[j][:, :QK])
                t2 = _mk(wp, [P, QK], BF16, "t2")
                rot = _v(qkn[:, :], [[HD, NB], [-32, 2], [1, 32]],
                         extra_offset=32)
                nc.vector.tensor_mul(
                    _v(t2, [[HD, NB], [32, 2], [1, 32]]), rot,
                    _v(css[j][:, QK:], [[HD, NB], [32, 2], [1, 32]]))
                qkr = _mk(qp, [P, QK], BF16, "qkr")
                nc.vector.tensor_add(qkr, t1, t2)
                return qkr

            def c_trans(j, qkr, act=False):
                """Transpose q packs and k of s-tile j into [hd, s]."""
                cp = (nc.scalar.copy if act else
                      (lambda o, i: nc.vector.tensor_copy(out=o, in_=i)))
                jq, jc = j // 4, (j % 4) * P
                for p in range(2):
                    pt = _mk(psp, [P, P], BF16, "ps")
                    nc.tensor.transpose(pt, qkr[:, p * P:(p + 1) * P], ident)
                    cp(qth[2 * p][jq][:, jc:jc + P], pt[:HD, :])
                    cp(qth[2 * p + 1][jq][:, jc:jc + P], pt[HD:, :])
                ptk = _mk(psp, [HD, P], BF16, "ps")
                nc.tensor.transpose(ptk, qkr[:, 2 * P:2 * P + HD], ident)
                cp(kts[jq][:, jc:jc + P], ptk)

            def c_tile(j, dve=False, act=False):
                c_trans(j, c_rope(j, dve), act=act)

            def c_fillers(js, act=False):
                """Rope now (no PE), transposes as filler closures."""
                roped = [(j, c_rope(j)) for j in js]
                return [lambda j=j, r=r: c_trans(j, r, act=act)
                        for j, r in roped]

            def e_slice(q, fqueue=()):
                """Attention for sq slice q, head pairs, sw-pipelined.

                The two heads of a pair share one [128, 2*SQ] scores psum
                (2 banks) and a single exp instruction; the masked prefix of
                the second half holds junk that the trimmed ctx matmuls
                never read."""
                nks = 4 * q + 4   # sk tiles 0..nks-1 intersect causally
                fqueue = list(fqueue)
                deferred = []     # hp0's den/ctxg, emitted inside hp1
                for hp in range(2):
                    heads = (2 * hp, 2 * hp + 1)
                    ps_ctx = {h: _mk(psp, [HD + 1, SQ], F32, "ps")
                              for h in heads}
                    pend = []   # [(jk, c0, pr2)] pending ctx, depth 2
                    for jk in range(nks):
                        if jk == 2 and deferred:
                            deferred.pop(0)()
                        if fqueue:
                            fqueue.pop(0)()
                        dlt = jk - 4 * q
                        c0 = max(dlt, 0) * P  # cols < c0 fully masked
                        diag = dlt >= 0
                        kslice = kts[jk // 4][:, (jk % 4) * P:(jk % 4 + 1) * P]
                        ps_s = _mk(psp2, [P, 2 * SQ], F32, "ps2")
                        for hh, h in enumerate(heads):
                            o = hh * SQ
                            nc.tensor.matmul(
                                ps_s[:, o + c0:o + SQ], kslice,
                                qth[h][q][:, c0:],
                                start=True, stop=not diag)
                            if diag:
                                # accumulate -30000 on the masked (k>q) part
                                nc.tensor.matmul(
                                    ps_s[:, o + c0:o + c0 + P], ident,
                                    mtri_sb, start=False, stop=True)
                        pr2 = _mk(prp, [P, 2 * SQ], BF16, "pr")
                        nc.scalar.activation(
                            pr2[:, c0:], ps_s[:, c0:],
                            mybir.ActivationFunctionType.Exp, scale=SCALE)
                        pend.append((jk, c0, pr2))
                        if len(pend) > 4:
                            pjk, pc0, ppr = pend.pop(0)
                            for hh, h in enumerate(heads):
                                o = hh * SQ
                                nc.tensor.matmul(
                                    ps_ctx[h][:, pc0:], vs[pjk],
                                    ppr[:, o + pc0:o + SQ],
                                    start=(pjk == 0), stop=False)
                    while pend:
                        pjk, pc0, ppr = pend.pop(0)
                        for hh, h in enumerate(heads):
                            o = hh * SQ
                            nc.tensor.matmul(
                                ps_ctx[h][:, pc0:], vs[pjk],
                                ppr[:, o + pc0:o + SQ],
                                start=(pjk == 0), stop=(not pend))
                    # per head: den recip (bf16) -> PE broadcast -> ctxg;
                    # hp0's blocks are deferred into hp1's loop so the den
                    # broadcast's recip-wait doesn't head-of-line block the
                    # next pass's scores on the in-order PE
                    def den_block(hp=hp, ps_ctx=ps_ctx, heads=heads):
                        for h in heads:
                            ho = (h % 2) * HD
                            denb = _mk(sp, [1, SQ], BF16, "denb")
                            with nc.allow_low_precision("softmax den bf16"):
                                nc.vector.reciprocal(denb,
                                                     ps_ctx[h][HD:HD + 1, :])
                            ps_db = _mk(psp, [HD, SQ], F32, "ps")
                            nc.tensor.matmul(ps_db, halfones[:, :HD], denb,
                                             start=True, stop=True)
                            # gud = (tanh(g/2) + 1) * (0.5/den)
                            gud = _mk(wp, [HD, SQ], BF16, "gud")
                            nc.vector.scalar_tensor_tensor(
                                gud, gus[hp][q][ho:ho + HD, :], 1.0, ps_db,
                                mybir.AluOpType.add, mybir.AluOpType.mult)
                            nc.vector.tensor_mul(
                                ctxgs[hp][q][ho:ho + HD, :],
                                ps_ctx[h][:HD, :], gud)
                    if hp == 0:
                        deferred.append(den_block)
                    else:
                        for g in deferred:
                            g()
                        den_block()
                for g in fqueue:
                    g()

            def f_groups(q, alt_copies=False):
                """Output projection for sq slice q as 16 deferred groups,
                interleaved one-per-jk-step into the next e_slice. The four
                n-slices of one s-row-block share an [128, 2048] staging
                tile and a single out-DMA (HWDGE holds ~625ns per dma_start
                regardless of size, so fewer+bigger DMAs win)."""
                groups = []
                for jj in range(4):
                    oc4_box = {}
                    for n in range(NSQ):
                        def emit(jj=jj, n=n, oc4_box=oc4_box):
                            jc = jj * P
                            j = 4 * q + jj
                            ps_o = _mk(psp, [P, SQ], F32, "ps")
                            for e in range(2):
                                nc.tensor.matmul(
                                    ps_o, ctxgs[e][q][:, jc:jc + P],
                                    wos[e][:, n * SQ:(n + 1) * SQ],
                                    start=(e == 0), stop=(e == 1))
                            if n == 0:
                                oc4_box["t"] = _mk(ocp, [P, 4 * SQ], BF16,
                                                   "oc4")
                            oc4 = oc4_box["t"]
                            dst = oc4[:, n * SQ:(n + 1) * SQ]
                            if alt_copies and n % 2 == 0:
                                nc.scalar.copy(dst, ps_o)
                            else:
                                nc.vector.tensor_copy(out=dst, in_=ps_o)
                            if n == NSQ - 1:
                                nc.sync.dma_start(
                                    out=out[j * P:(j + 1) * P, :], in_=oc4)
                        groups.append(emit)
                return groups

            # ---- schedule: gate sweeps fill the DMA window; the last 8
            # qkv groups and all out-projection groups are fed one-per-jk-
            # step into the attention slices, so the in-order PE program
            # stays dense from DMA arrival to the final output DMA ----
            _mark(nc, "phaseD0")
            g0 = d_sweep_mm(0, (0, 1))
            _mark(nc, "phaseA")
            for j in range(4):
                a_group(j)
            aux_dmas()
            xb_dmas()
            d_sweep_act(0, (0, 1), g0)
            _mark(nc, "phaseB")
            b_chain(0, 4)
            a_group(4)
            a_group(5)
            a_group(6)
            a_group(7)
            b_chain(4, 8)
            _mark(nc, "phaseC0")
            for j in range(4):
                c_tile(j, dve=(j % 2 == 1))
            ct47 = c_fillers(range(4, 8))
            _mark(nc, "phaseE0")
            # second gate sweep deferred into E0's fillers: its matmuls
            # competed with qkv groups 0-3 for PE during the DMA window;
            # its activation is only read at E0's pass-end den blocks
            g1box = {}
            e_slice(0, [lambda: g1box.__setitem__(
                            "g", d_sweep_mm(1, (0, 1))),
                        lambda: d_sweep_act(1, (0, 1), g1box["g"])]
                    + [lambda j=j: a_group(j, dve_stats=True)
                       for j in range(8, 12)] + ct47)
            b_chain(8, 12)
            ct811 = c_fillers(range(8, 12))
            _mark(nc, "phaseD1")
            d_sweep(0, (2, 3))
            _mark(nc, "phaseE1")
            e_slice(1, [lambda j=j: a_group(j, dve_stats=True)
                        for j in range(12, 16)]
                    + ct811 + f_groups(0, alt_copies=True))
            b_chain(12, 16)
            ct1215 = c_fillers(range(12, 16))
            _mark(nc, "phaseD2")
            d_sweep(1, (2, 3))
            _mark(nc, "phaseE2")
            e_slice(2, ct1215 + f_groups(1))
            _mark(nc, "phaseE3")
            e_slice(3, f_groups(2))
            for g in f_groups(3, alt_copies=True):
                g()

    nc.compile()
    return nc


def prep_inputs(x, cos, sin, Wq, Wk, Wv, Wo, q_norm_w, k_norm_w):
    """Host-side shard + layout prep. Returns per-core input maps."""
    xtn = x.reshape(S, D).T.astype(NBF)

    # rope tables with (1 + norm_w) folded in, k-block appended, and the
    # sin first-half pre-negated (so rope is out = q*cos5 + rot(q)*sin5
    # with rot(q) = [q2, q1])
    half = HD // 2
    wq1 = (1.0 + q_norm_w).astype(np.float32)
    wk1 = (1.0 + k_norm_w).astype(np.float32)

    def rotw(w):
        return np.concatenate([w[half:], w[:half]])

    sin_m = sin.copy()
    sin_m[:, :half] = -sin_m[:, :half]
    cos_q = cos * wq1
    cos_k = cos * wk1
    sin_q = sin_m * rotw(wq1)
    sin_k = sin_m * rotw(wk1)
    cos5 = np.concatenate([np.tile(cos_q, (1, NHL)), cos_k], axis=1)
    sin5 = np.concatenate([np.tile(sin_q, (1, NHL)), sin_k], axis=1)
    cs = np.ascontiguousarray(
        np.concatenate([cos5, sin5], axis=1)).astype(NBF)

    # strict lower triangle (k > q within the diagonal block) gets -30000,
    # accumulated into the scores psum before exp
    mtri = (np.tril(np.full((P, P), NEG, dtype=np.float32), k=-1)).astype(NBF)

    Wqh = Wq.reshape(H, 2 * HD, D)
    in_maps = []
    for c in range(NCORE):
        hs = slice(NHL * c, NHL * (c + 1))
        wq_c = Wqh[hs, :HD, :].reshape(EL, D)       # q rows, 4 heads
        wgt_c = Wqh[hs, HD:, :].reshape(EL, D)      # gate rows
        wk_c = Wk[HD * c:HD * (c + 1), :]
        wv_c = Wv[HD * c:HD * (c + 1), :]
        # [640, D]: q | k | v | gate
        wqg_c = np.concatenate([wq_c, wk_c, wv_c, wgt_c], axis=0)
        xw_c = np.ascontiguousarray(
            np.concatenate([wqg_c.T.astype(NBF), xtn], axis=1))
        in_maps.append({
            "xw": xw_c,
            "wo": np.ascontiguousarray(
                Wo[:, EL * c:EL * (c + 1)].T).astype(NBF),
            "cs": cs,
            "mtri": mtri,
        })
    return in_maps


_NC_CACHE = {}


def get_nc():
    if "nc" not in _NC_CACHE:
        _NC_CACHE["nc"] = build_nc()
    return _NC_CACHE["nc"]


def run(in_maps, trace=False, **kw):
    nc = get_nc()
    return run_bass_kernel_spmd(nc, in_maps, list(range(NCORE)),
                                trace=trace, **kw)


def kernel(x, mask, cos, sin, Wq, Wk, Wv, Wo, q_norm_w, k_norm_w):
    in_maps = prep_inputs(np.asarray(x, dtype=np.float32), np.asarray(cos),
                          np.asarray(sin), np.asarray(Wq), np.asarray(Wk),
                          np.asarray(Wv), np.asarray(Wo),
                          np.asarray(q_norm_w), np.asarray(k_norm_w))
    res = run(in_maps)
    acc = np.zeros((S, D), dtype=np.float32)
    for r in res.results:
        acc += np.asarray(r["out"], dtype=np.float32)
    return acc.reshape(1, S, D)


# revision 76
# speedup vs baseline: 1.1585x; 1.0010x over previous
"""GQA attention block (B=1, S=2048, D=2048, H=32, G=8, HD=64) on 8 trn2 cores.

Sharding: tensor-parallel over heads/KV-groups. Core c owns q-heads
4c..4c+3 and KV group c. Wq/Wk/Wv column-parallel, Wo row-parallel;
each core computes a partial [S, D] output, host sums the 8 partials.

Per-core dataflow (all matmuls bf16, stats f32), creation-ordered for
engine overlap (the tile scheduler keeps per-engine programs roughly in
creation order, so independent work is interleaved at emission time):
  Two DMAs per d-tile ([wqg | x-half-a] first, x-half-b deferred:
  the critical front only reads w + x cols < 1024), ALL input DMAs on
  one queue in priority order --
  HWDGE holds a fixed ~625ns per dma_start, so DMA count is the lever
  and a second queue's sequencer would front-run the critical x stream.
  Out-DMAs merged 4:1 into full [128,2048] row-blocks for the same
  reason.
  D (gate proj, [e,s] layout): 4 sweeps of (p, q-pair); first two fill
    the x-DMA window, last two fill later PE gaps. gus = tanh(g/2).
  A: qkv proj per s-tile -> psum [s,384]; ACT: square + raw-qk copy;
    DVE: v copy + block row-sums into one batched stats tile.
  B: one batched Newton-rsqrt chain over all stats (no per-tile chain).
  C: norm-mul + rope split over gpsimd and DVE + PE transposes to
    [hd,s]; psum->sbuf copies split over ACT and DVE.
  E: per (q-slice, head-pair): software-pipelined over k-tiles with
    depth 4 — scores/exp for tile jk are emitted well before the ctx
    matmuls of jk-4, so the in-order PE never waits on the exp. The two
    heads of a pair share one [128,1024] scores psum and a single exp.
    Causal mask by accumulating a -30000 strict-lower-tri matmul on
    diagonal blocks; probs = exp(scale*scores) on ACT (c0-trimmed);
    ctxT (+den row) = [v|1].T @ probsT with trimmed widths. Per head:
    den recip (bf16, low-precision ok) -> PE broadcast; gud =
    (tanh+1)*(0.5/den) via one scalar_tensor_tensor; ctxg = ctx * gud.
  F: out[s,dout] = ctxg.T @ woT; its 16 groups per q-slice are fed
    one-per-jk-step into the NEXT slice's attention loop (fqueue), as
    are the last 8 qkv groups, keeping the PE program dense end-to-end;
    psum->bf16 copies on DVE, partials summed on host.
"""

import numpy as np
import ml_dtypes

import concourse.bass as bass
import concourse.tile as tile
from concourse import bacc, mybir
from concourse.bass_utils import run_bass_kernel_spmd
from concourse.masks import make_identity

BF16 = mybir.dt.bfloat16
F32 = mybir.dt.float32
NBF = ml_dtypes.bfloat16

S = 2048
D = 2048
H = 32
G = 8
HD = 64
NCORE = 8
NHL = H // NCORE          # 4 q heads per core
EL = NHL * HD             # 256 local q (and gate, and ctx) features
QK = EL + HD              # 320: q + k features
QKV = QK + HD             # 384: q + k + v
QG = QKV + EL             # 640: qkv + gate columns in the merged weight
P = 128
NS = S // P               # 16 s-tiles
ND = D // P               # 16 d-tiles
SQ = 512
NSQ = S // SQ             # 4 sq slices
NB = QK // HD             # 5 (hd,) blocks in the q|k strip
SCALE = HD ** -0.5
EPS = 1e-6
NEG = -30000.0


def _v(ap, dims, extra_offset=0):
    """Reshape the free dims of a 2D AP into `dims` ([step, count] pairs),
    keeping the partition dim."""
    return bass.AP(
        tensor=ap.tensor,
        offset=ap.offset + extra_offset,
        ap=[list(ap.ap[0])] + [list(d) for d in dims],
    )


def _mk(pool, shape, dtype, tag):
    return pool.tile(shape, dtype, tag=tag, name=tag)


PHASES = []  # (phase_name, first_instruction_index) — debug aid for sim.py


def _mark(nc, name):
    PHASES.append((name, int(nc.get_next_instruction_name().split("-")[1])))


def build_nc():
    nc = bacc.Bacc("TRN2", target_bir_lowering=False, debug=False,
                   num_devices=NCORE)

    xw = nc.dram_tensor("xw", [D, S + QG], BF16, kind="ExternalInput").ap()
    wo = nc.dram_tensor("wo", [EL, D], BF16, kind="ExternalInput").ap()
    cs = nc.dram_tensor("cs", [S, 2 * QK], BF16, kind="ExternalInput").ap()
    mtri = nc.dram_tensor("mtri", [P, P], BF16, kind="ExternalInput").ap()
    out = nc.dram_tensor("out", [S, D], BF16, kind="ExternalOutput").ap()

    with tile.TileContext(nc) as tc:
        with (
            tc.tile_pool(name="persist", bufs=1) as pp,
            tc.tile_pool(name="work", bufs=3) as wp,
            tc.tile_pool(name="stats", bufs=2) as sp,
            tc.tile_pool(name="qkr", bufs=3) as qp,
            tc.tile_pool(name="probs", bufs=8) as prp,
            tc.tile_pool(name="outc", bufs=2) as ocp,
            tc.tile_pool(name="psum", bufs=4, space="PSUM") as psp,
            tc.tile_pool(name="psum2", bufs=2, space="PSUM") as psp2,
        ):
            # ---- persistent loads. Host layout per d-tile row-block is
            # [wqg | x]; the critical front (gate sweeps + qkv groups 0-7)
            # needs only wqg + the first half of x's columns, so each tile
            # is fetched as two DMAs and the x-half-b fetches are deferred
            # behind everything the front needs. ----
            HA = QG + S // 2          # cols in the front half
            tws, tbs, wqgs = [], [], []
            for i in range(ND):
                t = _mk(pp, [P, HA], BF16, f"xwa{i}")
                nc.sync.dma_start(out=t, in_=xw[i * P:(i + 1) * P, :HA])
                tws.append(t)
                wqgs.append(t[:, :QG])
                tbs.append(_mk(pp, [P, S // 2], BF16, f"xwb{i}"))

            def xsl(i, c0, c1):
                """x[:, c0:c1] of d-tile i (slices never span the halves)."""
                if c1 <= S // 2:
                    return tws[i][:, QG + c0:QG + c1]
                return tbs[i][:, c0 - S // 2:c1 - S // 2]

            def xb_dmas():
                for i in range(ND):
                    nc.sync.dma_start(out=tbs[i],
                                      in_=xw[i * P:(i + 1) * P, HA:])
            # cs/mtri/wo DMAs are deferred until after the x stream —
            # every dma_start holds the shared HWDGE ~625ns, and issuing
            # these early interleaves them into the critical xw cadence
            css = []
            for j in range(NS):
                css.append(_mk(pp, [P, 2 * QK], BF16, f"cs{j}"))
            mtri_sb = _mk(pp, [P, P], BF16, "mtri")
            wos = [_mk(pp, [P, D], BF16, f"wo{e}") for e in range(2)]

            def aux_dmas():
                for j in range(NS):
                    nc.sync.dma_start(out=css[j],
                                      in_=cs[j * P:(j + 1) * P, :])
                nc.sync.dma_start(out=mtri_sb, in_=mtri)
                for e in range(2):
                    nc.sync.dma_start(out=wos[e],
                                      in_=wo[e * P:(e + 1) * P, :])

            ident = _mk(pp, [P, P], BF16, "ident")
            make_identity(nc, ident)
            halfones = _mk(pp, [1, P], BF16, "halfones")
            nc.vector.memset(halfones, 0.5)

            # persistent intermediate tensors
            qth = [[_mk(pp, [HD, SQ], BF16, f"qt{h}_{q}") for q in range(NSQ)]
                   for h in range(NHL)]
            kts = [_mk(pp, [HD, SQ], BF16, f"kt{q}") for q in range(NSQ)]
            vs = [_mk(pp, [P, HD + 1], BF16, f"v{j}") for j in range(NS)]
            gus = [[_mk(pp, [P, SQ], BF16, f"gu{p}_{q}") for q in range(NSQ)]
                   for p in range(2)]
            ctxgs = [[_mk(pp, [P, SQ], BF16, f"cg{p}_{q}") for q in range(NSQ)]
                     for p in range(2)]
            qk_all = [_mk(pp, [P, QK], BF16, f"qk{j}") for j in range(NS)]
            ss_all = _mk(pp, [P, NB * NS], F32, "ss_all")
            y_all = _mk(pp, [P, NB * NS], F32, "y_all")

            def d_sweep_mm(p, qs):
                """Gate projection sweep matmuls: fixed p, q-pair qs in one
                2-bank tile from the scores pool (free outside attention)."""
                ps_g = _mk(psp2, [P, 2 * SQ], F32, "ps2")
                for i in range(ND):
                    for o, qq in enumerate(qs):
                        nc.tensor.matmul(
                            ps_g[:, o * SQ:(o + 1) * SQ],
                            wqgs[i][:, QKV + p * P:QKV + (p + 1) * P],
                            xsl(i, qq * SQ, (qq + 1) * SQ),
                            start=(i == 0), stop=(i == ND - 1))
                return ps_g

            def d_sweep_act(p, qs, ps_g):
                """Deferred tanh part of a gate sweep (keeps the in-order
                ACT queue from blocking later work on the sweep's finish)."""
                for o, qq in enumerate(qs):
                    # gus = tanh(g/2); the (1 + .)*0.5/den fold happens in E
                    nc.scalar.activation(gus[p][qq],
                                         ps_g[:, o * SQ:(o + 1) * SQ],
                                         mybir.ActivationFunctionType.Tanh,
                                         scale=0.5)

            def d_sweep(p, qs):
                d_sweep_act(p, qs, d_sweep_mm(p, qs))


            def a_group(j, dve_stats=False):
                """QKV projection + stats for s-tile j.

                dve_stats=True keeps ACT out of it entirely (for groups
                fed as fillers into the exp-paced attention regions):
                qk copy on DVE, square from the bf16 copy on DVE 2x."""
                ps_qkv = _mk(psp, [P, QKV], F32, "ps")
                for i in range(ND):
                    nc.tensor.matmul(
                        ps_qkv, xsl(i, j * P, (j + 1) * P),
                        wqgs[i][:, :QKV],
                        start=(i == 0), stop=(i == ND - 1))
                # v (+ ones column) straight to SBUF
                nc.vector.tensor_copy(out=vs[j][:, :HD], in_=ps_qkv[:, QK:QKV])
                nc.vector.memset(vs[j][:, HD:HD + 1], 1.0)
                qk = ps_qkv[:, :QK]
                sqr = _mk(wp, [P, QK], F32, "sqr")
                if dve_stats:
                    nc.vector.tensor_copy(out=qk_all[j], in_=qk)
                    nc.vector.tensor_mul(sqr, qk_all[j], qk_all[j])
                else:
                    # squares + raw qk copy on ACT, row-sums on DVE
                    nc.scalar.activation(sqr, qk,
                                         mybir.ActivationFunctionType.Square)
                    nc.scalar.copy(qk_all[j], qk)
                nc.vector.tensor_reduce(
                    ss_all[:, NB * j:NB * (j + 1)],
                    _v(sqr, [[HD, NB], [1, HD]]),
                    axis=mybir.AxisListType.X, op=mybir.AluOpType.add)

            def b_chain(lo, hi, eng=None):
                """Batched Newton rsqrt for s-tiles [lo, hi). DVE only:
                tensor_scalar/scalar_tensor_tensor are not legal Pool-engine
                ops (walrus NCC_IXCG966), so no gpsimd half here."""
                if eng is None:
                    eng = nc.vector
                c0, c1 = NB * lo, NB * hi
                n = c1 - c0
                ss = ss_all[:, c0:c1]
                y = y_all[:, c0:c1]
                m = _mk(sp, [P, n], F32, "m")
                eng.tensor_scalar(m, ss, 1.0 / HD, EPS,
                                        mybir.AluOpType.mult,
                                        mybir.AluOpType.add)
                mc = _mk(sp, [P, n], F32, "mc")
                eng.tensor_scalar(mc, m, 5.5, 0.45,
                                        mybir.AluOpType.min,
                                        mybir.AluOpType.max)
                m2 = _mk(sp, [P, n], F32, "m2")
                eng.tensor_mul(m2, mc, mc)
                lin = _mk(sp, [P, n], F32, "lin")
                eng.tensor_scalar(lin, mc, -0.48330447, 1.51774376,
                                        mybir.AluOpType.mult,
                                        mybir.AluOpType.add)
                eng.scalar_tensor_tensor(y, m2, 0.0534932, lin,
                                               mybir.AluOpType.mult,
                                               mybir.AluOpType.add)
                ytmp = _mk(sp, [P, n], F32, "ytmp")
                # 2 Newton steps: seed err ~5% -> ~4e-3 -> ~2e-5, far below
                # bf16 resolution
                for _ in range(2):
                    eng.tensor_mul(ytmp, y, y)              # y^2
                    eng.tensor_mul(ytmp, ytmp, m)           # m y^2
                    eng.tensor_scalar(ytmp, ytmp, -0.5, 1.5,
                                            mybir.AluOpType.mult,
                                            mybir.AluOpType.add)
                    eng.tensor_mul(y, y, ytmp)

            def c_rope(j, dve=False):
                """Norm + rope for s-tile j (gpsimd + DVE only, no PE).

                qkn/t1 on gpsimd, t2/qkr on DVE — splits the serial chain
                across two engines and halves each one's load. Returns the
                roped tile for the deferred transpose."""
                eng = nc.vector if dve else nc.gpsimd
                yb = _v(y_all[:, NB * j:NB * (j + 1)], [[1, NB], [0, HD]])
                qkn = _mk(wp, [P, QK], BF16, "qkn")
                eng.tensor_mul(
                    _v(qkn, [[HD, NB], [1, HD]]),
                    _v(qk_all[j][:, :], [[HD, NB], [1, HD]]), yb)
                # rope: out = qkn*cos5 + rot(qkn)*sin5  (sin pre-negated on
                # the first half on host; cos/sin already include 1+norm_w)
                t1 = _mk(wp, [P, QK], BF16, "t1")
                eng.tensor_mul(t1, qkn, css[j][:, :QK])
                t2 = _mk(wp, [P, QK], BF16, "t2")
                rot = _v(qkn[:, :], [[HD, NB], [-32, 2], [1, 32]],
                         extra_offset=32)
                nc.vector.tensor_mul(
                    _v(t2, [[HD, NB], [32, 2], [1, 32]]), rot,
                    _v(css[j][:, QK:], [[HD, NB], [32, 2], [1, 32]]))
                qkr = _mk(qp, [P, QK], BF16, "qkr")
                nc.vector.tensor_add(qkr, t1, t2)
                return qkr

            def c_trans(j, qkr, act=False):
                """Transpose q packs and k of s-tile j into [hd, s]."""
                cp = (nc.scalar.copy if act else
                      (lambda o, i: nc.vector.tensor_copy(out=o, in_=i)))
                jq, jc = j // 4, (j % 4) * P
                for p in range(2):
                    pt = _mk(psp, [P, P], BF16, "ps")
                    nc.tensor.transpose(pt, qkr[:, p * P:(p + 1) * P], ident)
                    cp(qth[2 * p][jq][:, jc:jc + P], pt[:HD, :])
                    cp(qth[2 * p + 1][jq][:, jc:jc + P], pt[HD:, :])
                ptk = _mk(psp, [HD, P], BF16, "ps")
                nc.tensor.transpose(ptk, qkr[:, 2 * P:2 * P + HD], ident)
                cp(kts[jq][:, jc:jc + P], ptk)

            def c_tile(j, dve=False, act=False):
                c_trans(j, c_rope(j, dve), act=act)

            def c_fillers(js, act=False):
                """Rope now (no PE), transposes as filler closures."""
                roped = [(j, c_rope(j)) for j in js]
                return [lambda j=j, r=r: c_trans(j, r, act=act)
                        for j, r in roped]

            def e_slice(q, fqueue=()):
                """Attention for sq slice q, head pairs, sw-pipelined.

                The two heads of a pair share one [128, 2*SQ] scores psum
                (2 banks) and a single exp instruction; the masked prefix of
                the second half holds junk that the trimmed ctx matmuls
                never read."""
                nks = 4 * q + 4   # sk tiles 0..nks-1 intersect causally
                fqueue = list(fqueue)
                deferred = []     # hp0's den/ctxg, emitted inside hp1
                for hp in range(2):
                    heads = (2 * hp, 2 * hp + 1)
                    ps_ctx = {h: _mk(psp, [HD + 1, SQ], F32, "ps")
                              for h in heads}
                    pend = []   # [(jk, c0, pr2)] pending ctx, depth 2
                    for jk in range(nks):
                        if jk == 2 and deferred:
                            deferred.pop(0)()
                        if fqueue:
                            fqueue.pop(0)()
                        dlt = jk - 4 * q
                        c0 = max(dlt, 0) * P  # cols < c0 fully masked
                        diag = dlt >= 0
                        kslice = kts[jk // 4][:, (jk % 4) * P:(jk % 4 + 1) * P]
                        ps_s = _mk(psp2, [P, 2 * SQ], F32, "ps2")
                        for hh, h in enumerate(heads):
                            o = hh * SQ
                            nc.tensor.matmul(
                                ps_s[:, o + c0:o + SQ], kslice,
                                qth[h][q][:, c0:],
                                start=True, stop=not diag)
                            if diag:
                                # accumulate -30000 on the masked (k>q) part
                                nc.tensor.matmul(
                                    ps_s[:, o + c0:o + c0 + P], ident,
                                    mtri_sb, start=False, stop=True)
                        pr2 = _mk(prp, [P, 2 * SQ], BF16, "pr")
                        if c0 >= 2 * P:
                            # deep-diagonal: the pair-wide exp would cover
                            # c0 junk cols in the 2nd half; two trimmed
                            # per-head exps are cheaper (172-cyc overhead
                            # < c0 cols)
                            for o in (0, SQ):
                                nc.scalar.activation(
                                    pr2[:, o + c0:o + SQ],
                                    ps_s[:, o + c0:o + SQ],
                                    mybir.ActivationFunctionType.Exp,
                                    scale=SCALE)
                        else:
                            nc.scalar.activation(
                                pr2[:, c0:], ps_s[:, c0:],
                                mybir.ActivationFunctionType.Exp, scale=SCALE)
                        pend.append((jk, c0, pr2))
                        if len(pend) > 4:
                            pjk, pc0, ppr = pend.pop(0)
                            for hh, h in enumerate(heads):
                                o = hh * SQ
                                nc.tensor.matmul(
                                    ps_ctx[h][:, pc0:], vs[pjk],
                                    ppr[:, o + pc0:o + SQ],
                                    start=(pjk == 0), stop=False)
                    while pend:
                        pjk, pc0, ppr = pend.pop(0)
                        for hh, h in enumerate(heads):
                            o = hh * SQ
                            nc.tensor.matmul(
                                ps_ctx[h][:, pc0:], vs[pjk],
                                ppr[:, o + pc0:o + SQ],
                                start=(pjk == 0), stop=(not pend))
                    # per head: den recip (bf16) -> PE broadcast -> ctxg;
                    # hp0's blocks are deferred into hp1's loop so the den
                    # broadcast's recip-wait doesn't head-of-line block the
                    # next pass's scores on the in-order PE
                    def den_block(hp=hp, ps_ctx=ps_ctx, heads=heads):
                        for h in heads:
                            ho = (h % 2) * HD
                            denb = _mk(sp, [1, SQ], BF16, "denb")
                            with nc.allow_low_precision("softmax den bf16"):
                                nc.vector.reciprocal(denb,
                                                     ps_ctx[h][HD:HD + 1, :])
                            ps_db = _mk(psp, [HD, SQ], F32, "ps")
                            nc.tensor.matmul(ps_db, halfones[:, :HD], denb,
                                             start=True, stop=True)
                            # gud = (tanh(g/2) + 1) * (0.5/den)
                            gud = _mk(wp, [HD, SQ], BF16, "gud")
                            nc.vector.scalar_tensor_tensor(
                                gud, gus[hp][q][ho:ho + HD, :], 1.0, ps_db,
                                mybir.AluOpType.add, mybir.AluOpType.mult)
                            nc.vector.tensor_mul(
                                ctxgs[hp][q][ho:ho + HD, :],
                                ps_ctx[h][:HD, :], gud)
                    if hp == 0:
                        deferred.append(den_block)
                    else:
                        for g in deferred:
                            g()
                        den_block()
                for g in fqueue:
                    g()

            def f_groups(q, alt_copies=False):
                """Output projection for sq slice q as 16 deferred groups,
                interleaved one-per-jk-step into the next e_slice. The four
                n-slices of one s-row-block share an [128, 2048] staging
                tile and a single out-DMA (HWDGE holds ~625ns per dma_start
                regardless of size, so fewer+bigger DMAs win)."""
                groups = []
                for jj in range(4):
                    oc4_box = {}
                    for n in range(NSQ):
                        def emit(jj=jj, n=n, oc4_box=oc4_box):
                            jc = jj * P
                            j = 4 * q + jj
                            ps_o = _mk(psp, [P, SQ], F32, "ps")
                            for e in range(2):
                                nc.tensor.matmul(
                                    ps_o, ctxgs[e][q][:, jc:jc + P],
                                    wos[e][:, n * SQ:(n + 1) * SQ],
                                    start=(e == 0), stop=(e == 1))
                            if n == 0:
                                oc4_box["t"] = _mk(ocp, [P, 4 * SQ], BF16,
                                                   "oc4")
                            oc4 = oc4_box["t"]
                            dst = oc4[:, n * SQ:(n + 1) * SQ]
                            if alt_copies and n % 2 == 0:
                                nc.scalar.copy(dst, ps_o)
                            else:
                                nc.vector.tensor_copy(out=dst, in_=ps_o)
                            if n == NSQ - 1:
                                nc.sync.dma_start(
                                    out=out[j * P:(j + 1) * P, :], in_=oc4)
                        groups.append(emit)
                return groups

            # ---- schedule: gate sweeps fill the DMA window; the last 8
            # qkv groups and all out-projection groups are fed one-per-jk-
            # step into the attention slices, so the in-order PE program
            # stays dense from DMA arrival to the final output DMA ----
            _mark(nc, "phaseD0")
            g0 = d_sweep_mm(0, (0, 1))
            _mark(nc, "phaseA")
            for j in range(4):
                a_group(j)
            aux_dmas()
            xb_dmas()
            d_sweep_act(0, (0, 1), g0)
            _mark(nc, "phaseB")
            b_chain(0, 4)
            a_group(4)
            a_group(5)
            a_group(6)
            a_group(7)
            b_chain(4, 8)
            _mark(nc, "phaseC0")
            for j in range(4):
                c_tile(j, dve=(j % 2 == 1))
            ct47 = c_fillers(range(4, 8))
            _mark(nc, "phaseE0")
            # second gate sweep deferred into E0's fillers: its matmuls
            # competed with qkv groups 0-3 for PE during the DMA window;
            # its activation is only read at E0's pass-end den blocks
            g1box = {}
            e_slice(0, [lambda: g1box.__setitem__(
                            "g", d_sweep_mm(1, (0, 1))),
                        lambda: d_sweep_act(1, (0, 1), g1box["g"])]
                    + [lambda j=j: a_group(j, dve_stats=True)
                       for j in range(8, 12)] + ct47)
            b_chain(8, 12)
            ct811 = c_fillers(range(8, 12))
            _mark(nc, "phaseD1")
            d_sweep(0, (2, 3))
            _mark(nc, "phaseE1")
            e_slice(1, [lambda j=j: a_group(j, dve_stats=True)
                        for j in range(12, 16)]
                    + ct811 + f_groups(0, alt_copies=True))
            b_chain(12, 16)
            ct1215 = c_fillers(range(12, 16))
            _mark(nc, "phaseD2")
            d_sweep(1, (2, 3))
            _mark(nc, "phaseE2")
            e_slice(2, ct1215 + f_groups(1))
            _mark(nc, "phaseE3")
            e_slice(3, f_groups(2))
            for g in f_groups(3, alt_copies=True):
                g()

    nc.compile()
    return nc


def prep_inputs(x, cos, sin, Wq, Wk, Wv, Wo, q_norm_w, k_norm_w):
    """Host-side shard + layout prep. Returns per-core input maps."""
    xtn = x.reshape(S, D).T.astype(NBF)

    # rope tables with (1 + norm_w) folded in, k-block appended, and the
    # sin first-half pre-negated (so rope is out = q*cos5 + rot(q)*sin5
    # with rot(q) = [q2, q1])
    half = HD // 2
    wq1 = (1.0 + q_norm_w).astype(np.float32)
    wk1 = (1.0 + k_norm_w).astype(np.float32)

    def rotw(w):
        return np.concatenate([w[half:], w[:half]])

    sin_m = sin.copy()
    sin_m[:, :half] = -sin_m[:, :half]
    cos_q = cos * wq1
    cos_k = cos * wk1
    sin_q = sin_m * rotw(wq1)
    sin_k = sin_m * rotw(wk1)
    cos5 = np.concatenate([np.tile(cos_q, (1, NHL)), cos_k], axis=1)
    sin5 = np.concatenate([np.tile(sin_q, (1, NHL)), sin_k], axis=1)
    cs = np.ascontiguousarray(
        np.concatenate([cos5, sin5], axis=1)).astype(NBF)

    # strict lower triangle (k > q within the diagonal block) gets -30000,
    # accumulated into the scores psum before exp
    mtri = (np.tril(np.full((P, P), NEG, dtype=np.float32), k=-1)).astype(NBF)

    Wqh = Wq.reshape(H, 2 * HD, D)
    in_maps = []
    for c in range(NCORE):
        hs = slice(NHL * c, NHL * (c + 1))
        wq_c = Wqh[hs, :HD, :].reshape(EL, D)       # q rows, 4 heads
        wgt_c = Wqh[hs, HD:, :].reshape(EL, D)      # gate rows
        wk_c = Wk[HD * c:HD * (c + 1), :]
        wv_c = Wv[HD * c:HD * (c + 1), :]
        # [640, D]: q | k | v | gate
        wqg_c = np.concatenate([wq_c, wk_c, wv_c, wgt_c], axis=0)
        xw_c = np.ascontiguousarray(
            np.concatenate([wqg_c.T.astype(NBF), xtn], axis=1))
        in_maps.append({
            "xw": xw_c,
            "wo": np.ascontiguousarray(
                Wo[:, EL * c:EL * (c + 1)].T).astype(NBF),
            "cs": cs,
            "mtri": mtri,
        })
    return in_maps


_NC_CACHE = {}


def get_nc():
    if "nc" not in _NC_CACHE:
        _NC_CACHE["nc"] = build_nc()
    return _NC_CACHE["nc"]


def run(in_maps, trace=False, **kw):
    nc = get_nc()
    return run_bass_kernel_spmd(nc, in_maps, list(range(NCORE)),
                                trace=trace, **kw)


def kernel(x, mask, cos, sin, Wq, Wk, Wv, Wo, q_norm_w, k_norm_w):
    in_maps = prep_inputs(np.asarray(x, dtype=np.float32), np.asarray(cos),
                          np.asarray(sin), np.asarray(Wq), np.asarray(Wk),
                          np.asarray(Wv), np.asarray(Wo),
                          np.asarray(q_norm_w), np.asarray(k_norm_w))
    res = run(in_maps)
    acc = np.zeros((S, D), dtype=np.float32)
    for r in res.results:
        acc += np.asarray(r["out"], dtype=np.float32)
    return acc.reshape(1, S, D)
